# revision 22
# baseline (speedup 1.0000x reference)
"""2-layer GAT on 8 Trainium2 NeuronCores (Bass/Tile, SPMD) — v4.

Strategy (edge-parallel, dst-sharded): edges are sharded by destination-node
range (6250 nodes / core) and, per core, sorted by (src<32768 split,
dst-block-of-128, src). Node tables are bf16 [*, 128] rows (256B) with the
per-src attention logits folded into spare columns, so one SWDGE gather per
edge fetches everything src-side. Per 128-edge tile the kernel
  - builds one-hot [edge x dst] and its transpose [dst x edge] on DVE
    (is_equal vs iota consts; the dst ids arrive in both layouts, the
    [dst x edge] one via a partition-broadcast DMA),
  - gets per-edge dst logits ad_e with one small PE matmul (OHT^T @ ab_blk)
    against a per-block [128 x H] table (no per-edge dst gather),
  - computes w = exp(leakyrelu(as_e + ad_e)) on DVE/ACT (ACT writes w
    directly into the msg tile),
  - segment-sums [w*h | w] into PSUM via one PE matmul with the one-hot as
    stationary (replaces v1's dma_scatter_add RMW packets entirely).
Per-dst-block U/S accumulators live in SBUF; softmax is the U/S ratio so no
per-edge normalization. Layer boundaries fuse normalize+ELU+projection per
block. 4 SWDGE queues round-robin so descriptor generation overlaps DMA.
"""
import os
import numpy as np
import ml_dtypes

from concourse import bacc, mybir, tile
from concourse.bass_utils import run_bass_kernel_spmd

NCORES = 8
CH = 64          # feature channels (L2 zero-padded 40->64); table rows 128
TW = 128         # table row width (bf16 -> 256B rows)
GC = int(os.environ.get("GAT_GC", "1024"))  # idxs per SWDGE gather call
TPG = GC // 128  # tiles per gather group
SPLIT = 32768    # int16 gather index reach (rows)
NQ = 4           # SWDGE queues, round-robin over gather calls
F32 = mybir.dt.float32
BF16 = mybir.dt.bfloat16
I16 = mybir.dt.int16
AL = mybir.AluOpType
AF = mybir.ActivationFunctionType

_prog_cache = {}
LAST_RESULTS = None  # BassKernelResults of the last device run (for test.py)


def _build(meta):
    NLOCP = meta["NLOCP"]
    NB = meta["NB"]
    NPAD = NCORES * NLOCP
    NGl, NGh = meta["NGl"], meta["NGh"]
    tiles_l, tiles_h = meta["tiles_l"], meta["tiles_h"]  # [(blk, first, last)]
    SL, SH = NGl * GC, NGh * GC

    nc = bacc.Bacc(num_devices=NCORES, num_swdge_queues=NQ,
                   dynamic_dma_scratch_size=16 * GC)

    # ---- I/O ----
    xTs = nc.dram_tensor("xTs", [128, NLOCP], F32, kind="ExternalInput")
    W1 = nc.dram_tensor("W1", [128, CH], F32, kind="ExternalInput")
    W2p = nc.dram_tensor("W2p", [CH, CH], F32, kind="ExternalInput")
    IOTAB = nc.dram_tensor("IOTAB", [128, 128], BF16, kind="ExternalInput")
    IOTAPB = nc.dram_tensor("IOTAPB", [128, 1], BF16, kind="ExternalInput")
    IDN = nc.dram_tensor("IDN", [128, 128], F32, kind="ExternalInput")
    asrc1t = nc.dram_tensor("asrc1t", [128, CH], F32, kind="ExternalInput")
    adst1r = nc.dram_tensor("adst1r", [128, CH], F32, kind="ExternalInput")
    b1r = nc.dram_tensor("b1r", [128, CH], F32, kind="ExternalInput")
    b2r = nc.dram_tensor("b2r", [128, 40], F32, kind="ExternalInput")
    gl = nc.dram_tensor("gl", [128, SL // 16], I16, kind="ExternalInput")
    gh = nc.dram_tensor("gh", [128, SH // 16], I16, kind="ExternalInput")
    dTl = nc.dram_tensor("dTl", [128, SL // 128], BF16, kind="ExternalInput")
    dTh = nc.dram_tensor("dTh", [128, SH // 128], BF16, kind="ExternalInput")
    dRl = nc.dram_tensor("dRl", [1, SL], BF16, kind="ExternalInput")
    dRh = nc.dram_tensor("dRh", [1, SH], BF16, kind="ExternalInput")
    OUT = nc.dram_tensor("OUT", [NLOCP, 40], F32, kind="ExternalOutput")

    # ---- scratch ----
    h1loc = nc.dram_tensor("h1loc", [NLOCP, TW], BF16, kind="Internal")
    h1full = nc.dram_tensor("h1full", [NPAD, TW], BF16, kind="Internal",
                            addr_space="Shared")
    ab1 = nc.dram_tensor("ab1", [NLOCP, 4], BF16, kind="Internal")
    h2loc = nc.dram_tensor("h2loc", [NLOCP, TW], BF16, kind="Internal")
    h2full = nc.dram_tensor("h2full", [NPAD, TW], BF16, kind="Internal",
                            addr_space="Shared")
    ab2 = nc.dram_tensor("ab2", [NLOCP, 1], BF16, kind="Internal")

    debug = bool(os.environ.get("GAT_DEBUG"))
    if debug:
        Dh1 = nc.dram_tensor("Dh1", [NLOCP, TW], BF16, kind="ExternalOutput")
        DU1 = nc.dram_tensor("DU1", [NLOCP, 68], F32, kind="ExternalOutput")
        Dh2 = nc.dram_tensor("Dh2", [NLOCP, TW], BF16, kind="ExternalOutput")
        DU2 = nc.dram_tensor("DU2", [NLOCP, 41], F32, kind="ExternalOutput")

    groups = [list(range(NCORES))]

    with tile.TileContext(nc) as tc:
        with (
            tc.tile_pool(name="const", bufs=1) as cpool,
            tc.tile_pool(name="dense", bufs=3) as dpool,
            tc.tile_pool(name="dpsum", bufs=2, space="PSUM") as dps,
            tc.tile_pool(name="apsum", bufs=2, space="PSUM") as aps,
            tc.tile_pool(name="upsum", bufs=2, space="PSUM") as ups,
            tc.tile_pool(name="idx", bufs=3) as ipool,
            tc.tile_pool(name="edge", bufs=3) as epool,
            tc.tile_pool(name="onehot", bufs=3) as opool,
            tc.tile_pool(name="msg", bufs=3) as mpool,
            tc.tile_pool(name="small", bufs=3) as spool,
            tc.tile_pool(name="ab", bufs=2) as abpool,
        ):
            # constants
            w1sb = cpool.tile([128, CH], F32)
            nc.sync.dma_start(w1sb[:], W1[:])
            w2sb = cpool.tile([CH, CH], F32)
            nc.sync.dma_start(w2sb[:], W2p[:])
            iotab = cpool.tile([128, 128], BF16)
            nc.sync.dma_start(iotab[:], IOTAB[:])
            iotapb = cpool.tile([128, 1], BF16)
            nc.sync.dma_start(iotapb[:], IOTAPB[:])
            idn = cpool.tile([128, 128], F32)
            nc.sync.dma_start(idn[:], IDN[:])
            as1sb = cpool.tile([128, CH], F32)
            nc.sync.dma_start(as1sb[:], asrc1t[:])
            ad1sb = cpool.tile([128, CH], F32)
            nc.sync.dma_start(ad1sb[:], adst1r[:])
            b1sb = cpool.tile([128, CH], F32)
            nc.sync.dma_start(b1sb[:], b1r[:])
            b2sb = cpool.tile([128, 40], F32)
            nc.sync.dma_start(b2sb[:], b2r[:])

            # persistent per-block U/S accumulators (SBUF)
            U1 = cpool.tile([128, NB, 68], F32)
            U2 = cpool.tile([128, NB, 41], F32)

            # ---- D1: h1 = x @ W1; table row = [h1 | h1.a_src1 | 0]; ab1 ----
            for i in range(NB):
                r0 = i * 128
                xt = dpool.tile([128, 128], F32, tag="xt")
                nc.sync.dma_start(xt[:], xTs[:, r0:r0 + 128])
                ps = dps.tile([128, CH], F32, tag="mm")
                nc.tensor.matmul(ps[:], xt[:], w1sb[:])
                ht = dpool.tile([128, CH], F32, tag="ht")
                nc.vector.tensor_copy(ht[:], ps[:])
                htb = dpool.tile([128, TW], BF16, tag="htb")
                nc.vector.tensor_copy(htb[:, 0:CH], ht[:])
                tmp = dpool.tile([128, CH], F32, tag="tmp")
                nc.vector.tensor_mul(tmp[:], ht[:], as1sb[:])
                asf = spool.tile([128, 4], F32, tag="asf")
                nc.vector.tensor_reduce(
                    asf[:], tmp[:].rearrange("p (h c) -> p h c", c=16),
                    mybir.AxisListType.X, AL.add)
                nc.vector.tensor_copy(htb[:, CH:CH + 4], asf[:])
                nc.vector.memset(htb[:, CH + 4:TW], 0.0)
                nc.sync.dma_start(h1loc[r0:r0 + 128, :], htb[:])
                tmp2 = dpool.tile([128, CH], F32, tag="tmp2")
                nc.vector.tensor_mul(tmp2[:], ht[:], ad1sb[:])
                dpf = spool.tile([128, 4], F32, tag="dpf")
                nc.vector.tensor_reduce(
                    dpf[:], tmp2[:].rearrange("p (h c) -> p h c", c=16),
                    mybir.AxisListType.X, AL.add)
                dp = spool.tile([128, 4], BF16, tag="dp")
                nc.vector.tensor_copy(dp[:], dpf[:])
                nc.sync.dma_start(ab1[r0:r0 + 128, :], dp[:])

            # ---- AllGather h1 ----
            nc.gpsimd.collective_compute(
                "AllGather", AL.bypass, groups, [h1loc[:, :]], [h1full[:, :]])

            def edge_pass(pass_id, tiles, NG, gidx, dT, dR, base, abt, U, H,
                          UW, as_col, first_pass_for_blk, qoff):
                """One lo/hi pass over all dst blocks of one layer.
                tiles: [(blk, seg_first, seg_last)] per tile slot.
                base: gather base AP; abt: [NLOCP, H] bf16 per-dst logits.
                U: [128, NB, UW] f32 SBUF accumulator. as_col: first table
                column holding the H per-src logits."""
                cur_ab = {}
                U_ps = None
                for g in range(NG):
                    it = ipool.tile([128, GC // 16], I16, tag="it")
                    nc.sync.dma_start(it[:], gidx[:, g * (GC // 16):(g + 1) * (GC // 16)])
                    dt = ipool.tile([128, TPG], BF16, tag="dt")
                    nc.sync.dma_start(dt[:], dT[:, g * TPG:(g + 1) * TPG])
                    # dst ids replicated to all partitions ([d, e] layout)
                    db = ipool.tile([128, GC], BF16, tag="db")
                    nc.sync.dma_start(
                        db[:],
                        dR[0:1, g * GC:(g + 1) * GC].broadcast_to([128, GC]))
                    G = epool.tile([128, TPG, TW], BF16, tag="G")
                    nc.gpsimd.dma_gather(
                        G[:], base, it[:], GC, GC, TW,
                        queue_num=(g + qoff) % NQ)

                    # one-hot [e, d] and transposed [d, e] (pads match nothing)
                    OH = opool.tile([128, TPG, 128], BF16, tag="OH")
                    nc.vector.tensor_tensor(
                        OH[:],
                        iotab[:].unsqueeze(1).broadcast_to([128, TPG, 128]),
                        dt[:].unsqueeze(2).broadcast_to([128, TPG, 128]),
                        AL.is_equal)
                    OHT = opool.tile([128, TPG, 128], BF16, tag="OHT")
                    nc.vector.tensor_tensor(
                        OHT[:],
                        db[:].rearrange("p (t e) -> p t e", e=128),
                        iotapb[:].unsqueeze(2).broadcast_to([128, TPG, 128]),
                        AL.is_equal)

                    # ad_e = OHT^T @ ab_blk
                    ad_ps = aps.tile([128, TPG, H], F32, tag="ad")
                    for t in range(TPG):
                        blk = tiles[g * TPG + t][0]
                        if blk not in cur_ab:
                            abk = abpool.tile([128, H], BF16, tag="abk")
                            nc.sync.dma_start(
                                abk[:], abt[blk * 128:(blk + 1) * 128, :])
                            cur_ab = {blk: abk}
                        nc.tensor.matmul(
                            ad_ps[:, t:t + 1, :].rearrange("p a b -> p (a b)"),
                            OHT[:, t:t + 1, :].rearrange("p a b -> p (a b)"),
                            cur_ab[blk][:])

                    # w = exp(leakyrelu(as_e + ad_e)); ACT writes w into msg
                    e = spool.tile([128, TPG, H], F32, tag="e")
                    nc.vector.tensor_add(
                        e[:], G[:, :, as_col:as_col + H], ad_ps[:])
                    lr = spool.tile([128, TPG * H], F32, tag="lr")
                    nc.vector.scalar_tensor_tensor(
                        lr[:], e[:].rearrange("p a b -> p (a b)"), 0.2,
                        e[:].rearrange("p a b -> p (a b)"), AL.mult, AL.max)
                    wt = spool.tile([128, TPG * H], BF16, tag="wt")
                    nc.scalar.activation(wt[:], lr[:], AF.Exp)
                    wb = wt[:].rearrange("p (t h) -> p t h", h=H)
                    msg = mpool.tile([128, TPG, UW], BF16, tag="msg")
                    nc.scalar.activation(
                        msg[:, :, UW - H:UW], lr[:].rearrange(
                            "p (t h) -> p t h", h=H), AF.Exp)
                    if H > 1:
                        nc.vector.tensor_mul(
                            msg[:, :, 0:CH].rearrange("p t (h c) -> p t h c", c=CH // H),
                            G[:, :, 0:CH].rearrange("p t (h c) -> p t h c", c=CH // H),
                            wb.unsqueeze(3).broadcast_to([128, TPG, H, CH // H]))
                    else:
                        nc.vector.tensor_mul(
                            msg[:, :, 0:UW - 1], G[:, :, 0:UW - 1],
                            wb.broadcast_to([128, TPG, UW - 1]))

                    # segment-sum via PE: U_ps[d, :] += OH^T @ msg
                    for t in range(TPG):
                        blk, sfirst, slast = tiles[g * TPG + t]
                        if sfirst:
                            U_ps = ups.tile([128, UW], F32, tag="ups")
                        nc.tensor.matmul(
                            U_ps[:],
                            OH[:, t:t + 1, :].rearrange("p a b -> p (a b)"),
                            msg[:, t:t + 1, :].rearrange("p a b -> p (a b)"),
                            start=sfirst, stop=slast)
                        if slast:
                            if first_pass_for_blk[blk] == pass_id:
                                nc.vector.tensor_copy(U[:, blk, :], U_ps[:])
                            else:
                                nc.vector.tensor_add(
                                    U[:, blk, :], U[:, blk, :], U_ps[:])
                return NG

            # ---- E1 ----
            fp1 = meta["first_pass_for_blk"]
            edge_pass(0, tiles_l, NGl, gl, dTl, dRl, h1full[0:SPLIT, :], ab1,
                      U1, 4, 68, CH, fp1, 0)
            edge_pass(1, tiles_h, NGh, gh, dTh, dRh, h1full[SPLIT:NPAD, :],
                      ab1, U1, 4, 68, CH, fp1, NGl)

            # ---- F1: z = U/S + b1; ELU; h2 = mid @ W2p; table + ab2 ----
            for i in range(NB):
                r0 = i * 128
                ut = U1[:, i, :]
                sp = spool.tile([128, 4], F32, tag="sp")
                nc.vector.tensor_scalar(out=sp[:], in0=ut[:, 64:68],
                                        scalar1=1e-16, scalar2=None, op0=AL.add)
                rec = spool.tile([128, 4], F32, tag="rec")
                nc.vector.reciprocal(rec[:], sp[:])
                z = dpool.tile([128, CH], F32, tag="z")
                for h in range(4):
                    nc.vector.scalar_tensor_tensor(
                        z[:, 16 * h:16 * h + 16], ut[:, 16 * h:16 * h + 16],
                        rec[:, h:h + 1], b1sb[:, 16 * h:16 * h + 16],
                        AL.mult, AL.add)
                # ELU(z) = relu(z) + exp(min(z,0)) - 1
                r = dpool.tile([128, CH], F32, tag="r")
                nc.scalar.activation(r[:], z[:], AF.Relu)
                u = dpool.tile([128, CH], F32, tag="u")
                nc.vector.tensor_scalar(out=u[:], in0=z[:], scalar1=0.0,
                                        scalar2=None, op0=AL.min)
                tE = dpool.tile([128, CH], F32, tag="tE")
                nc.scalar.activation(tE[:], u[:], AF.Exp)
                mid = dpool.tile([128, CH], F32, tag="mid")
                nc.vector.scalar_tensor_tensor(
                    mid[:], tE[:], -1.0, r[:], AL.add, AL.add)
                tp2 = dps.tile([CH, 128], F32, tag="tp2")
                nc.tensor.transpose(tp2[:], mid[:, 0:CH], idn[:])
                tps2 = dpool.tile([CH, 128], F32, tag="tps2")
                nc.vector.tensor_copy(tps2[:], tp2[:])
                ps2 = dps.tile([128, CH], F32, tag="mm")
                nc.tensor.matmul(ps2[:], tps2[:], w2sb[:])
                h2b = dpool.tile([128, TW], BF16, tag="h2b")
                nc.vector.tensor_copy(h2b[:, 0:CH], ps2[:])
                nc.vector.memset(h2b[:, CH:TW], 0.0)
                nc.sync.dma_start(h2loc[r0:r0 + 128, :], h2b[:])
                ab2t = spool.tile([128, 1], BF16, tag="ab2t")
                nc.vector.tensor_copy(ab2t[:], h2b[:, 41:42])
                nc.sync.dma_start(ab2[r0:r0 + 128, :], ab2t[:])

            # ---- AllGather h2 ----
            nc.gpsimd.collective_compute(
                "AllGather", AL.bypass, groups, [h2loc[:, :]], [h2full[:, :]])

            # ---- E2 (same edge order; as/ad folded into table cols 40/41) ----
            edge_pass(0, tiles_l, NGl, gl, dTl, dRl, h2full[0:SPLIT, :], ab2,
                      U2, 1, 41, 40, fp1, 0)
            edge_pass(1, tiles_h, NGh, gh, dTh, dRh, h2full[SPLIT:NPAD, :],
                      ab2, U2, 1, 41, 40, fp1, NGl)

            # ---- F2: out = U2/S2 + b2 ----
            for i in range(NB):
                r0 = i * 128
                ut = U2[:, i, :]
                sp = spool.tile([128, 1], F32, tag="sp2")
                nc.vector.tensor_scalar(out=sp[:], in0=ut[:, 40:41],
                                        scalar1=1e-16, scalar2=None, op0=AL.add)
                rec = spool.tile([128, 1], F32, tag="rec2")
                nc.vector.reciprocal(rec[:], sp[:])
                ot = dpool.tile([128, 40], F32, tag="ot")
                nc.vector.scalar_tensor_tensor(
                    ot[:], ut[:, 0:40], rec[:, 0:1], b2sb[:], AL.mult, AL.add)
                nc.sync.dma_start(OUT[r0:r0 + 128, :], ot[:])

            if debug:
                for i in range(NB):
                    r0 = i * 128
                    for src_d, dst_d in ((h1loc, Dh1), (h2loc, Dh2)):
                        tt = dpool.tile([128, TW], BF16, tag="dbg")
                        nc.sync.dma_start(tt[:], src_d[r0:r0 + 128, :])
                        nc.sync.dma_start(dst_d[r0:r0 + 128, :], tt[:])
                    du1 = dpool.tile([128, 68], F32, tag="du1")
                    nc.vector.tensor_copy(du1[:], U1[:, i, :])
                    nc.sync.dma_start(DU1[r0:r0 + 128, :], du1[:])
                    du2 = dpool.tile([128, 41], F32, tag="du2")
                    nc.vector.tensor_copy(du2[:], U2[:, i, :])
                    nc.sync.dma_start(DU2[r0:r0 + 128, :], du2[:])

    nc.finalize()
    return nc


def _wrap_idx(a):
    """int16 [cap] -> wrapped [16, cap/16] replicated to [128, cap/16]."""
    w = a.reshape(-1, 16).T.copy()
    return np.ascontiguousarray(np.tile(w, (8, 1)))


def prep(x, edge_index, W1, a_src1, a_dst1, b1, W2, a_src2, a_dst2, b2):
    """Host-side sharding/index prep. Returns (meta, in_maps, (N, FIN))."""
    x = np.asarray(x, np.float32)
    N, FIN = x.shape
    NLOC = (N + NCORES - 1) // NCORES                       # 6250
    NLOCP = ((NLOC + 127) // 128) * 128                     # 6272
    NB = NLOCP // 128                                       # 49
    NPAD = NCORES * NLOCP

    ei0 = np.asarray(edge_index[0]).astype(np.int64)
    ei1 = np.asarray(edge_index[1]).astype(np.int64)
    loops = np.arange(N, dtype=np.int64)
    src = np.concatenate([ei0, loops])
    dst = np.concatenate([ei1, loops])

    rmap = (src // NLOC) * NLOCP + (src % NLOC)             # table row of src
    core = dst // NLOC
    dloc = dst % NLOC

    # per (core, pass): edges sorted by (dst-block, src-row)
    per = []  # [core][pass] = (rs_sorted, blk_sorted, dlm_sorted)
    for c in range(NCORES):
        m = core == c
        rs_c, dl_c = rmap[m], dloc[m]
        blk_c, dlm_c = dl_c // 128, dl_c % 128
        lo = rs_c < SPLIT
        rows = []
        for p, sel in enumerate((lo, ~lo)):
            rs, blk, dlm = rs_c[sel], blk_c[sel], dlm_c[sel]
            o = np.lexsort((rs, blk))
            rows.append((rs[o] - (SPLIT if p else 0), blk[o], dlm[o]))
        per.append(rows)

    # uniform tiles-per-(pass, block) across cores
    TPB = np.zeros((2, NB), np.int64)
    for c in range(NCORES):
        for p in range(2):
            cnt = np.bincount(per[c][p][1], minlength=NB)
            TPB[p] = np.maximum(TPB[p], (cnt + 127) // 128)
    first_pass_for_blk = np.where(TPB[0] > 0, 0, 1).tolist()

    def pass_meta(p):
        bids = np.repeat(np.arange(NB), TPB[p])
        NG = (len(bids) + TPG - 1) // TPG
        pad = NG * TPG - len(bids)
        if pad:
            bids = np.concatenate([bids, np.full(pad, bids[-1])])
        first = np.ones(len(bids), bool)
        first[1:] = bids[1:] != bids[:-1]
        last = np.ones(len(bids), bool)
        last[:-1] = bids[1:] != bids[:-1]
        return NG, list(zip(bids.tolist(), first.tolist(), last.tolist()))

    NGl, tiles_l = pass_meta(0)
    NGh, tiles_h = pass_meta(1)
    slot_base = [np.concatenate([[0], np.cumsum(TPB[p]) * 128]) for p in range(2)]

    # ---- constant inputs (replicated) ----
    W1 = np.asarray(W1, np.float32)
    W2p = np.zeros((CH, CH), np.float32)
    W2p[:, :40] = np.asarray(W2, np.float32)
    W2p[:, 40] = W2p[:, :40] @ np.asarray(a_src2, np.float32).reshape(40)
    W2p[:, 41] = W2p[:, :40] @ np.asarray(a_dst2, np.float32).reshape(40)
    IOTAB = np.ascontiguousarray(np.tile(
        np.arange(128, dtype=np.float32)[None, :], (128, 1))).astype(
            ml_dtypes.bfloat16)
    IOTAPB = np.arange(128, dtype=np.float32)[:, None].astype(
        ml_dtypes.bfloat16)
    IDN = np.eye(128, dtype=np.float32)
    as1 = np.asarray(a_src1, np.float32).reshape(CH)
    ad1 = np.asarray(a_dst1, np.float32).reshape(CH)
    asrc1t = np.ascontiguousarray(np.tile(as1[None, :], (128, 1)))
    adst1r = np.ascontiguousarray(np.tile(ad1[None, :], (128, 1)))
    b1r = np.ascontiguousarray(
        np.tile(np.asarray(b1, np.float32)[None, :], (128, 1)))
    b2r = np.ascontiguousarray(
        np.tile(np.asarray(b2, np.float32)[None, :], (128, 1)))

    xpad = np.zeros((NPAD, FIN), np.float32)
    for c in range(NCORES):
        n0 = c * NLOC
        take = min(NLOCP, N - n0)
        xpad[c * NLOCP:c * NLOCP + take] = x[n0:n0 + take]

    in_maps = []
    for c in range(NCORES):
        packs = []
        for p, NG in ((0, NGl), (1, NGh)):
            slots = NG * GC
            idx_arr = np.zeros(slots, np.int64)
            dloc_arr = np.full(slots, 999.0, np.float32)
            rs, blk, dlm = per[c][p]
            if len(blk):
                starts = np.concatenate([[0], np.cumsum(np.bincount(blk, minlength=NB))])
                rank = np.arange(len(blk)) - starts[blk]
                pos = slot_base[p][blk] + rank
                idx_arr[pos] = rs
                dloc_arr[pos] = dlm.astype(np.float32)
            packs.append((
                _wrap_idx(idx_arr.astype(np.int16)),
                np.ascontiguousarray(
                    dloc_arr.reshape(-1, 128).T).astype(ml_dtypes.bfloat16),
                dloc_arr[None, :].astype(ml_dtypes.bfloat16)))
        xT = np.ascontiguousarray(xpad[c * NLOCP:(c + 1) * NLOCP].T)
        in_maps.append({
            "xTs": xT, "W1": W1, "W2p": W2p, "IOTAB": IOTAB, "IOTAPB": IOTAPB,
            "IDN": IDN,
            "asrc1t": asrc1t, "adst1r": adst1r, "b1r": b1r, "b2r": b2r,
            "gl": packs[0][0], "dTl": packs[0][1], "dRl": packs[0][2],
            "gh": packs[1][0], "dTh": packs[1][1], "dRh": packs[1][2],
        })

    meta = {
        "NLOC": NLOC, "NLOCP": NLOCP, "NB": NB,
        "NGl": NGl, "NGh": NGh, "tiles_l": tiles_l, "tiles_h": tiles_h,
        "first_pass_for_blk": first_pass_for_blk,
    }
    return meta, in_maps, (N, FIN)


def kernel(**inputs):
    global LAST_RESULTS
    meta, in_maps, (N, FIN) = prep(**inputs)
    NLOC = meta["NLOC"]
    key = (N, FIN, meta["NGl"], meta["NGh"],
           tuple(t[0] for t in meta["tiles_l"]),
           tuple(t[0] for t in meta["tiles_h"]))
    if key not in _prog_cache:
        _prog_cache[key] = _build(meta)
    nc = _prog_cache[key]

    want_trace = bool(os.environ.get("GAT_TRACE"))
    if want_trace:
        try:
            from antenv import axon_hooks  # noqa: F401
        except ImportError:
            want_trace = False
    res = run_bass_kernel_spmd(
        nc, in_maps, core_ids=list(range(NCORES)), trace=want_trace)
    LAST_RESULTS = res
    out = np.empty((N, 40), np.float32)
    for c in range(NCORES):
        n0 = c * NLOC
        take = min(NLOC, N - n0)
        out[n0:n0 + take] = res.results[c]["OUT"][:take]
    return out


# revision 23
# speedup vs baseline: 1.2350x; 1.2350x over previous
"""2-layer GAT on 8 Trainium2 NeuronCores (Bass/Tile, SPMD) — v5.

Strategy (edge-parallel, dst-sharded): destination nodes are bin-packed on
the host into 392 (core, block-of-128) bins balanced by in-degree, so every
block sees ~equal edge work (minimal tile padding, SPMD-uniform program).
Per core, edges are sorted by (src<32768 split, dst-block, src). Node tables
are bf16 [*, 128] rows (256B) with per-src attention logits folded into
spare columns, so one SWDGE gather per edge fetches everything src-side.
Per 128-edge tile the kernel
  - builds one-hot [edge x dst] and its transpose [dst x edge] on DVE
    (is_equal vs iota consts; dst ids arrive in both layouts, the [dst x
    edge] one via a partition-broadcast DMA),
  - gets per-edge dst logits ad_e with one small PE matmul (OHT^T @ ab_blk),
  - computes w = exp(leakyrelu(as_e + ad_e)) on DVE/ACT (ACT writes w
    straight into the msg tile),
  - segment-sums [w*h | w] into PSUM via one PE matmul with the one-hot as
    stationary (no dma_scatter_add RMW).
Self-loop edges never enter the edge stream: their contribution initializes
the per-block U/S SBUF accumulators element-wise during the dense phases.
Softmax is the U/S ratio so no per-edge normalization. Layer boundaries
fuse normalize+ELU+projection per block, interleaved into the tail edge
pass so the PE/DVE work overlaps the remaining gathers. 4 SWDGE queues
round-robin so descriptor generation overlaps DMA.
"""
import heapq
import os
import numpy as np
import ml_dtypes

from concourse import bacc, mybir, tile
from concourse.bass_utils import run_bass_kernel_spmd

NCORES = 8
CH = 64          # feature channels (L2 zero-padded 40->64); table rows 128
TW = 128         # table row width (bf16 -> 256B rows)
GC = int(os.environ.get("GAT_GC", "1024"))  # idxs per SWDGE gather call
TPG = GC // 128  # tiles per gather group
SPLIT = 32768    # int16 gather index reach (rows)
NQ = 4           # SWDGE queues, round-robin over gather calls
F32 = mybir.dt.float32
BF16 = mybir.dt.bfloat16
I16 = mybir.dt.int16
AL = mybir.AluOpType
AF = mybir.ActivationFunctionType

_prog_cache = {}
LAST_RESULTS = None  # BassKernelResults of the last device run (for test.py)


def _build(meta):
    NLOCP = meta["NLOCP"]
    NB = meta["NB"]
    NPAD = NCORES * NLOCP
    NGl, NGh = meta["NGl"], meta["NGh"]
    tiles_l, tiles_h = meta["tiles_l"], meta["tiles_h"]  # [(blk, first, last)]
    last_pass = meta["last_pass_for_blk"]
    SL, SH = NGl * GC, NGh * GC

    nc = bacc.Bacc(num_devices=NCORES, num_swdge_queues=NQ,
                   dynamic_dma_scratch_size=16 * GC)

    # ---- I/O ----
    xTs = nc.dram_tensor("xTs", [128, NLOCP], F32, kind="ExternalInput")
    W1 = nc.dram_tensor("W1", [128, CH], F32, kind="ExternalInput")
    W2p = nc.dram_tensor("W2p", [CH, CH], F32, kind="ExternalInput")
    IOTAB = nc.dram_tensor("IOTAB", [128, 128], BF16, kind="ExternalInput")
    IOTAPB = nc.dram_tensor("IOTAPB", [128, 1], BF16, kind="ExternalInput")
    IDN = nc.dram_tensor("IDN", [128, 128], F32, kind="ExternalInput")
    asrc1t = nc.dram_tensor("asrc1t", [128, CH], F32, kind="ExternalInput")
    adst1r = nc.dram_tensor("adst1r", [128, CH], F32, kind="ExternalInput")
    b1r = nc.dram_tensor("b1r", [128, CH], F32, kind="ExternalInput")
    b2r = nc.dram_tensor("b2r", [128, 40], F32, kind="ExternalInput")
    gl = nc.dram_tensor("gl", [128, SL // 16], I16, kind="ExternalInput")
    gh = nc.dram_tensor("gh", [128, SH // 16], I16, kind="ExternalInput")
    dTl = nc.dram_tensor("dTl", [128, SL // 128], BF16, kind="ExternalInput")
    dTh = nc.dram_tensor("dTh", [128, SH // 128], BF16, kind="ExternalInput")
    dRl = nc.dram_tensor("dRl", [1, SL], BF16, kind="ExternalInput")
    dRh = nc.dram_tensor("dRh", [1, SH], BF16, kind="ExternalInput")
    OUT = nc.dram_tensor("OUT", [NLOCP, 40], F32, kind="ExternalOutput")

    # ---- scratch ----
    h1loc = nc.dram_tensor("h1loc", [NLOCP, TW], BF16, kind="Internal")
    h1full = nc.dram_tensor("h1full", [NPAD, TW], BF16, kind="Internal",
                            addr_space="Shared")
    ab1 = nc.dram_tensor("ab1", [NLOCP, 4], BF16, kind="Internal")
    h2loc = nc.dram_tensor("h2loc", [NLOCP, TW], BF16, kind="Internal")
    h2full = nc.dram_tensor("h2full", [NPAD, TW], BF16, kind="Internal",
                            addr_space="Shared")
    ab2 = nc.dram_tensor("ab2", [NLOCP, 1], BF16, kind="Internal")

    debug = bool(os.environ.get("GAT_DEBUG"))
    if debug:
        Dh1 = nc.dram_tensor("Dh1", [NLOCP, TW], BF16, kind="ExternalOutput")
        DU1 = nc.dram_tensor("DU1", [NLOCP, 68], F32, kind="ExternalOutput")
        Dh2 = nc.dram_tensor("Dh2", [NLOCP, TW], BF16, kind="ExternalOutput")
        DU2 = nc.dram_tensor("DU2", [NLOCP, 41], F32, kind="ExternalOutput")

    groups = [list(range(NCORES))]

    with tile.TileContext(nc) as tc:
        with (
            tc.tile_pool(name="const", bufs=1) as cpool,
            tc.tile_pool(name="dense", bufs=3) as dpool,
            tc.tile_pool(name="dpsum", bufs=2, space="PSUM") as dps,
            tc.tile_pool(name="apsum", bufs=2, space="PSUM") as aps,
            tc.tile_pool(name="upsum", bufs=2, space="PSUM") as ups,
            tc.tile_pool(name="idx", bufs=3) as ipool,
            tc.tile_pool(name="edge", bufs=3) as epool,
            tc.tile_pool(name="onehot", bufs=3) as opool,
            tc.tile_pool(name="msg", bufs=3) as mpool,
            tc.tile_pool(name="small", bufs=3) as spool,
            tc.tile_pool(name="ab", bufs=2) as abpool,
        ):
            # constants
            w1sb = cpool.tile([128, CH], F32)
            nc.sync.dma_start(w1sb[:], W1[:])
            w2sb = cpool.tile([CH, CH], F32)
            nc.sync.dma_start(w2sb[:], W2p[:])
            iotab = cpool.tile([128, 128], BF16)
            nc.sync.dma_start(iotab[:], IOTAB[:])
            iotapb = cpool.tile([128, 1], BF16)
            nc.sync.dma_start(iotapb[:], IOTAPB[:])
            idn = cpool.tile([128, 128], F32)
            nc.sync.dma_start(idn[:], IDN[:])
            as1sb = cpool.tile([128, CH], F32)
            nc.sync.dma_start(as1sb[:], asrc1t[:])
            ad1sb = cpool.tile([128, CH], F32)
            nc.sync.dma_start(ad1sb[:], adst1r[:])
            b1sb = cpool.tile([128, CH], F32)
            nc.sync.dma_start(b1sb[:], b1r[:])
            b2sb = cpool.tile([128, 40], F32)
            nc.sync.dma_start(b2sb[:], b2r[:])

            # persistent per-block U/S accumulators (SBUF)
            U1 = cpool.tile([128, NB, 68], F32)
            U2 = cpool.tile([128, NB, 41], F32)

            # ---- D1: h1 = x @ W1; table row = [h1 | h1.a_src1 | 0]; ab1;
            #      U1[blk] initialized with the self-loop contribution ----
            for i in range(NB):
                r0 = i * 128
                xt = dpool.tile([128, 128], F32, tag="xt")
                nc.sync.dma_start(xt[:], xTs[:, r0:r0 + 128])
                ps = dps.tile([128, CH], F32, tag="mm")
                nc.tensor.matmul(ps[:], xt[:], w1sb[:])
                ht = dpool.tile([128, CH], F32, tag="ht")
                nc.vector.tensor_copy(ht[:], ps[:])
                htb = dpool.tile([128, TW], BF16, tag="htb")
                nc.vector.tensor_copy(htb[:, 0:CH], ht[:])
                tmp = dpool.tile([128, CH], F32, tag="tmp")
                nc.vector.tensor_mul(tmp[:], ht[:], as1sb[:])
                asf = spool.tile([128, 4], F32, tag="asf")
                nc.vector.tensor_reduce(
                    asf[:], tmp[:].rearrange("p (h c) -> p h c", c=16),
                    mybir.AxisListType.X, AL.add)
                nc.vector.tensor_copy(htb[:, CH:CH + 4], asf[:])
                nc.vector.memset(htb[:, CH + 4:TW], 0.0)
                nc.sync.dma_start(h1loc[r0:r0 + 128, :], htb[:])
                tmp2 = dpool.tile([128, CH], F32, tag="tmp2")
                nc.vector.tensor_mul(tmp2[:], ht[:], ad1sb[:])
                dpf = spool.tile([128, 4], F32, tag="dpf")
                nc.vector.tensor_reduce(
                    dpf[:], tmp2[:].rearrange("p (h c) -> p h c", c=16),
                    mybir.AxisListType.X, AL.add)
                dp = spool.tile([128, 4], BF16, tag="dp")
                nc.vector.tensor_copy(dp[:], dpf[:])
                nc.sync.dma_start(ab1[r0:r0 + 128, :], dp[:])
                # self-loop: U1[blk] = [w*h | w], w = exp(lrelu(as+ad))
                e0 = spool.tile([128, 4], F32, tag="e0")
                nc.vector.tensor_add(e0[:], asf[:], dpf[:])
                lr0 = spool.tile([128, 4], F32, tag="lr0")
                nc.vector.scalar_tensor_tensor(
                    lr0[:], e0[:], 0.2, e0[:], AL.mult, AL.max)
                w0 = spool.tile([128, 4], F32, tag="w0")
                nc.scalar.activation(w0[:], lr0[:], AF.Exp)
                nc.vector.tensor_mul(
                    U1[:, i, 0:CH].rearrange("p (h c) -> p h c", c=16),
                    ht[:].rearrange("p (h c) -> p h c", c=16),
                    w0[:].unsqueeze(2).broadcast_to([128, 4, 16]))
                nc.vector.tensor_copy(U1[:, i, CH:CH + 4], w0[:])

            # ---- AllGather h1 ----
            nc.gpsimd.collective_compute(
                "AllGather", AL.bypass, groups, [h1loc[:, :]], [h1full[:, :]])

            def edge_pass(pass_id, tiles, NG, gidx, dT, dR, base, abt, U, H,
                          UW, as_col, qoff, fblock):
                """One lo/hi pass over all dst blocks of one layer.
                tiles: [(blk, seg_first, seg_last)] per tile slot.
                base: gather base AP; abt: [NLOCP, H] bf16 per-dst logits.
                U: [128, NB, UW] f32 SBUF accumulator (pre-initialized with
                the self-loop term). as_col: first table column holding the
                H per-src logits. fblock(blk): emitted when blk's U is
                complete (layer-boundary fusion)."""
                cur_ab = {}
                U_ps = None
                for g in range(NG):
                    it = ipool.tile([128, GC // 16], I16, tag="it")
                    nc.sync.dma_start(it[:], gidx[:, g * (GC // 16):(g + 1) * (GC // 16)])
                    dt = ipool.tile([128, TPG], BF16, tag="dt")
                    nc.sync.dma_start(dt[:], dT[:, g * TPG:(g + 1) * TPG])
                    # dst ids replicated to all partitions ([d, e] layout)
                    db = ipool.tile([128, GC], BF16, tag="db")
                    nc.sync.dma_start(
                        db[:],
                        dR[0:1, g * GC:(g + 1) * GC].broadcast_to([128, GC]))
                    G = epool.tile([128, TPG, TW], BF16, tag="G")
                    nc.gpsimd.dma_gather(
                        G[:], base, it[:], GC, GC, TW,
                        queue_num=(g + qoff) % NQ)

                    # one-hot [e, d] and transposed [d, e] (pads match nothing)
                    OH = opool.tile([128, TPG, 128], BF16, tag="OH")
                    nc.vector.tensor_tensor(
                        OH[:],
                        iotab[:].unsqueeze(1).broadcast_to([128, TPG, 128]),
                        dt[:].unsqueeze(2).broadcast_to([128, TPG, 128]),
                        AL.is_equal)
                    OHT = opool.tile([128, TPG, 128], BF16, tag="OHT")
                    nc.vector.tensor_tensor(
                        OHT[:],
                        db[:].rearrange("p (t e) -> p t e", e=128),
                        iotapb[:].unsqueeze(2).broadcast_to([128, TPG, 128]),
                        AL.is_equal)

                    # ad_e = OHT^T @ ab_blk
                    ad_ps = aps.tile([128, TPG, H], F32, tag="ad")
                    for t in range(TPG):
                        blk = tiles[g * TPG + t][0]
                        if blk not in cur_ab:
                            abk = abpool.tile([128, H], BF16, tag="abk")
                            nc.sync.dma_start(
                                abk[:], abt[blk * 128:(blk + 1) * 128, :])
                            cur_ab = {blk: abk}
                        nc.tensor.matmul(
                            ad_ps[:, t:t + 1, :].rearrange("p a b -> p (a b)"),
                            OHT[:, t:t + 1, :].rearrange("p a b -> p (a b)"),
                            cur_ab[blk][:])

                    # w = exp(leakyrelu(as_e + ad_e)); ACT writes w into msg
                    e = spool.tile([128, TPG, H], F32, tag="e")
                    nc.vector.tensor_add(
                        e[:], G[:, :, as_col:as_col + H], ad_ps[:])
                    lr = spool.tile([128, TPG * H], F32, tag="lr")
                    nc.vector.scalar_tensor_tensor(
                        lr[:], e[:].rearrange("p a b -> p (a b)"), 0.2,
                        e[:].rearrange("p a b -> p (a b)"), AL.mult, AL.max)
                    wt = spool.tile([128, TPG * H], BF16, tag="wt")
                    nc.scalar.activation(wt[:], lr[:], AF.Exp)
                    wb = wt[:].rearrange("p (t h) -> p t h", h=H)
                    msg = mpool.tile([128, TPG, UW], BF16, tag="msg")
                    nc.scalar.activation(
                        msg[:, :, UW - H:UW], lr[:].rearrange(
                            "p (t h) -> p t h", h=H), AF.Exp)
                    if H > 1:
                        nc.vector.tensor_mul(
                            msg[:, :, 0:CH].rearrange("p t (h c) -> p t h c", c=CH // H),
                            G[:, :, 0:CH].rearrange("p t (h c) -> p t h c", c=CH // H),
                            wb.unsqueeze(3).broadcast_to([128, TPG, H, CH // H]))
                    else:
                        nc.vector.tensor_mul(
                            msg[:, :, 0:UW - 1], G[:, :, 0:UW - 1],
                            wb.broadcast_to([128, TPG, UW - 1]))

                    # segment-sum via PE: U_ps[d, :] += OH^T @ msg
                    for t in range(TPG):
                        blk, sfirst, slast = tiles[g * TPG + t]
                        if sfirst:
                            U_ps = ups.tile([128, UW], F32, tag="ups")
                        nc.tensor.matmul(
                            U_ps[:],
                            OH[:, t:t + 1, :].rearrange("p a b -> p (a b)"),
                            msg[:, t:t + 1, :].rearrange("p a b -> p (a b)"),
                            start=sfirst, stop=slast)
                        if slast:
                            nc.vector.tensor_add(
                                U[:, blk, :], U[:, blk, :], U_ps[:])
                            if fblock is not None and last_pass[blk] == pass_id:
                                fblock(blk)
                return NG

            # ---- F1 (per block, fused into E1 tail): z = U/S + b1; ELU;
            #      h2 = mid @ W2p; table + ab2; U2[blk] self-loop init ----
            def f1_block(i):
                r0 = i * 128
                ut = U1[:, i, :]
                sp = spool.tile([128, 4], F32, tag="sp")
                nc.vector.tensor_scalar(out=sp[:], in0=ut[:, 64:68],
                                        scalar1=1e-16, scalar2=None, op0=AL.add)
                rec = spool.tile([128, 4], F32, tag="rec")
                nc.vector.reciprocal(rec[:], sp[:])
                z = dpool.tile([128, CH], F32, tag="z")
                for h in range(4):
                    nc.vector.scalar_tensor_tensor(
                        z[:, 16 * h:16 * h + 16], ut[:, 16 * h:16 * h + 16],
                        rec[:, h:h + 1], b1sb[:, 16 * h:16 * h + 16],
                        AL.mult, AL.add)
                # ELU(z) = relu(z) + exp(min(z,0)) - 1
                r = dpool.tile([128, CH], F32, tag="r")
                nc.scalar.activation(r[:], z[:], AF.Relu)
                u = dpool.tile([128, CH], F32, tag="u")
                nc.vector.tensor_scalar(out=u[:], in0=z[:], scalar1=0.0,
                                        scalar2=None, op0=AL.min)
                tE = dpool.tile([128, CH], F32, tag="tE")
                nc.scalar.activation(tE[:], u[:], AF.Exp)
                mid = dpool.tile([128, CH], F32, tag="mid")
                nc.vector.scalar_tensor_tensor(
                    mid[:], tE[:], -1.0, r[:], AL.add, AL.add)
                tp2 = dps.tile([CH, 128], F32, tag="tp2")
                nc.tensor.transpose(tp2[:], mid[:, 0:CH], idn[:])
                tps2 = dpool.tile([CH, 128], F32, tag="tps2")
                nc.vector.tensor_copy(tps2[:], tp2[:])
                ps2 = dps.tile([128, CH], F32, tag="mm")
                nc.tensor.matmul(ps2[:], tps2[:], w2sb[:])
                h2b = dpool.tile([128, TW], BF16, tag="h2b")
                nc.vector.tensor_copy(h2b[:, 0:CH], ps2[:])
                nc.vector.memset(h2b[:, CH:TW], 0.0)
                nc.sync.dma_start(h2loc[r0:r0 + 128, :], h2b[:])
                ab2t = spool.tile([128, 1], BF16, tag="ab2t")
                nc.vector.tensor_copy(ab2t[:], h2b[:, 41:42])
                nc.sync.dma_start(ab2[r0:r0 + 128, :], ab2t[:])
                # self-loop init for U2[blk] from the same tile
                e2s = spool.tile([128, 1], F32, tag="e2s")
                nc.vector.tensor_add(e2s[:], h2b[:, 40:41], h2b[:, 41:42])
                lr2s = spool.tile([128, 1], F32, tag="lr2s")
                nc.vector.scalar_tensor_tensor(
                    lr2s[:], e2s[:], 0.2, e2s[:], AL.mult, AL.max)
                w2s = spool.tile([128, 1], F32, tag="w2s")
                nc.scalar.activation(w2s[:], lr2s[:], AF.Exp)
                nc.vector.tensor_mul(
                    U2[:, i, 0:40], h2b[:, 0:40],
                    w2s[:].broadcast_to([128, 40]))
                nc.vector.tensor_copy(U2[:, i, 40:41], w2s[:])

            # ---- F2 (per block, fused into E2 tail): out = U2/S2 + b2 ----
            def f2_block(i):
                r0 = i * 128
                ut = U2[:, i, :]
                sp = spool.tile([128, 1], F32, tag="sp2")
                nc.vector.tensor_scalar(out=sp[:], in0=ut[:, 40:41],
                                        scalar1=1e-16, scalar2=None, op0=AL.add)
                rec = spool.tile([128, 1], F32, tag="rec2")
                nc.vector.reciprocal(rec[:], sp[:])
                ot = dpool.tile([128, 40], F32, tag="ot")
                nc.vector.scalar_tensor_tensor(
                    ot[:], ut[:, 0:40], rec[:, 0:1], b2sb[:], AL.mult, AL.add)
                nc.sync.dma_start(OUT[r0:r0 + 128, :], ot[:])

            # ---- E1 (f1 fused into the final pass per block) ----
            edge_pass(0, tiles_l, NGl, gl, dTl, dRl, h1full[0:SPLIT, :], ab1,
                      U1, 4, 68, CH, 0, f1_block)
            edge_pass(1, tiles_h, NGh, gh, dTh, dRh, h1full[SPLIT:NPAD, :],
                      ab1, U1, 4, 68, CH, NGl, f1_block)

            # ---- AllGather h2 ----
            nc.gpsimd.collective_compute(
                "AllGather", AL.bypass, groups, [h2loc[:, :]], [h2full[:, :]])

            # ---- E2 (as/ad folded into table cols 40/41; f2 fused) ----
            edge_pass(0, tiles_l, NGl, gl, dTl, dRl, h2full[0:SPLIT, :], ab2,
                      U2, 1, 41, 40, 0, f2_block)
            edge_pass(1, tiles_h, NGh, gh, dTh, dRh, h2full[SPLIT:NPAD, :],
                      ab2, U2, 1, 41, 40, NGl, f2_block)

            if debug:
                for i in range(NB):
                    r0 = i * 128
                    for src_d, dst_d in ((h1loc, Dh1), (h2loc, Dh2)):
                        tt = dpool.tile([128, TW], BF16, tag="dbg")
                        nc.sync.dma_start(tt[:], src_d[r0:r0 + 128, :])
                        nc.sync.dma_start(dst_d[r0:r0 + 128, :], tt[:])
                    du1 = dpool.tile([128, 68], F32, tag="du1")
                    nc.vector.tensor_copy(du1[:], U1[:, i, :])
                    nc.sync.dma_start(DU1[r0:r0 + 128, :], du1[:])
                    du2 = dpool.tile([128, 41], F32, tag="du2")
                    nc.vector.tensor_copy(du2[:], U2[:, i, :])
                    nc.sync.dma_start(DU2[r0:r0 + 128, :], du2[:])

    nc.finalize()
    return nc


def _wrap_idx(a):
    """int16 [cap] -> wrapped [16, cap/16] replicated to [128, cap/16]."""
    w = a.reshape(-1, 16).T.copy()
    return np.ascontiguousarray(np.tile(w, (8, 1)))


def prep(x, edge_index, W1, a_src1, a_dst1, b1, W2, a_src2, a_dst2, b2):
    """Host-side sharding/index prep. Returns (meta, in_maps, (N, FIN))."""
    x = np.asarray(x, np.float32)
    N, FIN = x.shape
    NLOC = (N + NCORES - 1) // NCORES                       # 6250
    NLOCP = ((NLOC + 127) // 128) * 128                     # 6272
    NB = NLOCP // 128                                       # 49
    NPAD = NCORES * NLOCP
    NBINS = NCORES * NB

    ei0 = np.asarray(edge_index[0]).astype(np.int64)
    ei1 = np.asarray(edge_index[1]).astype(np.int64)

    # balanced bin-pack: nodes -> 392 (core, block) bins by in-degree,
    # so per-block edge counts are ~equal across cores and blocks
    deg = np.bincount(ei1, minlength=N)
    order = np.argsort(-deg, kind="stable")
    heap = [(0, b) for b in range(NBINS)]
    heapq.heapify(heap)
    count = np.zeros(NBINS, np.int64)
    newrow = np.empty(N, np.int64)          # node -> global padded row
    for n in order:
        while True:
            load, b = heapq.heappop(heap)
            if count[b] < 128:
                break
        newrow[n] = b * 128 + count[b]
        count[b] += 1
        heapq.heappush(heap, (load + int(deg[n]), b))

    rmap = newrow[ei0]                       # table row of src
    drow = newrow[ei1]
    core = drow // NLOCP
    dl = drow % NLOCP
    blk_all = dl // 128
    dlm_all = dl % 128

    # per (core, pass): edges sorted by (dst-block, src-row)
    per = []  # [core][pass] = (rs_sorted, blk_sorted, dlm_sorted)
    for c in range(NCORES):
        m = core == c
        rs_c, blk_c, dlm_c = rmap[m], blk_all[m], dlm_all[m]
        lo = rs_c < SPLIT
        rows = []
        for p, sel in enumerate((lo, ~lo)):
            rs, blk, dlm = rs_c[sel], blk_c[sel], dlm_c[sel]
            o = np.lexsort((rs, blk))
            rows.append((rs[o] - (SPLIT if p else 0), blk[o], dlm[o]))
        per.append(rows)

    # uniform tiles-per-(pass, block) across cores
    TPB = np.zeros((2, NB), np.int64)
    for c in range(NCORES):
        for p in range(2):
            cnt = np.bincount(per[c][p][1], minlength=NB)
            TPB[p] = np.maximum(TPB[p], (cnt + 127) // 128)
    last_pass_for_blk = np.where(TPB[1] > 0, 1, 0).tolist()

    def pass_meta(p):
        bids = np.repeat(np.arange(NB), TPB[p])
        NG = (len(bids) + TPG - 1) // TPG
        pad = NG * TPG - len(bids)
        if pad:
            bids = np.concatenate([bids, np.full(pad, bids[-1])])
        first = np.ones(len(bids), bool)
        first[1:] = bids[1:] != bids[:-1]
        last = np.ones(len(bids), bool)
        last[:-1] = bids[1:] != bids[:-1]
        return NG, list(zip(bids.tolist(), first.tolist(), last.tolist()))

    NGl, tiles_l = pass_meta(0)
    NGh, tiles_h = pass_meta(1)
    slot_base = [np.concatenate([[0], np.cumsum(TPB[p]) * 128]) for p in range(2)]

    # ---- constant inputs (replicated) ----
    W1 = np.asarray(W1, np.float32)
    W2p = np.zeros((CH, CH), np.float32)
    W2p[:, :40] = np.asarray(W2, np.float32)
    W2p[:, 40] = W2p[:, :40] @ np.asarray(a_src2, np.float32).reshape(40)
    W2p[:, 41] = W2p[:, :40] @ np.asarray(a_dst2, np.float32).reshape(40)
    IOTAB = np.ascontiguousarray(np.tile(
        np.arange(128, dtype=np.float32)[None, :], (128, 1))).astype(
            ml_dtypes.bfloat16)
    IOTAPB = np.arange(128, dtype=np.float32)[:, None].astype(
        ml_dtypes.bfloat16)
    IDN = np.eye(128, dtype=np.float32)
    as1 = np.asarray(a_src1, np.float32).reshape(CH)
    ad1 = np.asarray(a_dst1, np.float32).reshape(CH)
    asrc1t = np.ascontiguousarray(np.tile(as1[None, :], (128, 1)))
    adst1r = np.ascontiguousarray(np.tile(ad1[None, :], (128, 1)))
    b1r = np.ascontiguousarray(
        np.tile(np.asarray(b1, np.float32)[None, :], (128, 1)))
    b2r = np.ascontiguousarray(
        np.tile(np.asarray(b2, np.float32)[None, :], (128, 1)))

    xpad = np.zeros((NPAD, FIN), np.float32)
    xpad[newrow] = x

    in_maps = []
    for c in range(NCORES):
        packs = []
        for p, NG in ((0, NGl), (1, NGh)):
            slots = NG * GC
            idx_arr = np.zeros(slots, np.int64)
            dloc_arr = np.full(slots, 999.0, np.float32)
            rs, blk, dlm = per[c][p]
            if len(blk):
                starts = np.concatenate([[0], np.cumsum(np.bincount(blk, minlength=NB))])
                rank = np.arange(len(blk)) - starts[blk]
                pos = slot_base[p][blk] + rank
                idx_arr[pos] = rs
                dloc_arr[pos] = dlm.astype(np.float32)
            packs.append((
                _wrap_idx(idx_arr.astype(np.int16)),
                np.ascontiguousarray(
                    dloc_arr.reshape(-1, 128).T).astype(ml_dtypes.bfloat16),
                dloc_arr[None, :].astype(ml_dtypes.bfloat16)))
        xT = np.ascontiguousarray(xpad[c * NLOCP:(c + 1) * NLOCP].T)
        in_maps.append({
            "xTs": xT, "W1": W1, "W2p": W2p, "IOTAB": IOTAB, "IOTAPB": IOTAPB,
            "IDN": IDN,
            "asrc1t": asrc1t, "adst1r": adst1r, "b1r": b1r, "b2r": b2r,
            "gl": packs[0][0], "dTl": packs[0][1], "dRl": packs[0][2],
            "gh": packs[1][0], "dTh": packs[1][1], "dRh": packs[1][2],
        })

    meta = {
        "NLOC": NLOC, "NLOCP": NLOCP, "NB": NB,
        "NGl": NGl, "NGh": NGh, "tiles_l": tiles_l, "tiles_h": tiles_h,
        "last_pass_for_blk": last_pass_for_blk,
        "newrow": newrow,
    }
    return meta, in_maps, (N, FIN)


def kernel(**inputs):
    global LAST_RESULTS
    meta, in_maps, (N, FIN) = prep(**inputs)
    NLOCP = meta["NLOCP"]
    key = (N, FIN, meta["NGl"], meta["NGh"],
           tuple(t[0] for t in meta["tiles_l"]),
           tuple(t[0] for t in meta["tiles_h"]))
    if key not in _prog_cache:
        _prog_cache[key] = _build(meta)
    nc = _prog_cache[key]

    want_trace = bool(os.environ.get("GAT_TRACE"))
    if want_trace:
        try:
            from antenv import axon_hooks  # noqa: F401
        except ImportError:
            want_trace = False
    res = run_bass_kernel_spmd(
        nc, in_maps, core_ids=list(range(NCORES)), trace=want_trace)
    LAST_RESULTS = res
    full = np.concatenate([res.results[c]["OUT"] for c in range(NCORES)], 0)
    return np.ascontiguousarray(full[meta["newrow"]])


# revision 26
# speedup vs baseline: 1.3689x; 1.1085x over previous
"""2-layer GAT on 8 Trainium2 NeuronCores (Bass/Tile, SPMD) — v5.

Strategy (edge-parallel, dst-sharded): destination nodes are bin-packed on
the host into 392 (core, block-of-128) bins balanced by in-degree, so every
block sees ~equal edge work (minimal tile padding, SPMD-uniform program).
Per core, edges are sorted by (src<32768 split, dst-block, src). Node tables
are bf16 [*, 128] rows (256B) with per-src attention logits folded into
spare columns, so one SWDGE gather per edge fetches everything src-side.
Per 128-edge tile the kernel
  - builds one-hot [edge x dst] and its transpose [dst x edge] on DVE
    (is_equal vs iota consts; dst ids arrive in both layouts, the [dst x
    edge] one via a partition-broadcast DMA),
  - gets per-edge dst logits ad_e with one small PE matmul (OHT^T @ ab_blk),
  - computes w = exp(leakyrelu(as_e + ad_e)) on DVE/ACT (ACT writes w
    straight into the msg tile),
  - segment-sums [w*h | w] into PSUM via one PE matmul with the one-hot as
    stationary (no dma_scatter_add RMW).
Self-loop edges never enter the edge stream: their contribution initializes
the per-block U/S SBUF accumulators element-wise during the dense phases.
Softmax is the U/S ratio so no per-edge normalization. Layer boundaries
fuse normalize+ELU+projection per block, interleaved into the tail edge
pass so the PE/DVE work overlaps the remaining gathers. 4 SWDGE queues
round-robin so descriptor generation overlaps DMA.
"""
import heapq
import os
import numpy as np
import ml_dtypes

from concourse import bacc, mybir, tile
from concourse.bass_utils import run_bass_kernel_spmd

NCORES = 8
CH = 64          # feature channels (L2 zero-padded 40->64); table rows 128
TW = 128         # table row width (bf16 -> 256B rows)
GC = int(os.environ.get("GAT_GC", "1024"))  # idxs per SWDGE gather call
TPG = GC // 128  # tiles per gather group
SPLIT = 32768    # int16 gather index reach (rows)
NQ = 4           # SWDGE queues, round-robin over gather calls
F32 = mybir.dt.float32
BF16 = mybir.dt.bfloat16
I16 = mybir.dt.int16
AL = mybir.AluOpType
AF = mybir.ActivationFunctionType

_prog_cache = {}
LAST_RESULTS = None  # BassKernelResults of the last device run (for test.py)


def _build(meta):
    NLOCP = meta["NLOCP"]
    NB = meta["NB"]
    NPAD = NCORES * NLOCP
    NGl, NGh = meta["NGl"], meta["NGh"]
    tiles_l, tiles_h = meta["tiles_l"], meta["tiles_h"]  # [(blk, first, last)]
    last_pass = meta["last_pass_for_blk"]
    SL, SH = NGl * GC, NGh * GC

    nc = bacc.Bacc(num_devices=NCORES, num_swdge_queues=NQ,
                   dynamic_dma_scratch_size=16 * GC)

    # ---- I/O ----
    xTs = nc.dram_tensor("xTs", [128, NLOCP], F32, kind="ExternalInput")
    W1 = nc.dram_tensor("W1", [128, CH], F32, kind="ExternalInput")
    W2p = nc.dram_tensor("W2p", [CH, CH], F32, kind="ExternalInput")
    IOTAB = nc.dram_tensor("IOTAB", [128, 128], BF16, kind="ExternalInput")
    IOTAPB = nc.dram_tensor("IOTAPB", [128, 1], BF16, kind="ExternalInput")
    IDN = nc.dram_tensor("IDN", [128, 128], F32, kind="ExternalInput")
    asrc1t = nc.dram_tensor("asrc1t", [128, CH], F32, kind="ExternalInput")
    adst1r = nc.dram_tensor("adst1r", [128, CH], F32, kind="ExternalInput")
    b1r = nc.dram_tensor("b1r", [128, CH], F32, kind="ExternalInput")
    b2r = nc.dram_tensor("b2r", [128, 40], F32, kind="ExternalInput")
    gl = nc.dram_tensor("gl", [128, SL // 16], I16, kind="ExternalInput")
    gh = nc.dram_tensor("gh", [128, SH // 16], I16, kind="ExternalInput")
    dTl = nc.dram_tensor("dTl", [128, SL // 128], BF16, kind="ExternalInput")
    dTh = nc.dram_tensor("dTh", [128, SH // 128], BF16, kind="ExternalInput")
    dRl = nc.dram_tensor("dRl", [1, SL], BF16, kind="ExternalInput")
    dRh = nc.dram_tensor("dRh", [1, SH], BF16, kind="ExternalInput")
    OUT = nc.dram_tensor("OUT", [NLOCP, 40], F32, kind="ExternalOutput")

    # ---- scratch ----
    h1loc = nc.dram_tensor("h1loc", [NLOCP, TW], BF16, kind="Internal")
    h1full = nc.dram_tensor("h1full", [NPAD, TW], BF16, kind="Internal",
                            addr_space="Shared")
    ab1 = nc.dram_tensor("ab1", [NLOCP, 4], BF16, kind="Internal")
    h2loc = nc.dram_tensor("h2loc", [NLOCP, TW], BF16, kind="Internal")
    h2full = nc.dram_tensor("h2full", [NPAD, TW], BF16, kind="Internal",
                            addr_space="Shared")
    ab2 = nc.dram_tensor("ab2", [NLOCP, 1], BF16, kind="Internal")

    debug = bool(os.environ.get("GAT_DEBUG"))
    if debug:
        Dh1 = nc.dram_tensor("Dh1", [NLOCP, TW], BF16, kind="ExternalOutput")
        DU1 = nc.dram_tensor("DU1", [NLOCP, 68], F32, kind="ExternalOutput")
        Dh2 = nc.dram_tensor("Dh2", [NLOCP, TW], BF16, kind="ExternalOutput")
        DU2 = nc.dram_tensor("DU2", [NLOCP, 41], F32, kind="ExternalOutput")

    groups = [list(range(NCORES))]

    with tile.TileContext(nc) as tc:
        with (
            tc.tile_pool(name="const", bufs=1) as cpool,
            tc.tile_pool(name="dense", bufs=3) as dpool,
            tc.tile_pool(name="dpsum", bufs=1, space="PSUM") as dps,
            tc.tile_pool(name="apsum", bufs=2, space="PSUM") as aps,
            tc.tile_pool(name="upsum", bufs=3, space="PSUM") as ups,
            tc.tile_pool(name="idx", bufs=4) as ipool,
            tc.tile_pool(name="edge", bufs=4) as epool,
            tc.tile_pool(name="onehot", bufs=4) as opool,
            tc.tile_pool(name="msg", bufs=4) as mpool,
            tc.tile_pool(name="small", bufs=4) as spool,
            tc.tile_pool(name="ab", bufs=2) as abpool,
        ):
            # constants
            w1sb = cpool.tile([128, CH], F32)
            nc.sync.dma_start(w1sb[:], W1[:])
            w2sb = cpool.tile([CH, CH], F32)
            nc.sync.dma_start(w2sb[:], W2p[:])
            iotab = cpool.tile([128, 128], BF16)
            nc.sync.dma_start(iotab[:], IOTAB[:])
            iotapb = cpool.tile([128, 1], BF16)
            nc.sync.dma_start(iotapb[:], IOTAPB[:])
            idn = cpool.tile([128, 128], F32)
            nc.sync.dma_start(idn[:], IDN[:])
            as1sb = cpool.tile([128, CH], F32)
            nc.sync.dma_start(as1sb[:], asrc1t[:])
            ad1sb = cpool.tile([128, CH], F32)
            nc.sync.dma_start(ad1sb[:], adst1r[:])
            b1sb = cpool.tile([128, CH], F32)
            nc.sync.dma_start(b1sb[:], b1r[:])
            b2sb = cpool.tile([128, 40], F32)
            nc.sync.dma_start(b2sb[:], b2r[:])

            # persistent per-block U/S accumulators (SBUF)
            U1 = cpool.tile([128, NB, 68], F32)
            U2 = cpool.tile([128, NB, 41], F32)

            # ---- D1: h1 = x @ W1; table row = [h1 | h1.a_src1 | 0]; ab1;
            #      U1[blk] initialized with the self-loop contribution ----
            for i in range(NB):
                r0 = i * 128
                xt = dpool.tile([128, 128], F32, tag="xt")
                nc.sync.dma_start(xt[:], xTs[:, r0:r0 + 128])
                ps = dps.tile([128, CH], F32, tag="mm")
                nc.tensor.matmul(ps[:], xt[:], w1sb[:])
                ht = dpool.tile([128, CH], F32, tag="ht")
                nc.vector.tensor_copy(ht[:], ps[:])
                htb = dpool.tile([128, TW], BF16, tag="htb")
                nc.vector.tensor_copy(htb[:, 0:CH], ht[:])
                tmp = dpool.tile([128, CH], F32, tag="tmp")
                nc.vector.tensor_mul(tmp[:], ht[:], as1sb[:])
                asf = spool.tile([128, 4], F32, tag="asf")
                nc.vector.tensor_reduce(
                    asf[:], tmp[:].rearrange("p (h c) -> p h c", c=16),
                    mybir.AxisListType.X, AL.add)
                nc.vector.tensor_copy(htb[:, CH:CH + 4], asf[:])
                nc.vector.memset(htb[:, CH + 4:TW], 0.0)
                nc.sync.dma_start(h1loc[r0:r0 + 128, :], htb[:])
                tmp2 = dpool.tile([128, CH], F32, tag="tmp2")
                nc.vector.tensor_mul(tmp2[:], ht[:], ad1sb[:])
                dpf = spool.tile([128, 4], F32, tag="dpf")
                nc.vector.tensor_reduce(
                    dpf[:], tmp2[:].rearrange("p (h c) -> p h c", c=16),
                    mybir.AxisListType.X, AL.add)
                dp = spool.tile([128, 4], BF16, tag="dp")
                nc.vector.tensor_copy(dp[:], dpf[:])
                nc.sync.dma_start(ab1[r0:r0 + 128, :], dp[:])
                # self-loop: U1[blk] = [w*h | w], w = exp(lrelu(as+ad))
                e0 = spool.tile([128, 4], F32, tag="e0")
                nc.vector.tensor_add(e0[:], asf[:], dpf[:])
                lr0 = spool.tile([128, 4], F32, tag="lr0")
                nc.vector.scalar_tensor_tensor(
                    lr0[:], e0[:], 0.2, e0[:], AL.mult, AL.max)
                w0 = spool.tile([128, 4], F32, tag="w0")
                nc.scalar.activation(w0[:], lr0[:], AF.Exp)
                nc.vector.tensor_mul(
                    U1[:, i, 0:CH].rearrange("p (h c) -> p h c", c=16),
                    ht[:].rearrange("p (h c) -> p h c", c=16),
                    w0[:].unsqueeze(2).broadcast_to([128, 4, 16]))
                nc.vector.tensor_copy(U1[:, i, CH:CH + 4], w0[:])

            # ---- AllGather h1 ----
            nc.gpsimd.collective_compute(
                "AllGather", AL.bypass, groups, [h1loc[:, :]], [h1full[:, :]])

            def edge_pass(pass_id, tiles, NG, gidx, dT, dR, base, abt, U, H,
                          UW, as_col, qoff, fblock):
                """One lo/hi pass over all dst blocks of one layer.
                tiles: [(blk, seg_first, seg_last)] per tile slot.
                base: gather base AP; abt: [NLOCP, H] bf16 per-dst logits.
                U: [128, NB, UW] f32 SBUF accumulator (pre-initialized with
                the self-loop term). as_col: first table column holding the
                H per-src logits. fblock(blk): emitted when blk's U is
                complete (layer-boundary fusion)."""
                cur_ab = {}
                U_ps = None
                for g in range(NG):
                    it = ipool.tile([128, GC // 16], I16, tag="it")
                    nc.sync.dma_start(it[:], gidx[:, g * (GC // 16):(g + 1) * (GC // 16)])
                    dt = ipool.tile([128, TPG], BF16, tag="dt")
                    nc.sync.dma_start(dt[:], dT[:, g * TPG:(g + 1) * TPG])
                    # dst ids replicated to all partitions ([d, e] layout);
                    # issued on the Activation HWDGE queue to offload Sync
                    db = ipool.tile([128, GC], BF16, tag="db")
                    nc.scalar.dma_start(
                        db[:],
                        dR[0:1, g * GC:(g + 1) * GC].broadcast_to([128, GC]))
                    G = epool.tile([128, TPG, TW], BF16, tag="G")
                    nc.gpsimd.dma_gather(
                        G[:], base, it[:], GC, GC, TW,
                        queue_num=(g + qoff) % NQ)

                    # one-hot [e, d] and transposed [d, e] (pads match nothing)
                    OH = opool.tile([128, TPG, 128], BF16, tag="OH")
                    nc.vector.tensor_tensor(
                        OH[:],
                        iotab[:].unsqueeze(1).broadcast_to([128, TPG, 128]),
                        dt[:].unsqueeze(2).broadcast_to([128, TPG, 128]),
                        AL.is_equal)
                    OHT = opool.tile([128, TPG, 128], BF16, tag="OHT")
                    nc.vector.tensor_tensor(
                        OHT[:],
                        db[:].rearrange("p (t e) -> p t e", e=128),
                        iotapb[:].unsqueeze(2).broadcast_to([128, TPG, 128]),
                        AL.is_equal)

                    # ad_e = OHT^T @ ab_blk
                    ad_ps = aps.tile([128, TPG, H], F32, tag="ad")
                    for t in range(TPG):
                        blk = tiles[g * TPG + t][0]
                        if blk not in cur_ab:
                            abk = abpool.tile([128, H], BF16, tag="abk")
                            nc.sync.dma_start(
                                abk[:], abt[blk * 128:(blk + 1) * 128, :])
                            cur_ab = {blk: abk}
                        nc.tensor.matmul(
                            ad_ps[:, t:t + 1, :].rearrange("p a b -> p (a b)"),
                            OHT[:, t:t + 1, :].rearrange("p a b -> p (a b)"),
                            cur_ab[blk][:])

                    # w = exp(leakyrelu(as_e + ad_e)); ACT writes w into msg
                    e = spool.tile([128, TPG, H], F32, tag="e")
                    nc.vector.tensor_add(
                        e[:], G[:, :, as_col:as_col + H], ad_ps[:])
                    lr = spool.tile([128, TPG * H], F32, tag="lr")
                    nc.vector.scalar_tensor_tensor(
                        lr[:], e[:].rearrange("p a b -> p (a b)"), 0.2,
                        e[:].rearrange("p a b -> p (a b)"), AL.mult, AL.max)
                    wt = spool.tile([128, TPG * H], BF16, tag="wt")
                    nc.scalar.activation(wt[:], lr[:], AF.Exp)
                    wb = wt[:].rearrange("p (t h) -> p t h", h=H)
                    msg = mpool.tile([128, TPG, UW], BF16, tag="msg")
                    nc.scalar.activation(
                        msg[:, :, UW - H:UW], lr[:].rearrange(
                            "p (t h) -> p t h", h=H), AF.Exp)
                    if H > 1:
                        nc.vector.tensor_mul(
                            msg[:, :, 0:CH].rearrange("p t (h c) -> p t h c", c=CH // H),
                            G[:, :, 0:CH].rearrange("p t (h c) -> p t h c", c=CH // H),
                            wb.unsqueeze(3).broadcast_to([128, TPG, H, CH // H]))
                    else:
                        nc.vector.tensor_mul(
                            msg[:, :, 0:UW - 1], G[:, :, 0:UW - 1],
                            wb.broadcast_to([128, TPG, UW - 1]))

                    # segment-sum via PE: U_ps[d, :] += OH^T @ msg
                    for t in range(TPG):
                        blk, sfirst, slast = tiles[g * TPG + t]
                        if sfirst:
                            U_ps = ups.tile([128, UW], F32, tag="ups")
                        nc.tensor.matmul(
                            U_ps[:],
                            OH[:, t:t + 1, :].rearrange("p a b -> p (a b)"),
                            msg[:, t:t + 1, :].rearrange("p a b -> p (a b)"),
                            start=sfirst, stop=slast)
                        if slast:
                            nc.vector.tensor_add(
                                U[:, blk, :], U[:, blk, :], U_ps[:])
                            if fblock is not None and last_pass[blk] == pass_id:
                                fblock(blk)
                return NG

            # ---- F1 (batched over runs of completed blocks, fused into the
            #      E1 tail): z = U/S + b1; ELU; h2 = mid @ W2p; table + ab2;
            #      U2 self-loop init ----
            def f1_batch(batch):
                i0, nb = batch[0], len(batch)
                ut = U1[:, i0:i0 + nb, :]
                sp = spool.tile([128, nb, 4], F32, tag=f"sp{nb}")
                nc.vector.tensor_scalar(out=sp[:], in0=ut[:, :, 64:68],
                                        scalar1=1e-16, scalar2=None, op0=AL.add)
                rec = spool.tile([128, nb, 4], F32, tag=f"rec{nb}")
                nc.vector.reciprocal(rec[:], sp[:])
                z = dpool.tile([128, nb, CH], F32, tag=f"z{nb}")
                nc.vector.tensor_mul(
                    z[:].rearrange("p t (h c) -> p t h c", c=16),
                    ut[:, :, 0:CH].rearrange("p t (h c) -> p t h c", c=16),
                    rec[:].unsqueeze(3).broadcast_to([128, nb, 4, 16]))
                nc.vector.tensor_add(
                    z[:], z[:], b1sb[:].unsqueeze(1).broadcast_to([128, nb, CH]))
                # ELU(z) = relu(z) + exp(min(z,0)) - 1
                r = dpool.tile([128, nb, CH], F32, tag=f"r{nb}")
                nc.scalar.activation(r[:], z[:], AF.Relu)
                u = dpool.tile([128, nb, CH], F32, tag=f"u{nb}")
                nc.vector.tensor_scalar(out=u[:], in0=z[:], scalar1=0.0,
                                        scalar2=None, op0=AL.min)
                tE = dpool.tile([128, nb, CH], F32, tag=f"tE{nb}")
                nc.scalar.activation(tE[:], u[:], AF.Exp)
                mid = dpool.tile([128, nb, CH], F32, tag=f"mid{nb}")
                nc.vector.scalar_tensor_tensor(
                    mid[:].rearrange("p a b -> p (a b)"),
                    tE[:].rearrange("p a b -> p (a b)"), -1.0,
                    r[:].rearrange("p a b -> p (a b)"), AL.add, AL.add)
                h2bb = dpool.tile([128, nb, TW], BF16, tag=f"h2bb{nb}")
                nc.vector.memset(h2bb[:, :, CH:TW], 0.0)
                for k, i in enumerate(batch):
                    tp2 = dps.tile([CH, 128], F32, tag="tp2")
                    nc.tensor.transpose(
                        tp2[:],
                        mid[:, k:k + 1, :].rearrange("p a b -> p (a b)"),
                        idn[:])
                    tps2 = dpool.tile([CH, 128], F32, tag="tps2")
                    nc.vector.tensor_copy(tps2[:], tp2[:])
                    ps2 = dps.tile([128, CH], F32, tag="mm")
                    nc.tensor.matmul(ps2[:], tps2[:], w2sb[:])
                    nc.vector.tensor_copy(h2bb[:, k, 0:CH], ps2[:])
                for k, i in enumerate(batch):
                    nc.sync.dma_start(h2loc[i * 128:(i + 1) * 128, :],
                                      h2bb[:, k, :])
                ab2t = spool.tile([128, nb], BF16, tag=f"ab2t{nb}")
                nc.vector.tensor_copy(
                    ab2t[:], h2bb[:, :, 41:42].rearrange("p a b -> p (a b)"))
                for k, i in enumerate(batch):
                    nc.sync.dma_start(ab2[i * 128:(i + 1) * 128, :],
                                      ab2t[:, k:k + 1])
                # self-loop init for U2 blocks from the same tiles
                e2s = spool.tile([128, nb], F32, tag=f"e2s{nb}")
                nc.vector.tensor_add(
                    e2s[:], h2bb[:, :, 40:41].rearrange("p a b -> p (a b)"),
                    h2bb[:, :, 41:42].rearrange("p a b -> p (a b)"))
                lr2s = spool.tile([128, nb], F32, tag=f"lr2s{nb}")
                nc.vector.scalar_tensor_tensor(
                    lr2s[:], e2s[:], 0.2, e2s[:], AL.mult, AL.max)
                w2s = spool.tile([128, nb], F32, tag=f"w2s{nb}")
                nc.scalar.activation(w2s[:], lr2s[:], AF.Exp)
                nc.vector.tensor_mul(
                    U2[:, i0:i0 + nb, 0:40], h2bb[:, :, 0:40],
                    w2s[:].unsqueeze(2).broadcast_to([128, nb, 40]))
                nc.vector.tensor_copy(U2[:, i0:i0 + nb, 40:41],
                                      w2s[:].unsqueeze(2))

            # ---- F2 (batched, fused into E2 tail): out = U2/S2 + b2 ----
            def f2_batch(batch):
                i0, nb = batch[0], len(batch)
                ut = U2[:, i0:i0 + nb, :]
                sp = spool.tile([128, nb], F32, tag=f"sp2{nb}")
                nc.vector.tensor_scalar(
                    out=sp[:], in0=ut[:, :, 40:41].rearrange("p a b -> p (a b)"),
                    scalar1=1e-16, scalar2=None, op0=AL.add)
                rec = spool.tile([128, nb], F32, tag=f"rec2{nb}")
                nc.vector.reciprocal(rec[:], sp[:])
                ot = dpool.tile([128, nb, 40], F32, tag=f"ot{nb}")
                nc.vector.tensor_mul(
                    ot[:], ut[:, :, 0:40],
                    rec[:].unsqueeze(2).broadcast_to([128, nb, 40]))
                nc.vector.tensor_add(
                    ot[:], ot[:], b2sb[:].unsqueeze(1).broadcast_to([128, nb, 40]))
                for k, i in enumerate(batch):
                    nc.sync.dma_start(OUT[i * 128:(i + 1) * 128, :],
                                      ot[:, k, :])

            def batcher(emit):
                batch = []

                def add(blk):
                    if batch and (blk != batch[-1] + 1 or len(batch) == 4):
                        emit(batch[:])
                        batch.clear()
                    batch.append(blk)

                def flush():
                    if batch:
                        emit(batch[:])
                        batch.clear()
                return add, flush

            # ---- E1 (f1 fused into the final pass per block) ----
            f1_add, f1_flush = batcher(f1_batch)
            edge_pass(0, tiles_l, NGl, gl, dTl, dRl, h1full[0:SPLIT, :], ab1,
                      U1, 4, 68, CH, 0, f1_add)
            edge_pass(1, tiles_h, NGh, gh, dTh, dRh, h1full[SPLIT:NPAD, :],
                      ab1, U1, 4, 68, CH, NGl, f1_add)
            f1_flush()

            # ---- AllGather h2 ----
            nc.gpsimd.collective_compute(
                "AllGather", AL.bypass, groups, [h2loc[:, :]], [h2full[:, :]])

            # ---- E2 (as/ad folded into table cols 40/41; f2 fused) ----
            f2_add, f2_flush = batcher(f2_batch)
            edge_pass(0, tiles_l, NGl, gl, dTl, dRl, h2full[0:SPLIT, :], ab2,
                      U2, 1, 41, 40, 0, f2_add)
            edge_pass(1, tiles_h, NGh, gh, dTh, dRh, h2full[SPLIT:NPAD, :],
                      ab2, U2, 1, 41, 40, NGl, f2_add)
            f2_flush()

            if debug:
                for i in range(NB):
                    r0 = i * 128
                    for src_d, dst_d in ((h1loc, Dh1), (h2loc, Dh2)):
                        tt = dpool.tile([128, TW], BF16, tag="dbg")
                        nc.sync.dma_start(tt[:], src_d[r0:r0 + 128, :])
                        nc.sync.dma_start(dst_d[r0:r0 + 128, :], tt[:])
                    du1 = dpool.tile([128, 68], F32, tag="du1")
                    nc.vector.tensor_copy(du1[:], U1[:, i, :])
                    nc.sync.dma_start(DU1[r0:r0 + 128, :], du1[:])
                    du2 = dpool.tile([128, 41], F32, tag="du2")
                    nc.vector.tensor_copy(du2[:], U2[:, i, :])
                    nc.sync.dma_start(DU2[r0:r0 + 128, :], du2[:])

    nc.finalize()
    return nc


def _wrap_idx(a):
    """int16 [cap] -> wrapped [16, cap/16] replicated to [128, cap/16]."""
    w = a.reshape(-1, 16).T.copy()
    return np.ascontiguousarray(np.tile(w, (8, 1)))


def prep(x, edge_index, W1, a_src1, a_dst1, b1, W2, a_src2, a_dst2, b2):
    """Host-side sharding/index prep. Returns (meta, in_maps, (N, FIN))."""
    x = np.asarray(x, np.float32)
    N, FIN = x.shape
    NLOC = (N + NCORES - 1) // NCORES                       # 6250
    NLOCP = ((NLOC + 127) // 128) * 128                     # 6272
    NB = NLOCP // 128                                       # 49
    NPAD = NCORES * NLOCP
    NBINS = NCORES * NB

    ei0 = np.asarray(edge_index[0]).astype(np.int64)
    ei1 = np.asarray(edge_index[1]).astype(np.int64)

    # balanced bin-pack: nodes -> 392 (core, block) bins by in-degree,
    # so per-block edge counts are ~equal across cores and blocks
    deg = np.bincount(ei1, minlength=N)
    order = np.argsort(-deg, kind="stable")
    heap = [(0, b) for b in range(NBINS)]
    heapq.heapify(heap)
    count = np.zeros(NBINS, np.int64)
    newrow = np.empty(N, np.int64)          # node -> global padded row
    for n in order:
        while True:
            load, b = heapq.heappop(heap)
            if count[b] < 128:
                break
        newrow[n] = b * 128 + count[b]
        count[b] += 1
        heapq.heappush(heap, (load + int(deg[n]), b))

    rmap = newrow[ei0]                       # table row of src
    drow = newrow[ei1]
    core = drow // NLOCP
    dl = drow % NLOCP
    blk_all = dl // 128
    dlm_all = dl % 128

    # per (core, pass): edges sorted by (dst-block, src-row)
    per = []  # [core][pass] = (rs_sorted, blk_sorted, dlm_sorted)
    for c in range(NCORES):
        m = core == c
        rs_c, blk_c, dlm_c = rmap[m], blk_all[m], dlm_all[m]
        lo = rs_c < SPLIT
        rows = []
        for p, sel in enumerate((lo, ~lo)):
            rs, blk, dlm = rs_c[sel], blk_c[sel], dlm_c[sel]
            o = np.lexsort((rs, blk))
            rows.append((rs[o] - (SPLIT if p else 0), blk[o], dlm[o]))
        per.append(rows)

    # uniform tiles-per-(pass, block) across cores
    TPB = np.zeros((2, NB), np.int64)
    for c in range(NCORES):
        for p in range(2):
            cnt = np.bincount(per[c][p][1], minlength=NB)
            TPB[p] = np.maximum(TPB[p], (cnt + 127) // 128)
    last_pass_for_blk = np.where(TPB[1] > 0, 1, 0).tolist()

    def pass_meta(p):
        bids = np.repeat(np.arange(NB), TPB[p])
        NG = (len(bids) + TPG - 1) // TPG
        pad = NG * TPG - len(bids)
        if pad:
            bids = np.concatenate([bids, np.full(pad, bids[-1])])
        first = np.ones(len(bids), bool)
        first[1:] = bids[1:] != bids[:-1]
        last = np.ones(len(bids), bool)
        last[:-1] = bids[1:] != bids[:-1]
        return NG, list(zip(bids.tolist(), first.tolist(), last.tolist()))

    NGl, tiles_l = pass_meta(0)
    NGh, tiles_h = pass_meta(1)
    slot_base = [np.concatenate([[0], np.cumsum(TPB[p]) * 128]) for p in range(2)]

    # ---- constant inputs (replicated) ----
    W1 = np.asarray(W1, np.float32)
    W2p = np.zeros((CH, CH), np.float32)
    W2p[:, :40] = np.asarray(W2, np.float32)
    W2p[:, 40] = W2p[:, :40] @ np.asarray(a_src2, np.float32).reshape(40)
    W2p[:, 41] = W2p[:, :40] @ np.asarray(a_dst2, np.float32).reshape(40)
    IOTAB = np.ascontiguousarray(np.tile(
        np.arange(128, dtype=np.float32)[None, :], (128, 1))).astype(
            ml_dtypes.bfloat16)
    IOTAPB = np.arange(128, dtype=np.float32)[:, None].astype(
        ml_dtypes.bfloat16)
    IDN = np.eye(128, dtype=np.float32)
    as1 = np.asarray(a_src1, np.float32).reshape(CH)
    ad1 = np.asarray(a_dst1, np.float32).reshape(CH)
    asrc1t = np.ascontiguousarray(np.tile(as1[None, :], (128, 1)))
    adst1r = np.ascontiguousarray(np.tile(ad1[None, :], (128, 1)))
    b1r = np.ascontiguousarray(
        np.tile(np.asarray(b1, np.float32)[None, :], (128, 1)))
    b2r = np.ascontiguousarray(
        np.tile(np.asarray(b2, np.float32)[None, :], (128, 1)))

    xpad = np.zeros((NPAD, FIN), np.float32)
    xpad[newrow] = x

    in_maps = []
    for c in range(NCORES):
        packs = []
        for p, NG in ((0, NGl), (1, NGh)):
            slots = NG * GC
            idx_arr = np.zeros(slots, np.int64)
            dloc_arr = np.full(slots, 999.0, np.float32)
            rs, blk, dlm = per[c][p]
            if len(blk):
                starts = np.concatenate([[0], np.cumsum(np.bincount(blk, minlength=NB))])
                rank = np.arange(len(blk)) - starts[blk]
                pos = slot_base[p][blk] + rank
                idx_arr[pos] = rs
                dloc_arr[pos] = dlm.astype(np.float32)
            packs.append((
                _wrap_idx(idx_arr.astype(np.int16)),
                np.ascontiguousarray(
                    dloc_arr.reshape(-1, 128).T).astype(ml_dtypes.bfloat16),
                dloc_arr[None, :].astype(ml_dtypes.bfloat16)))
        xT = np.ascontiguousarray(xpad[c * NLOCP:(c + 1) * NLOCP].T)
        in_maps.append({
            "xTs": xT, "W1": W1, "W2p": W2p, "IOTAB": IOTAB, "IOTAPB": IOTAPB,
            "IDN": IDN,
            "asrc1t": asrc1t, "adst1r": adst1r, "b1r": b1r, "b2r": b2r,
            "gl": packs[0][0], "dTl": packs[0][1], "dRl": packs[0][2],
            "gh": packs[1][0], "dTh": packs[1][1], "dRh": packs[1][2],
        })

    meta = {
        "NLOC": NLOC, "NLOCP": NLOCP, "NB": NB,
        "NGl": NGl, "NGh": NGh, "tiles_l": tiles_l, "tiles_h": tiles_h,
        "last_pass_for_blk": last_pass_for_blk,
        "newrow": newrow,
    }
    return meta, in_maps, (N, FIN)


def kernel(**inputs):
    global LAST_RESULTS
    meta, in_maps, (N, FIN) = prep(**inputs)
    NLOCP = meta["NLOCP"]
    key = (N, FIN, meta["NGl"], meta["NGh"],
           tuple(t[0] for t in meta["tiles_l"]),
           tuple(t[0] for t in meta["tiles_h"]))
    if key not in _prog_cache:
        _prog_cache[key] = _build(meta)
    nc = _prog_cache[key]

    want_trace = bool(os.environ.get("GAT_TRACE"))
    if want_trace:
        try:
            from antenv import axon_hooks  # noqa: F401
        except ImportError:
            want_trace = False
    res = run_bass_kernel_spmd(
        nc, in_maps, core_ids=list(range(NCORES)), trace=want_trace)
    LAST_RESULTS = res
    full = np.concatenate([res.results[c]["OUT"] for c in range(NCORES)], 0)
    return np.ascontiguousarray(full[meta["newrow"]])


# revision 34
# speedup vs baseline: 1.7639x; 1.2886x over previous
"""2-layer GAT on 8 Trainium2 NeuronCores (Bass/Tile, SPMD) — v5.

Strategy (edge-parallel, dst-sharded): destination nodes are bin-packed on
the host into 392 (core, block-of-128) bins balanced by in-degree, so every
block sees ~equal edge work (minimal tile padding, SPMD-uniform program).
Per core, edges are sorted by (src<32768 split, dst-block, src). Node tables
are bf16 [*, 128] rows (256B) with per-src attention logits folded into
spare columns, so one SWDGE gather per edge fetches everything src-side.
Per 128-edge tile the kernel
  - builds one-hot [edge x dst] and its transpose [dst x edge] on DVE
    (is_equal vs iota consts; dst ids arrive in both layouts, the [dst x
    edge] one via a partition-broadcast DMA),
  - gets per-edge dst logits ad_e with one small PE matmul (OHT^T @ ab_blk),
  - computes w = exp(leakyrelu(as_e + ad_e)) on DVE/ACT (ACT writes w
    straight into the msg tile),
  - segment-sums [w*h | w] into PSUM via one PE matmul with the one-hot as
    stationary (no dma_scatter_add RMW).
Self-loop edges never enter the edge stream: their contribution initializes
the per-block U/S SBUF accumulators element-wise during the dense phases.
Softmax is the U/S ratio so no per-edge normalization. Layer boundaries
fuse normalize+ELU+projection per block, interleaved into the tail edge
pass so the PE/DVE work overlaps the remaining gathers. 4 SWDGE queues
round-robin so descriptor generation overlaps DMA.
"""
import heapq
import os
import numpy as np
import ml_dtypes

from concourse import bacc, mybir, tile
from concourse.bass_utils import run_bass_kernel_spmd

NCORES = 8
CH = 64          # feature channels (L2 zero-padded 40->64); table rows 128
TW = 128         # table row width (bf16 -> 256B rows)
GC = int(os.environ.get("GAT_GC", "1024"))  # idxs per SWDGE gather call
TPG = GC // 128  # tiles per gather group
SPLIT = 32768    # int16 gather index reach (rows)
NQ = 4           # SWDGE queues, round-robin over gather calls
F32 = mybir.dt.float32
BF16 = mybir.dt.bfloat16
I16 = mybir.dt.int16
AL = mybir.AluOpType
AF = mybir.ActivationFunctionType

_prog_cache = {}
LAST_RESULTS = None  # BassKernelResults of the last device run (for test.py)


def _build(meta):
    NLOCP = meta["NLOCP"]
    NB = meta["NB"]
    NPAD = NCORES * NLOCP
    NGl, NGh = meta["NGl"], meta["NGh"]
    tiles_l, tiles_h = meta["tiles_l"], meta["tiles_h"]  # [(blk, first, last)]
    last_pass = meta["last_pass_for_blk"]
    SL, SH = NGl * GC, NGh * GC

    nc = bacc.Bacc(num_devices=NCORES, num_swdge_queues=NQ,
                   dynamic_dma_scratch_size=16 * GC)

    # ---- I/O ----
    xTs = nc.dram_tensor("xTs", [128, NLOCP], F32, kind="ExternalInput")
    W1 = nc.dram_tensor("W1", [128, CH], F32, kind="ExternalInput")
    W2p = nc.dram_tensor("W2p", [CH, CH], F32, kind="ExternalInput")
    IOTAB = nc.dram_tensor("IOTAB", [128, 128], BF16, kind="ExternalInput")
    IOTAPB = nc.dram_tensor("IOTAPB", [128, 1], BF16, kind="ExternalInput")
    IDN = nc.dram_tensor("IDN", [128, 128], F32, kind="ExternalInput")
    asrc1t = nc.dram_tensor("asrc1t", [128, CH], F32, kind="ExternalInput")
    adst1r = nc.dram_tensor("adst1r", [128, CH], F32, kind="ExternalInput")
    b1r = nc.dram_tensor("b1r", [128, CH], F32, kind="ExternalInput")
    b2r = nc.dram_tensor("b2r", [128, 40], F32, kind="ExternalInput")
    gl = nc.dram_tensor("gl", [128, SL // 16], I16, kind="ExternalInput")
    gh = nc.dram_tensor("gh", [128, SH // 16], I16, kind="ExternalInput")
    dTl = nc.dram_tensor("dTl", [128, SL // 128], BF16, kind="ExternalInput")
    dTh = nc.dram_tensor("dTh", [128, SH // 128], BF16, kind="ExternalInput")
    dRl = nc.dram_tensor("dRl", [1, SL], BF16, kind="ExternalInput")
    dRh = nc.dram_tensor("dRh", [1, SH], BF16, kind="ExternalInput")
    OUT = nc.dram_tensor("OUT", [NLOCP, 40], F32, kind="ExternalOutput")

    # ---- scratch ----
    h1loc = nc.dram_tensor("h1loc", [NLOCP, TW], BF16, kind="Internal")
    h1full = nc.dram_tensor("h1full", [NPAD, TW], BF16, kind="Internal",
                            addr_space="Shared")
    h2loc = nc.dram_tensor("h2loc", [NLOCP, TW], BF16, kind="Internal")
    h2full = nc.dram_tensor("h2full", [NPAD, TW], BF16, kind="Internal",
                            addr_space="Shared")

    debug = bool(os.environ.get("GAT_DEBUG"))
    if debug:
        Dh1 = nc.dram_tensor("Dh1", [NLOCP, TW], BF16, kind="ExternalOutput")
        DU1 = nc.dram_tensor("DU1", [NLOCP, 68], F32, kind="ExternalOutput")
        Dh2 = nc.dram_tensor("Dh2", [NLOCP, TW], BF16, kind="ExternalOutput")
        DU2 = nc.dram_tensor("DU2", [NLOCP, 41], F32, kind="ExternalOutput")

    groups = [list(range(NCORES))]

    with tile.TileContext(nc) as tc:
        with (
            tc.tile_pool(name="const", bufs=1) as cpool,
            tc.tile_pool(name="dense", bufs=3) as dpool,
            tc.tile_pool(name="dpsum", bufs=1, space="PSUM") as dps,
            tc.tile_pool(name="apsum", bufs=2, space="PSUM") as aps,
            tc.tile_pool(name="upsum", bufs=3, space="PSUM") as ups,
            tc.tile_pool(name="idx", bufs=4) as ipool,
            tc.tile_pool(name="edge", bufs=4) as epool,
            tc.tile_pool(name="onehot", bufs=4) as opool,
            tc.tile_pool(name="msg", bufs=4) as mpool,
            tc.tile_pool(name="small", bufs=4) as spool,
        ):
            # constants
            w1sb = cpool.tile([128, CH], F32)
            nc.sync.dma_start(w1sb[:], W1[:])
            w2sb = cpool.tile([CH, CH], F32)
            nc.sync.dma_start(w2sb[:], W2p[:])
            iotab = cpool.tile([128, 128], BF16)
            nc.sync.dma_start(iotab[:], IOTAB[:])
            iotapb = cpool.tile([128, 1], BF16)
            nc.sync.dma_start(iotapb[:], IOTAPB[:])
            idn = cpool.tile([128, 128], F32)
            nc.sync.dma_start(idn[:], IDN[:])
            as1sb = cpool.tile([128, CH], F32)
            nc.sync.dma_start(as1sb[:], asrc1t[:])
            ad1sb = cpool.tile([128, CH], F32)
            nc.sync.dma_start(ad1sb[:], adst1r[:])
            b1sb = cpool.tile([128, CH], F32)
            nc.sync.dma_start(b1sb[:], b1r[:])
            b2sb = cpool.tile([128, 40], F32)
            nc.sync.dma_start(b2sb[:], b2r[:])

            # persistent per-block U/S accumulators (SBUF)
            U1 = cpool.tile([128, NB, 68], F32)
            U2 = cpool.tile([128, NB, 41], F32)

            # SBUF-resident per-dst attention tables (written by D1/F1,
            # read by the ad_e matmuls — no DRAM round-trip)
            AB1 = cpool.tile([128, NB, 4], BF16)
            AB2 = cpool.tile([128, NB, 1], BF16)

            # ---- D1 (batched x4): h1 = x @ W1; table row = [h1 | as | 0];
            #      AB1; U1 initialized with the self-loop contribution ----
            for i0 in range(0, NB, 4):
                nb = min(4, NB - i0)
                htf = dpool.tile([128, nb, CH], F32, tag=f"htf{nb}")
                for k in range(nb):
                    r0 = (i0 + k) * 128
                    xt = dpool.tile([128, 128], F32, tag="xt")
                    nc.sync.dma_start(xt[:], xTs[:, r0:r0 + 128])
                    ps = dps.tile([128, CH], F32, tag="mm")
                    nc.tensor.matmul(ps[:], xt[:], w1sb[:])
                    nc.vector.tensor_copy(htf[:, k, :], ps[:])
                htb = dpool.tile([128, nb, TW], BF16, tag=f"htb{nb}")
                nc.vector.tensor_copy(htb[:, :, 0:CH], htf[:])
                tmp = dpool.tile([128, nb, CH], F32, tag=f"tmp{nb}")
                nc.vector.tensor_mul(
                    tmp[:], htf[:],
                    as1sb[:].unsqueeze(1).broadcast_to([128, nb, CH]))
                asf = spool.tile([128, nb, 4], F32, tag=f"asf{nb}")
                nc.vector.tensor_reduce(
                    asf[:], tmp[:].rearrange("p t (h c) -> p t h c", c=16),
                    mybir.AxisListType.X, AL.add)
                nc.vector.tensor_copy(htb[:, :, CH:CH + 4], asf[:])
                nc.vector.memset(htb[:, :, CH + 4:TW], 0.0)
                tmp2 = dpool.tile([128, nb, CH], F32, tag=f"tmp2{nb}")
                nc.vector.tensor_mul(
                    tmp2[:], htf[:],
                    ad1sb[:].unsqueeze(1).broadcast_to([128, nb, CH]))
                dpf = spool.tile([128, nb, 4], F32, tag=f"dpf{nb}")
                nc.vector.tensor_reduce(
                    dpf[:], tmp2[:].rearrange("p t (h c) -> p t h c", c=16),
                    mybir.AxisListType.X, AL.add)
                nc.vector.tensor_copy(AB1[:, i0:i0 + nb, :], dpf[:])
                for k in range(nb):
                    r0 = (i0 + k) * 128
                    nc.sync.dma_start(h1loc[r0:r0 + 128, :], htb[:, k, :])
                # self-loop: U1 = [w*h | w], w = exp(lrelu(as+ad))
                e0 = spool.tile([128, nb, 4], F32, tag=f"e0{nb}")
                nc.vector.tensor_add(e0[:], asf[:], dpf[:])
                lr0 = spool.tile([128, nb, 4], F32, tag=f"lr0{nb}")
                nc.vector.scalar_tensor_tensor(
                    lr0[:].rearrange("p a b -> p (a b)"),
                    e0[:].rearrange("p a b -> p (a b)"), 0.2,
                    e0[:].rearrange("p a b -> p (a b)"), AL.mult, AL.max)
                w0 = spool.tile([128, nb, 4], F32, tag=f"w0{nb}")
                nc.scalar.activation(w0[:], lr0[:], AF.Exp)
                nc.vector.tensor_mul(
                    U1[:, i0:i0 + nb, 0:CH].rearrange("p t (h c) -> p t h c", c=16),
                    htf[:].rearrange("p t (h c) -> p t h c", c=16),
                    w0[:].unsqueeze(3).broadcast_to([128, nb, 4, 16]))
                nc.vector.tensor_copy(U1[:, i0:i0 + nb, CH:CH + 4], w0[:])

            # ---- AllGather h1 ----
            nc.gpsimd.collective_compute(
                "AllGather", AL.bypass, groups, [h1loc[:, :]], [h1full[:, :]])

            def edge_pass(pass_id, tiles, NG, gidx, dT, dR, base, abt, U, H,
                          UW, as_col, qoff, fblock):
                """One lo/hi pass over all dst blocks of one layer.
                tiles: [(blk, seg_first, seg_last)] per tile slot.
                base: gather base AP; abt: [128, NB, H] bf16 SBUF per-dst
                logits. U: [128, NB, UW] f32 SBUF accumulator (pre-init
                with the self-loop term). as_col: first table column holding
                the H per-src logits. fblock(blk): emitted when blk's U is
                complete (layer-boundary fusion)."""
                U_ps = None
                for g in range(NG):
                    it = ipool.tile([128, GC // 16], I16, tag="it")
                    nc.sync.dma_start(it[:], gidx[:, g * (GC // 16):(g + 1) * (GC // 16)])
                    dt = ipool.tile([128, TPG], BF16, tag="dt")
                    nc.sync.dma_start(dt[:], dT[:, g * TPG:(g + 1) * TPG])
                    # dst ids replicated to all partitions ([d, e] layout);
                    # issued on the Activation HWDGE queue to offload Sync
                    db = ipool.tile([128, GC], BF16, tag="db")
                    nc.scalar.dma_start(
                        db[:],
                        dR[0:1, g * GC:(g + 1) * GC].broadcast_to([128, GC]))
                    G = epool.tile([128, TPG, TW], BF16, tag="G")
                    nc.gpsimd.dma_gather(
                        G[:], base, it[:], GC, GC, TW,
                        queue_num=(g + qoff) % NQ)

                    # one-hot [e, d] and transposed [d, e] (pads match nothing)
                    OH = opool.tile([128, TPG, 128], BF16, tag="OH")
                    nc.vector.tensor_tensor(
                        OH[:],
                        iotab[:].unsqueeze(1).broadcast_to([128, TPG, 128]),
                        dt[:].unsqueeze(2).broadcast_to([128, TPG, 128]),
                        AL.is_equal)
                    OHT = opool.tile([128, TPG, 128], BF16, tag="OHT")
                    nc.vector.tensor_tensor(
                        OHT[:],
                        db[:].rearrange("p (t e) -> p t e", e=128),
                        iotapb[:].unsqueeze(2).broadcast_to([128, TPG, 128]),
                        AL.is_equal)

                    # ad_e = OHT^T @ ab_blk (ab resident in SBUF)
                    ad_ps = aps.tile([128, TPG, H], F32, tag="ad")
                    for t in range(TPG):
                        blk = tiles[g * TPG + t][0]
                        nc.tensor.matmul(
                            ad_ps[:, t:t + 1, :].rearrange("p a b -> p (a b)"),
                            OHT[:, t:t + 1, :].rearrange("p a b -> p (a b)"),
                            abt[:, blk, :])

                    # w = exp(leakyrelu(as_e + ad_e)); ACT writes w into msg
                    e = spool.tile([128, TPG, H], F32, tag="e")
                    nc.vector.tensor_add(
                        e[:], G[:, :, as_col:as_col + H], ad_ps[:])
                    lr = spool.tile([128, TPG * H], F32, tag="lr")
                    nc.vector.scalar_tensor_tensor(
                        lr[:], e[:].rearrange("p a b -> p (a b)"), 0.2,
                        e[:].rearrange("p a b -> p (a b)"), AL.mult, AL.max)
                    wt = spool.tile([128, TPG * H], BF16, tag="wt")
                    nc.scalar.activation(wt[:], lr[:], AF.Exp)
                    wb = wt[:].rearrange("p (t h) -> p t h", h=H)
                    msg = mpool.tile([128, TPG, UW], BF16, tag="msg")
                    nc.scalar.activation(
                        msg[:, :, UW - H:UW], lr[:].rearrange(
                            "p (t h) -> p t h", h=H), AF.Exp)
                    if H > 1:
                        nc.vector.tensor_mul(
                            msg[:, :, 0:CH].rearrange("p t (h c) -> p t h c", c=CH // H),
                            G[:, :, 0:CH].rearrange("p t (h c) -> p t h c", c=CH // H),
                            wb.unsqueeze(3).broadcast_to([128, TPG, H, CH // H]))
                    else:
                        nc.vector.tensor_mul(
                            msg[:, :, 0:UW - 1], G[:, :, 0:UW - 1],
                            wb.broadcast_to([128, TPG, UW - 1]))

                    # segment-sum via PE: U_ps[d, :] += OH^T @ msg
                    for t in range(TPG):
                        blk, sfirst, slast = tiles[g * TPG + t]
                        if sfirst:
                            U_ps = ups.tile([128, UW], F32, tag="ups")
                        nc.tensor.matmul(
                            U_ps[:],
                            OH[:, t:t + 1, :].rearrange("p a b -> p (a b)"),
                            msg[:, t:t + 1, :].rearrange("p a b -> p (a b)"),
                            start=sfirst, stop=slast)
                        if slast:
                            nc.vector.tensor_add(
                                U[:, blk, :], U[:, blk, :], U_ps[:])
                            if fblock is not None and last_pass[blk] == pass_id:
                                fblock(blk)
                return NG

            # ---- F1 (batched over runs of completed blocks, fused into the
            #      E1 tail): z = U/S + b1; ELU; h2 = mid @ W2p; table + ab2;
            #      U2 self-loop init ----
            def f1_batch(batch):
                i0, nb = batch[0], len(batch)
                ut = U1[:, i0:i0 + nb, :]
                sp = spool.tile([128, nb, 4], F32, tag=f"sp{nb}")
                nc.vector.tensor_scalar(out=sp[:], in0=ut[:, :, 64:68],
                                        scalar1=1e-16, scalar2=None, op0=AL.add)
                rec = spool.tile([128, nb, 4], F32, tag=f"rec{nb}")
                nc.vector.reciprocal(rec[:], sp[:])
                z = dpool.tile([128, nb, CH], F32, tag=f"z{nb}")
                nc.vector.tensor_mul(
                    z[:].rearrange("p t (h c) -> p t h c", c=16),
                    ut[:, :, 0:CH].rearrange("p t (h c) -> p t h c", c=16),
                    rec[:].unsqueeze(3).broadcast_to([128, nb, 4, 16]))
                nc.vector.tensor_add(
                    z[:], z[:], b1sb[:].unsqueeze(1).broadcast_to([128, nb, CH]))
                # ELU(z) = relu(z) + exp(min(z,0)) - 1
                r = dpool.tile([128, nb, CH], F32, tag=f"r{nb}")
                nc.scalar.activation(r[:], z[:], AF.Relu)
                u = dpool.tile([128, nb, CH], F32, tag=f"u{nb}")
                nc.vector.tensor_scalar(out=u[:], in0=z[:], scalar1=0.0,
                                        scalar2=None, op0=AL.min)
                tE = dpool.tile([128, nb, CH], F32, tag=f"tE{nb}")
                nc.scalar.activation(tE[:], u[:], AF.Exp)
                mid = dpool.tile([128, nb, CH], F32, tag=f"mid{nb}")
                nc.vector.scalar_tensor_tensor(
                    mid[:].rearrange("p a b -> p (a b)"),
                    tE[:].rearrange("p a b -> p (a b)"), -1.0,
                    r[:].rearrange("p a b -> p (a b)"), AL.add, AL.add)
                h2bb = dpool.tile([128, nb, TW], BF16, tag=f"h2bb{nb}")
                nc.vector.memset(h2bb[:, :, CH:TW], 0.0)
                for k, i in enumerate(batch):
                    tp2 = dps.tile([CH, 128], F32, tag="tp2")
                    nc.tensor.transpose(
                        tp2[:],
                        mid[:, k:k + 1, :].rearrange("p a b -> p (a b)"),
                        idn[:])
                    tps2 = dpool.tile([CH, 128], F32, tag="tps2")
                    nc.vector.tensor_copy(tps2[:], tp2[:])
                    ps2 = dps.tile([128, CH], F32, tag="mm")
                    nc.tensor.matmul(ps2[:], tps2[:], w2sb[:])
                    nc.vector.tensor_copy(h2bb[:, k, 0:CH], ps2[:])
                for k, i in enumerate(batch):
                    nc.sync.dma_start(h2loc[i * 128:(i + 1) * 128, :],
                                      h2bb[:, k, :])
                nc.vector.tensor_copy(AB2[:, i0:i0 + nb, :], h2bb[:, :, 41:42])
                # self-loop init for U2 blocks from the same tiles
                e2s = spool.tile([128, nb], F32, tag=f"e2s{nb}")
                nc.vector.tensor_add(
                    e2s[:], h2bb[:, :, 40:41].rearrange("p a b -> p (a b)"),
                    h2bb[:, :, 41:42].rearrange("p a b -> p (a b)"))
                lr2s = spool.tile([128, nb], F32, tag=f"lr2s{nb}")
                nc.vector.scalar_tensor_tensor(
                    lr2s[:], e2s[:], 0.2, e2s[:], AL.mult, AL.max)
                w2s = spool.tile([128, nb], F32, tag=f"w2s{nb}")
                nc.scalar.activation(w2s[:], lr2s[:], AF.Exp)
                nc.vector.tensor_mul(
                    U2[:, i0:i0 + nb, 0:40], h2bb[:, :, 0:40],
                    w2s[:].unsqueeze(2).broadcast_to([128, nb, 40]))
                nc.vector.tensor_copy(U2[:, i0:i0 + nb, 40:41],
                                      w2s[:].unsqueeze(2))

            # ---- F2 (batched, fused into E2 tail): out = U2/S2 + b2 ----
            def f2_batch(batch):
                i0, nb = batch[0], len(batch)
                ut = U2[:, i0:i0 + nb, :]
                sp = spool.tile([128, nb], F32, tag=f"sp2{nb}")
                nc.vector.tensor_scalar(
                    out=sp[:], in0=ut[:, :, 40:41].rearrange("p a b -> p (a b)"),
                    scalar1=1e-16, scalar2=None, op0=AL.add)
                rec = spool.tile([128, nb], F32, tag=f"rec2{nb}")
                nc.vector.reciprocal(rec[:], sp[:])
                ot = dpool.tile([128, nb, 40], F32, tag=f"ot{nb}")
                nc.vector.tensor_mul(
                    ot[:], ut[:, :, 0:40],
                    rec[:].unsqueeze(2).broadcast_to([128, nb, 40]))
                nc.vector.tensor_add(
                    ot[:], ot[:], b2sb[:].unsqueeze(1).broadcast_to([128, nb, 40]))
                for k, i in enumerate(batch):
                    nc.sync.dma_start(OUT[i * 128:(i + 1) * 128, :],
                                      ot[:, k, :])

            def batcher(emit):
                batch = []

                def add(blk):
                    if batch and (blk != batch[-1] + 1 or len(batch) == 4):
                        emit(batch[:])
                        batch.clear()
                    batch.append(blk)

                def flush():
                    if batch:
                        emit(batch[:])
                        batch.clear()
                return add, flush

            # ---- E1 (f1 fused into the final pass per block) ----
            f1_add, f1_flush = batcher(f1_batch)
            edge_pass(0, tiles_l, NGl, gl, dTl, dRl, h1full[0:SPLIT, :], AB1,
                      U1, 4, 68, CH, 0, f1_add)
            edge_pass(1, tiles_h, NGh, gh, dTh, dRh, h1full[SPLIT:NPAD, :],
                      AB1, U1, 4, 68, CH, NGl, f1_add)
            f1_flush()

            # ---- AllGather h2 ----
            nc.gpsimd.collective_compute(
                "AllGather", AL.bypass, groups, [h2loc[:, :]], [h2full[:, :]])

            # ---- E2 (as/ad folded into table cols 40/41; f2 fused) ----
            f2_add, f2_flush = batcher(f2_batch)
            edge_pass(0, tiles_l, NGl, gl, dTl, dRl, h2full[0:SPLIT, :], AB2,
                      U2, 1, 41, 40, 0, f2_add)
            edge_pass(1, tiles_h, NGh, gh, dTh, dRh, h2full[SPLIT:NPAD, :],
                      AB2, U2, 1, 41, 40, NGl, f2_add)
            f2_flush()

            if debug:
                for i in range(NB):
                    r0 = i * 128
                    for src_d, dst_d in ((h1loc, Dh1), (h2loc, Dh2)):
                        tt = dpool.tile([128, TW], BF16, tag="dbg")
                        nc.sync.dma_start(tt[:], src_d[r0:r0 + 128, :])
                        nc.sync.dma_start(dst_d[r0:r0 + 128, :], tt[:])
                    du1 = dpool.tile([128, 68], F32, tag="du1")
                    nc.vector.tensor_copy(du1[:], U1[:, i, :])
                    nc.sync.dma_start(DU1[r0:r0 + 128, :], du1[:])
                    du2 = dpool.tile([128, 41], F32, tag="du2")
                    nc.vector.tensor_copy(du2[:], U2[:, i, :])
                    nc.sync.dma_start(DU2[r0:r0 + 128, :], du2[:])

    nc.finalize()
    return nc


def _wrap_idx(a):
    """int16 [cap] -> wrapped [16, cap/16] replicated to [128, cap/16]."""
    w = a.reshape(-1, 16).T.copy()
    return np.ascontiguousarray(np.tile(w, (8, 1)))


def prep(x, edge_index, W1, a_src1, a_dst1, b1, W2, a_src2, a_dst2, b2):
    """Host-side sharding/index prep. Returns (meta, in_maps, (N, FIN))."""
    x = np.asarray(x, np.float32)
    N, FIN = x.shape
    NLOC = (N + NCORES - 1) // NCORES                       # 6250
    NLOCP = ((NLOC + 127) // 128) * 128                     # 6272
    NB = NLOCP // 128                                       # 49
    NPAD = NCORES * NLOCP
    NBINS = NCORES * NB

    ei0 = np.asarray(edge_index[0]).astype(np.int64)
    ei1 = np.asarray(edge_index[1]).astype(np.int64)

    # balanced bin-pack: nodes -> 392 (core, block) bins by in-degree,
    # so per-block edge counts are ~equal across cores and blocks
    deg = np.bincount(ei1, minlength=N)
    order = np.argsort(-deg, kind="stable")
    heap = [(0, b) for b in range(NBINS)]
    heapq.heapify(heap)
    count = np.zeros(NBINS, np.int64)
    newrow = np.empty(N, np.int64)          # node -> global padded row
    for n in order:
        while True:
            load, b = heapq.heappop(heap)
            if count[b] < 128:
                break
        newrow[n] = b * 128 + count[b]
        count[b] += 1
        heapq.heappush(heap, (load + int(deg[n]), b))

    rmap = newrow[ei0]                       # table row of src
    drow = newrow[ei1]
    core = drow // NLOCP
    dl = drow % NLOCP
    blk_all = dl // 128
    dlm_all = dl % 128

    # per (core, pass): edges sorted by (dst-block, src-row)
    per = []  # [core][pass] = (rs_sorted, blk_sorted, dlm_sorted)
    for c in range(NCORES):
        m = core == c
        rs_c, blk_c, dlm_c = rmap[m], blk_all[m], dlm_all[m]
        lo = rs_c < SPLIT
        rows = []
        for p, sel in enumerate((lo, ~lo)):
            rs, blk, dlm = rs_c[sel], blk_c[sel], dlm_c[sel]
            o = np.lexsort((rs, blk))
            rows.append((rs[o] - (SPLIT if p else 0), blk[o], dlm[o]))
        per.append(rows)

    # uniform tiles-per-(pass, block) across cores
    TPB = np.zeros((2, NB), np.int64)
    for c in range(NCORES):
        for p in range(2):
            cnt = np.bincount(per[c][p][1], minlength=NB)
            TPB[p] = np.maximum(TPB[p], (cnt + 127) // 128)
    last_pass_for_blk = np.where(TPB[1] > 0, 1, 0).tolist()

    def pass_meta(p):
        bids = np.repeat(np.arange(NB), TPB[p])
        NG = (len(bids) + TPG - 1) // TPG
        pad = NG * TPG - len(bids)
        if pad:
            bids = np.concatenate([bids, np.full(pad, bids[-1])])
        first = np.ones(len(bids), bool)
        first[1:] = bids[1:] != bids[:-1]
        last = np.ones(len(bids), bool)
        last[:-1] = bids[1:] != bids[:-1]
        return NG, list(zip(bids.tolist(), first.tolist(), last.tolist()))

    NGl, tiles_l = pass_meta(0)
    NGh, tiles_h = pass_meta(1)
    slot_base = [np.concatenate([[0], np.cumsum(TPB[p]) * 128]) for p in range(2)]

    # ---- constant inputs (replicated) ----
    W1 = np.asarray(W1, np.float32)
    W2p = np.zeros((CH, CH), np.float32)
    W2p[:, :40] = np.asarray(W2, np.float32)
    W2p[:, 40] = W2p[:, :40] @ np.asarray(a_src2, np.float32).reshape(40)
    W2p[:, 41] = W2p[:, :40] @ np.asarray(a_dst2, np.float32).reshape(40)
    IOTAB = np.ascontiguousarray(np.tile(
        np.arange(128, dtype=np.float32)[None, :], (128, 1))).astype(
            ml_dtypes.bfloat16)
    IOTAPB = np.arange(128, dtype=np.float32)[:, None].astype(
        ml_dtypes.bfloat16)
    IDN = np.eye(128, dtype=np.float32)
    as1 = np.asarray(a_src1, np.float32).reshape(CH)
    ad1 = np.asarray(a_dst1, np.float32).reshape(CH)
    asrc1t = np.ascontiguousarray(np.tile(as1[None, :], (128, 1)))
    adst1r = np.ascontiguousarray(np.tile(ad1[None, :], (128, 1)))
    b1r = np.ascontiguousarray(
        np.tile(np.asarray(b1, np.float32)[None, :], (128, 1)))
    b2r = np.ascontiguousarray(
        np.tile(np.asarray(b2, np.float32)[None, :], (128, 1)))

    xpad = np.zeros((NPAD, FIN), np.float32)
    xpad[newrow] = x

    in_maps = []
    for c in range(NCORES):
        packs = []
        for p, NG in ((0, NGl), (1, NGh)):
            slots = NG * GC
            idx_arr = np.zeros(slots, np.int64)
            dloc_arr = np.full(slots, 999.0, np.float32)
            rs, blk, dlm = per[c][p]
            if len(blk):
                starts = np.concatenate([[0], np.cumsum(np.bincount(blk, minlength=NB))])
                rank = np.arange(len(blk)) - starts[blk]
                pos = slot_base[p][blk] + rank
                idx_arr[pos] = rs
                dloc_arr[pos] = dlm.astype(np.float32)
            packs.append((
                _wrap_idx(idx_arr.astype(np.int16)),
                np.ascontiguousarray(
                    dloc_arr.reshape(-1, 128).T).astype(ml_dtypes.bfloat16),
                dloc_arr[None, :].astype(ml_dtypes.bfloat16)))
        xT = np.ascontiguousarray(xpad[c * NLOCP:(c + 1) * NLOCP].T)
        in_maps.append({
            "xTs": xT, "W1": W1, "W2p": W2p, "IOTAB": IOTAB, "IOTAPB": IOTAPB,
            "IDN": IDN,
            "asrc1t": asrc1t, "adst1r": adst1r, "b1r": b1r, "b2r": b2r,
            "gl": packs[0][0], "dTl": packs[0][1], "dRl": packs[0][2],
            "gh": packs[1][0], "dTh": packs[1][1], "dRh": packs[1][2],
        })

    meta = {
        "NLOC": NLOC, "NLOCP": NLOCP, "NB": NB,
        "NGl": NGl, "NGh": NGh, "tiles_l": tiles_l, "tiles_h": tiles_h,
        "last_pass_for_blk": last_pass_for_blk,
        "newrow": newrow,
    }
    return meta, in_maps, (N, FIN)


def kernel(**inputs):
    global LAST_RESULTS
    meta, in_maps, (N, FIN) = prep(**inputs)
    NLOCP = meta["NLOCP"]
    key = (N, FIN, meta["NGl"], meta["NGh"],
           tuple(t[0] for t in meta["tiles_l"]),
           tuple(t[0] for t in meta["tiles_h"]))
    if key not in _prog_cache:
        _prog_cache[key] = _build(meta)
    nc = _prog_cache[key]

    want_trace = bool(os.environ.get("GAT_TRACE"))
    if want_trace:
        try:
            from antenv import axon_hooks  # noqa: F401
        except ImportError:
            want_trace = False
    res = run_bass_kernel_spmd(
        nc, in_maps, core_ids=list(range(NCORES)), trace=want_trace)
    LAST_RESULTS = res
    full = np.concatenate([res.results[c]["OUT"] for c in range(NCORES)], 0)
    return np.ascontiguousarray(full[meta["newrow"]])


# revision 37
# speedup vs baseline: 1.7719x; 1.0045x over previous
"""2-layer GAT on 8 Trainium2 NeuronCores (Bass/Tile, SPMD) — v5.

Strategy (edge-parallel, dst-sharded): destination nodes are bin-packed on
the host into 392 (core, block-of-128) bins balanced by in-degree, so every
block sees ~equal edge work (minimal tile padding, SPMD-uniform program).
Per core, edges are sorted by (src<32768 split, dst-block, src). Node tables
are bf16 [*, 128] rows (256B) with per-src attention logits folded into
spare columns, so one SWDGE gather per edge fetches everything src-side.
Per 128-edge tile the kernel
  - builds one-hot [edge x dst] and its transpose [dst x edge] on DVE
    (is_equal vs iota consts; dst ids arrive in both layouts, the [dst x
    edge] one via a partition-broadcast DMA),
  - gets per-edge dst logits ad_e with one small PE matmul (OHT^T @ ab_blk),
  - computes w = exp(leakyrelu(as_e + ad_e)) on DVE/ACT (ACT writes w
    straight into the msg tile),
  - segment-sums [w*h | w] into PSUM via one PE matmul with the one-hot as
    stationary (no dma_scatter_add RMW).
Self-loop edges never enter the edge stream: their contribution initializes
the per-block U/S SBUF accumulators element-wise during the dense phases.
Softmax is the U/S ratio so no per-edge normalization. Layer boundaries
fuse normalize+ELU+projection per block, interleaved into the tail edge
pass so the PE/DVE work overlaps the remaining gathers. 4 SWDGE queues
round-robin so descriptor generation overlaps DMA.
"""
import heapq
import os
import numpy as np
import ml_dtypes

from concourse import bacc, mybir, tile
from concourse.bass_utils import run_bass_kernel_spmd

NCORES = 8
CH = 64          # feature channels (L2 zero-padded 40->64); table rows 128
TW = 128         # table row width (bf16 -> 256B rows)
GC = int(os.environ.get("GAT_GC", "1024"))  # idxs per SWDGE gather call
TPG = GC // 128  # tiles per gather group
SPLIT = 32768    # int16 gather index reach (rows)
NQ = 4           # SWDGE queues, round-robin over gather calls
F32 = mybir.dt.float32
BF16 = mybir.dt.bfloat16
F8 = mybir.dt.float8e4
I16 = mybir.dt.int16
AL = mybir.AluOpType
AF = mybir.ActivationFunctionType

_prog_cache = {}
LAST_RESULTS = None  # BassKernelResults of the last device run (for test.py)


def _build(meta):
    NLOCP = meta["NLOCP"]
    NB = meta["NB"]
    NPAD = NCORES * NLOCP
    NGl, NGh = meta["NGl"], meta["NGh"]
    tiles_l, tiles_h = meta["tiles_l"], meta["tiles_h"]  # [(blk, first, last)]
    last_pass = meta["last_pass_for_blk"]
    SL, SH = NGl * GC, NGh * GC

    nc = bacc.Bacc(num_devices=NCORES, num_swdge_queues=NQ,
                   dynamic_dma_scratch_size=16 * GC)

    # ---- I/O ----
    xTs = nc.dram_tensor("xTs", [128, NLOCP], F32, kind="ExternalInput")
    W1 = nc.dram_tensor("W1", [128, CH], F32, kind="ExternalInput")
    W2p = nc.dram_tensor("W2p", [CH, CH], F32, kind="ExternalInput")
    IOTAB = nc.dram_tensor("IOTAB", [128, 128], BF16, kind="ExternalInput")
    IOTAPB = nc.dram_tensor("IOTAPB", [128, 1], BF16, kind="ExternalInput")
    IDN = nc.dram_tensor("IDN", [128, 128], F32, kind="ExternalInput")
    asrc1t = nc.dram_tensor("asrc1t", [128, CH], F32, kind="ExternalInput")
    adst1r = nc.dram_tensor("adst1r", [128, CH], F32, kind="ExternalInput")
    b1r = nc.dram_tensor("b1r", [128, CH], F32, kind="ExternalInput")
    b2r = nc.dram_tensor("b2r", [128, 40], F32, kind="ExternalInput")
    gl = nc.dram_tensor("gl", [128, SL // 16], I16, kind="ExternalInput")
    gh = nc.dram_tensor("gh", [128, SH // 16], I16, kind="ExternalInput")
    dTl = nc.dram_tensor("dTl", [128, SL // 128], BF16, kind="ExternalInput")
    dTh = nc.dram_tensor("dTh", [128, SH // 128], BF16, kind="ExternalInput")
    dRl = nc.dram_tensor("dRl", [1, SL], BF16, kind="ExternalInput")
    dRh = nc.dram_tensor("dRh", [1, SH], BF16, kind="ExternalInput")
    OUT = nc.dram_tensor("OUT", [NLOCP, 40], F32, kind="ExternalOutput")

    # ---- scratch ----
    h1loc = nc.dram_tensor("h1loc", [NLOCP, TW], BF16, kind="Internal")
    h1full = nc.dram_tensor("h1full", [NPAD, TW], BF16, kind="Internal",
                            addr_space="Shared")
    h2loc = nc.dram_tensor("h2loc", [NLOCP, TW], BF16, kind="Internal")
    h2full = nc.dram_tensor("h2full", [NPAD, TW], BF16, kind="Internal",
                            addr_space="Shared")

    debug = bool(os.environ.get("GAT_DEBUG"))
    if debug:
        Dh1 = nc.dram_tensor("Dh1", [NLOCP, TW], BF16, kind="ExternalOutput")
        DU1 = nc.dram_tensor("DU1", [NLOCP, 68], F32, kind="ExternalOutput")
        Dh2 = nc.dram_tensor("Dh2", [NLOCP, TW], BF16, kind="ExternalOutput")
        DU2 = nc.dram_tensor("DU2", [NLOCP, 41], F32, kind="ExternalOutput")

    groups = [list(range(NCORES))]

    with tile.TileContext(nc) as tc:
        with (
            tc.tile_pool(name="const", bufs=1) as cpool,
            tc.tile_pool(name="dense", bufs=3) as dpool,
            tc.tile_pool(name="dpsum", bufs=1, space="PSUM") as dps,
            tc.tile_pool(name="apsum", bufs=2, space="PSUM") as aps,
            tc.tile_pool(name="upsum", bufs=3, space="PSUM") as ups,
            tc.tile_pool(name="idx", bufs=4) as ipool,
            tc.tile_pool(name="edge", bufs=4) as epool,
            tc.tile_pool(name="onehot", bufs=4) as opool,
            tc.tile_pool(name="msg", bufs=4) as mpool,
            tc.tile_pool(name="small", bufs=4) as spool,
        ):
            # constants
            w1sb = cpool.tile([128, CH], F32)
            nc.sync.dma_start(w1sb[:], W1[:])
            w2sb = cpool.tile([CH, CH], F32)
            nc.sync.dma_start(w2sb[:], W2p[:])
            iotab = cpool.tile([128, 128], BF16)
            nc.sync.dma_start(iotab[:], IOTAB[:])
            iotapb = cpool.tile([128, 1], BF16)
            nc.sync.dma_start(iotapb[:], IOTAPB[:])
            idn = cpool.tile([128, 128], F32)
            nc.sync.dma_start(idn[:], IDN[:])
            as1sb = cpool.tile([128, CH], F32)
            nc.sync.dma_start(as1sb[:], asrc1t[:])
            ad1sb = cpool.tile([128, CH], F32)
            nc.sync.dma_start(ad1sb[:], adst1r[:])
            b1sb = cpool.tile([128, CH], F32)
            nc.sync.dma_start(b1sb[:], b1r[:])
            b2sb = cpool.tile([128, 40], F32)
            nc.sync.dma_start(b2sb[:], b2r[:])

            # persistent per-block U/S accumulators (SBUF)
            U1 = cpool.tile([128, NB, 68], F32)
            U2 = cpool.tile([128, NB, 41], F32)

            # SBUF-resident per-dst attention tables (written by D1/F1,
            # read by the ad_e matmuls — no DRAM round-trip)
            AB1 = cpool.tile([128, NB, 4], BF16)
            AB2 = cpool.tile([128, NB, 1], BF16)

            # ---- D1 (batched x4): h1 = x @ W1; table row = [h1 | as | 0];
            #      AB1; U1 initialized with the self-loop contribution ----
            for i0 in range(0, NB, 4):
                nb = min(4, NB - i0)
                htf = dpool.tile([128, nb, CH], F32, tag=f"htf{nb}")
                for k in range(nb):
                    r0 = (i0 + k) * 128
                    xt = dpool.tile([128, 128], F32, tag="xt")
                    nc.sync.dma_start(xt[:], xTs[:, r0:r0 + 128])
                    ps = dps.tile([128, CH], F32, tag="mm")
                    nc.tensor.matmul(ps[:], xt[:], w1sb[:])
                    nc.vector.tensor_copy(htf[:, k, :], ps[:])
                htb = dpool.tile([128, nb, TW], BF16, tag=f"htb{nb}")
                nc.vector.tensor_copy(htb[:, :, 0:CH], htf[:])
                tmp = dpool.tile([128, nb, CH], F32, tag=f"tmp{nb}")
                nc.vector.tensor_mul(
                    tmp[:], htf[:],
                    as1sb[:].unsqueeze(1).broadcast_to([128, nb, CH]))
                asf = spool.tile([128, nb, 4], F32, tag=f"asf{nb}")
                nc.vector.tensor_reduce(
                    asf[:], tmp[:].rearrange("p t (h c) -> p t h c", c=16),
                    mybir.AxisListType.X, AL.add)
                nc.vector.tensor_copy(htb[:, :, CH:CH + 4], asf[:])
                nc.vector.memset(htb[:, :, CH + 4:TW], 0.0)
                tmp2 = dpool.tile([128, nb, CH], F32, tag=f"tmp2{nb}")
                nc.vector.tensor_mul(
                    tmp2[:], htf[:],
                    ad1sb[:].unsqueeze(1).broadcast_to([128, nb, CH]))
                dpf = spool.tile([128, nb, 4], F32, tag=f"dpf{nb}")
                nc.vector.tensor_reduce(
                    dpf[:], tmp2[:].rearrange("p t (h c) -> p t h c", c=16),
                    mybir.AxisListType.X, AL.add)
                nc.vector.tensor_copy(AB1[:, i0:i0 + nb, :], dpf[:])
                for k in range(nb):
                    r0 = (i0 + k) * 128
                    nc.sync.dma_start(h1loc[r0:r0 + 128, :], htb[:, k, :])
                # self-loop: U1 = [w*h | w], w = exp(lrelu(as+ad))
                e0 = spool.tile([128, nb, 4], F32, tag=f"e0{nb}")
                nc.vector.tensor_add(e0[:], asf[:], dpf[:])
                lr0 = spool.tile([128, nb, 4], F32, tag=f"lr0{nb}")
                nc.vector.scalar_tensor_tensor(
                    lr0[:].rearrange("p a b -> p (a b)"),
                    e0[:].rearrange("p a b -> p (a b)"), 0.2,
                    e0[:].rearrange("p a b -> p (a b)"), AL.mult, AL.max)
                w0 = spool.tile([128, nb, 4], F32, tag=f"w0{nb}")
                nc.scalar.activation(w0[:], lr0[:], AF.Exp)
                nc.vector.tensor_mul(
                    U1[:, i0:i0 + nb, 0:CH].rearrange("p t (h c) -> p t h c", c=16),
                    htf[:].rearrange("p t (h c) -> p t h c", c=16),
                    w0[:].unsqueeze(3).broadcast_to([128, nb, 4, 16]))
                nc.vector.tensor_copy(U1[:, i0:i0 + nb, CH:CH + 4], w0[:])

            # ---- AllGather h1 ----
            nc.gpsimd.collective_compute(
                "AllGather", AL.bypass, groups, [h1loc[:, :]], [h1full[:, :]])

            def edge_pass(pass_id, tiles, NG, gidx, dT, dR, base, abt, U, H,
                          UW, as_col, qoff, fblock):
                """One lo/hi pass over all dst blocks of one layer.
                tiles: [(blk, seg_first, seg_last)] per tile slot.
                base: gather base AP; abt: [128, NB, H] bf16 SBUF per-dst
                logits. U: [128, NB, UW] f32 SBUF accumulator (pre-init
                with the self-loop term). as_col: first table column holding
                the H per-src logits. fblock(blk): emitted when blk's U is
                complete (layer-boundary fusion)."""
                U_ps = None
                for g in range(NG):
                    it = ipool.tile([128, GC // 16], I16, tag="it")
                    nc.sync.dma_start(it[:], gidx[:, g * (GC // 16):(g + 1) * (GC // 16)])
                    dt = ipool.tile([128, TPG], BF16, tag="dt")
                    nc.sync.dma_start(dt[:], dT[:, g * TPG:(g + 1) * TPG])
                    # dst ids replicated to all partitions ([d, e] layout);
                    # issued on the Activation HWDGE queue to offload Sync
                    db = ipool.tile([128, GC], BF16, tag="db")
                    nc.scalar.dma_start(
                        db[:],
                        dR[0:1, g * GC:(g + 1) * GC].broadcast_to([128, GC]))
                    G = epool.tile([128, TPG, TW], BF16, tag="G")
                    nc.gpsimd.dma_gather(
                        G[:], base, it[:], GC, GC, TW,
                        queue_num=(g + qoff) % NQ)

                    # one-hot [e, d] and transposed [d, e] (pads match nothing)
                    OH = opool.tile([128, TPG, 128], F8, tag="OH")
                    nc.vector.tensor_tensor(
                        OH[:],
                        iotab[:].unsqueeze(1).broadcast_to([128, TPG, 128]),
                        dt[:].unsqueeze(2).broadcast_to([128, TPG, 128]),
                        AL.is_equal)
                    OHT = opool.tile([128, TPG, 128], F8, tag="OHT")
                    nc.vector.tensor_tensor(
                        OHT[:],
                        db[:].rearrange("p (t e) -> p t e", e=128),
                        iotapb[:].unsqueeze(2).broadcast_to([128, TPG, 128]),
                        AL.is_equal)

                    # ad_e = OHT^T @ ab_blk (ab resident in SBUF)
                    ad_ps = aps.tile([128, TPG, H], F32, tag="ad")
                    for t in range(TPG):
                        blk = tiles[g * TPG + t][0]
                        nc.tensor.matmul(
                            ad_ps[:, t:t + 1, :].rearrange("p a b -> p (a b)"),
                            OHT[:, t:t + 1, :].rearrange("p a b -> p (a b)"),
                            abt[:, blk, :])

                    # w = exp(leakyrelu(as_e + ad_e)); ACT writes w into msg
                    e = spool.tile([128, TPG, H], F32, tag="e")
                    nc.vector.tensor_add(
                        e[:], G[:, :, as_col:as_col + H], ad_ps[:])
                    lr = spool.tile([128, TPG * H], F32, tag="lr")
                    nc.vector.scalar_tensor_tensor(
                        lr[:], e[:].rearrange("p a b -> p (a b)"), 0.2,
                        e[:].rearrange("p a b -> p (a b)"), AL.mult, AL.max)
                    wt = spool.tile([128, TPG * H], BF16, tag="wt")
                    nc.scalar.activation(wt[:], lr[:], AF.Exp)
                    wb = wt[:].rearrange("p (t h) -> p t h", h=H)
                    msg = mpool.tile([128, TPG, UW], BF16, tag="msg")
                    nc.scalar.activation(
                        msg[:, :, UW - H:UW], lr[:].rearrange(
                            "p (t h) -> p t h", h=H), AF.Exp)
                    if H > 1:
                        nc.vector.tensor_mul(
                            msg[:, :, 0:CH].rearrange("p t (h c) -> p t h c", c=CH // H),
                            G[:, :, 0:CH].rearrange("p t (h c) -> p t h c", c=CH // H),
                            wb.unsqueeze(3).broadcast_to([128, TPG, H, CH // H]))
                    else:
                        nc.vector.tensor_mul(
                            msg[:, :, 0:UW - 1], G[:, :, 0:UW - 1],
                            wb.broadcast_to([128, TPG, UW - 1]))

                    # segment-sum via PE: U_ps[d, :] += OH^T @ msg
                    for t in range(TPG):
                        blk, sfirst, slast = tiles[g * TPG + t]
                        if sfirst:
                            U_ps = ups.tile([128, UW], F32, tag="ups")
                        nc.tensor.matmul(
                            U_ps[:],
                            OH[:, t:t + 1, :].rearrange("p a b -> p (a b)"),
                            msg[:, t:t + 1, :].rearrange("p a b -> p (a b)"),
                            start=sfirst, stop=slast)
                        if slast:
                            nc.vector.tensor_add(
                                U[:, blk, :], U[:, blk, :], U_ps[:])
                            if fblock is not None and last_pass[blk] == pass_id:
                                fblock(blk)
                return NG

            # ---- F1 (batched over runs of completed blocks, fused into the
            #      E1 tail): z = U/S + b1; ELU; h2 = mid @ W2p; table + ab2;
            #      U2 self-loop init ----
            def f1_batch(batch):
                i0, nb = batch[0], len(batch)
                ut = U1[:, i0:i0 + nb, :]
                sp = spool.tile([128, nb, 4], F32, tag=f"sp{nb}")
                nc.vector.tensor_scalar(out=sp[:], in0=ut[:, :, 64:68],
                                        scalar1=1e-16, scalar2=None, op0=AL.add)
                rec = spool.tile([128, nb, 4], F32, tag=f"rec{nb}")
                nc.vector.reciprocal(rec[:], sp[:])
                z = dpool.tile([128, nb, CH], F32, tag=f"z{nb}")
                nc.vector.tensor_mul(
                    z[:].rearrange("p t (h c) -> p t h c", c=16),
                    ut[:, :, 0:CH].rearrange("p t (h c) -> p t h c", c=16),
                    rec[:].unsqueeze(3).broadcast_to([128, nb, 4, 16]))
                nc.vector.tensor_add(
                    z[:], z[:], b1sb[:].unsqueeze(1).broadcast_to([128, nb, CH]))
                # ELU(z) = relu(z) + exp(min(z,0)) - 1
                r = dpool.tile([128, nb, CH], F32, tag=f"r{nb}")
                nc.scalar.activation(r[:], z[:], AF.Relu)
                u = dpool.tile([128, nb, CH], F32, tag=f"u{nb}")
                nc.vector.tensor_scalar(out=u[:], in0=z[:], scalar1=0.0,
                                        scalar2=None, op0=AL.min)
                tE = dpool.tile([128, nb, CH], F32, tag=f"tE{nb}")
                nc.scalar.activation(tE[:], u[:], AF.Exp)
                mid = dpool.tile([128, nb, CH], F32, tag=f"mid{nb}")
                nc.vector.scalar_tensor_tensor(
                    mid[:].rearrange("p a b -> p (a b)"),
                    tE[:].rearrange("p a b -> p (a b)"), -1.0,
                    r[:].rearrange("p a b -> p (a b)"), AL.add, AL.add)
                h2bb = dpool.tile([128, nb, TW], BF16, tag=f"h2bb{nb}")
                nc.vector.memset(h2bb[:, :, CH:TW], 0.0)
                for k, i in enumerate(batch):
                    tp2 = dps.tile([CH, 128], F32, tag="tp2")
                    nc.tensor.transpose(
                        tp2[:],
                        mid[:, k:k + 1, :].rearrange("p a b -> p (a b)"),
                        idn[:])
                    tps2 = dpool.tile([CH, 128], F32, tag="tps2")
                    nc.vector.tensor_copy(tps2[:], tp2[:])
                    ps2 = dps.tile([128, CH], F32, tag="mm")
                    nc.tensor.matmul(ps2[:], tps2[:], w2sb[:])
                    nc.vector.tensor_copy(h2bb[:, k, 0:CH], ps2[:])
                for k, i in enumerate(batch):
                    nc.sync.dma_start(h2loc[i * 128:(i + 1) * 128, :],
                                      h2bb[:, k, :])
                nc.vector.tensor_copy(AB2[:, i0:i0 + nb, :], h2bb[:, :, 41:42])
                # self-loop init for U2 blocks from the same tiles
                e2s = spool.tile([128, nb], F32, tag=f"e2s{nb}")
                nc.vector.tensor_add(
                    e2s[:], h2bb[:, :, 40:41].rearrange("p a b -> p (a b)"),
                    h2bb[:, :, 41:42].rearrange("p a b -> p (a b)"))
                lr2s = spool.tile([128, nb], F32, tag=f"lr2s{nb}")
                nc.vector.scalar_tensor_tensor(
                    lr2s[:], e2s[:], 0.2, e2s[:], AL.mult, AL.max)
                w2s = spool.tile([128, nb], F32, tag=f"w2s{nb}")
                nc.scalar.activation(w2s[:], lr2s[:], AF.Exp)
                nc.vector.tensor_mul(
                    U2[:, i0:i0 + nb, 0:40], h2bb[:, :, 0:40],
                    w2s[:].unsqueeze(2).broadcast_to([128, nb, 40]))
                nc.vector.tensor_copy(U2[:, i0:i0 + nb, 40:41],
                                      w2s[:].unsqueeze(2))

            # ---- F2 (batched, fused into E2 tail): out = U2/S2 + b2 ----
            def f2_batch(batch):
                i0, nb = batch[0], len(batch)
                ut = U2[:, i0:i0 + nb, :]
                sp = spool.tile([128, nb], F32, tag=f"sp2{nb}")
                nc.vector.tensor_scalar(
                    out=sp[:], in0=ut[:, :, 40:41].rearrange("p a b -> p (a b)"),
                    scalar1=1e-16, scalar2=None, op0=AL.add)
                rec = spool.tile([128, nb], F32, tag=f"rec2{nb}")
                nc.vector.reciprocal(rec[:], sp[:])
                ot = dpool.tile([128, nb, 40], F32, tag=f"ot{nb}")
                nc.vector.tensor_mul(
                    ot[:], ut[:, :, 0:40],
                    rec[:].unsqueeze(2).broadcast_to([128, nb, 40]))
                nc.vector.tensor_add(
                    ot[:], ot[:], b2sb[:].unsqueeze(1).broadcast_to([128, nb, 40]))
                for k, i in enumerate(batch):
                    nc.sync.dma_start(OUT[i * 128:(i + 1) * 128, :],
                                      ot[:, k, :])

            def batcher(emit):
                batch = []

                def add(blk):
                    if batch and (blk != batch[-1] + 1 or len(batch) == 4):
                        emit(batch[:])
                        batch.clear()
                    batch.append(blk)

                def flush():
                    if batch:
                        emit(batch[:])
                        batch.clear()
                return add, flush

            # ---- E1 (f1 fused into the final pass per block) ----
            f1_add, f1_flush = batcher(f1_batch)
            edge_pass(0, tiles_l, NGl, gl, dTl, dRl, h1full[0:SPLIT, :], AB1,
                      U1, 4, 68, CH, 0, f1_add)
            edge_pass(1, tiles_h, NGh, gh, dTh, dRh, h1full[SPLIT:NPAD, :],
                      AB1, U1, 4, 68, CH, NGl, f1_add)
            f1_flush()

            # ---- AllGather h2 ----
            nc.gpsimd.collective_compute(
                "AllGather", AL.bypass, groups, [h2loc[:, :]], [h2full[:, :]])

            # ---- E2 (as/ad folded into table cols 40/41; f2 fused) ----
            f2_add, f2_flush = batcher(f2_batch)
            edge_pass(0, tiles_l, NGl, gl, dTl, dRl, h2full[0:SPLIT, :], AB2,
                      U2, 1, 41, 40, 0, f2_add)
            edge_pass(1, tiles_h, NGh, gh, dTh, dRh, h2full[SPLIT:NPAD, :],
                      AB2, U2, 1, 41, 40, NGl, f2_add)
            f2_flush()

            if debug:
                for i in range(NB):
                    r0 = i * 128
                    for src_d, dst_d in ((h1loc, Dh1), (h2loc, Dh2)):
                        tt = dpool.tile([128, TW], BF16, tag="dbg")
                        nc.sync.dma_start(tt[:], src_d[r0:r0 + 128, :])
                        nc.sync.dma_start(dst_d[r0:r0 + 128, :], tt[:])
                    du1 = dpool.tile([128, 68], F32, tag="du1")
                    nc.vector.tensor_copy(du1[:], U1[:, i, :])
                    nc.sync.dma_start(DU1[r0:r0 + 128, :], du1[:])
                    du2 = dpool.tile([128, 41], F32, tag="du2")
                    nc.vector.tensor_copy(du2[:], U2[:, i, :])
                    nc.sync.dma_start(DU2[r0:r0 + 128, :], du2[:])

    nc.finalize()
    return nc


def _wrap_idx(a):
    """int16 [cap] -> wrapped [16, cap/16] replicated to [128, cap/16]."""
    w = a.reshape(-1, 16).T.copy()
    return np.ascontiguousarray(np.tile(w, (8, 1)))


def prep(x, edge_index, W1, a_src1, a_dst1, b1, W2, a_src2, a_dst2, b2):
    """Host-side sharding/index prep. Returns (meta, in_maps, (N, FIN))."""
    x = np.asarray(x, np.float32)
    N, FIN = x.shape
    NLOC = (N + NCORES - 1) // NCORES                       # 6250
    NLOCP = ((NLOC + 127) // 128) * 128                     # 6272
    NB = NLOCP // 128                                       # 49
    NPAD = NCORES * NLOCP
    NBINS = NCORES * NB

    ei0 = np.asarray(edge_index[0]).astype(np.int64)
    ei1 = np.asarray(edge_index[1]).astype(np.int64)

    # balanced bin-pack: nodes -> 392 (core, block) bins by in-degree,
    # so per-block edge counts are ~equal across cores and blocks
    deg = np.bincount(ei1, minlength=N)
    order = np.argsort(-deg, kind="stable")
    heap = [(0, b) for b in range(NBINS)]
    heapq.heapify(heap)
    count = np.zeros(NBINS, np.int64)
    newrow = np.empty(N, np.int64)          # node -> global padded row
    for n in order:
        while True:
            load, b = heapq.heappop(heap)
            if count[b] < 128:
                break
        newrow[n] = b * 128 + count[b]
        count[b] += 1
        heapq.heappush(heap, (load + int(deg[n]), b))

    rmap = newrow[ei0]                       # table row of src
    drow = newrow[ei1]
    core = drow // NLOCP
    dl = drow % NLOCP
    blk_all = dl // 128
    dlm_all = dl % 128

    # per (core, pass): edges sorted by (dst-block, src-row)
    per = []  # [core][pass] = (rs_sorted, blk_sorted, dlm_sorted)
    for c in range(NCORES):
        m = core == c
        rs_c, blk_c, dlm_c = rmap[m], blk_all[m], dlm_all[m]
        lo = rs_c < SPLIT
        rows = []
        for p, sel in enumerate((lo, ~lo)):
            rs, blk, dlm = rs_c[sel], blk_c[sel], dlm_c[sel]
            o = np.lexsort((rs, blk))
            rows.append((rs[o] - (SPLIT if p else 0), blk[o], dlm[o]))
        per.append(rows)

    # uniform tiles-per-(pass, block) across cores
    TPB = np.zeros((2, NB), np.int64)
    for c in range(NCORES):
        for p in range(2):
            cnt = np.bincount(per[c][p][1], minlength=NB)
            TPB[p] = np.maximum(TPB[p], (cnt + 127) // 128)
    last_pass_for_blk = np.where(TPB[1] > 0, 1, 0).tolist()

    def pass_meta(p):
        bids = np.repeat(np.arange(NB), TPB[p])
        NG = (len(bids) + TPG - 1) // TPG
        pad = NG * TPG - len(bids)
        if pad:
            bids = np.concatenate([bids, np.full(pad, bids[-1])])
        first = np.ones(len(bids), bool)
        first[1:] = bids[1:] != bids[:-1]
        last = np.ones(len(bids), bool)
        last[:-1] = bids[1:] != bids[:-1]
        return NG, list(zip(bids.tolist(), first.tolist(), last.tolist()))

    NGl, tiles_l = pass_meta(0)
    NGh, tiles_h = pass_meta(1)
    slot_base = [np.concatenate([[0], np.cumsum(TPB[p]) * 128]) for p in range(2)]

    # ---- constant inputs (replicated) ----
    W1 = np.asarray(W1, np.float32)
    W2p = np.zeros((CH, CH), np.float32)
    W2p[:, :40] = np.asarray(W2, np.float32)
    W2p[:, 40] = W2p[:, :40] @ np.asarray(a_src2, np.float32).reshape(40)
    W2p[:, 41] = W2p[:, :40] @ np.asarray(a_dst2, np.float32).reshape(40)
    IOTAB = np.ascontiguousarray(np.tile(
        np.arange(128, dtype=np.float32)[None, :], (128, 1))).astype(
            ml_dtypes.bfloat16)
    IOTAPB = np.arange(128, dtype=np.float32)[:, None].astype(
        ml_dtypes.bfloat16)
    IDN = np.eye(128, dtype=np.float32)
    as1 = np.asarray(a_src1, np.float32).reshape(CH)
    ad1 = np.asarray(a_dst1, np.float32).reshape(CH)
    asrc1t = np.ascontiguousarray(np.tile(as1[None, :], (128, 1)))
    adst1r = np.ascontiguousarray(np.tile(ad1[None, :], (128, 1)))
    b1r = np.ascontiguousarray(
        np.tile(np.asarray(b1, np.float32)[None, :], (128, 1)))
    b2r = np.ascontiguousarray(
        np.tile(np.asarray(b2, np.float32)[None, :], (128, 1)))

    xpad = np.zeros((NPAD, FIN), np.float32)
    xpad[newrow] = x

    in_maps = []
    for c in range(NCORES):
        packs = []
        for p, NG in ((0, NGl), (1, NGh)):
            slots = NG * GC
            idx_arr = np.zeros(slots, np.int64)
            dloc_arr = np.full(slots, 999.0, np.float32)
            rs, blk, dlm = per[c][p]
            if len(blk):
                starts = np.concatenate([[0], np.cumsum(np.bincount(blk, minlength=NB))])
                rank = np.arange(len(blk)) - starts[blk]
                pos = slot_base[p][blk] + rank
                idx_arr[pos] = rs
                dloc_arr[pos] = dlm.astype(np.float32)
            packs.append((
                _wrap_idx(idx_arr.astype(np.int16)),
                np.ascontiguousarray(
                    dloc_arr.reshape(-1, 128).T).astype(ml_dtypes.bfloat16),
                dloc_arr[None, :].astype(ml_dtypes.bfloat16)))
        xT = np.ascontiguousarray(xpad[c * NLOCP:(c + 1) * NLOCP].T)
        in_maps.append({
            "xTs": xT, "W1": W1, "W2p": W2p, "IOTAB": IOTAB, "IOTAPB": IOTAPB,
            "IDN": IDN,
            "asrc1t": asrc1t, "adst1r": adst1r, "b1r": b1r, "b2r": b2r,
            "gl": packs[0][0], "dTl": packs[0][1], "dRl": packs[0][2],
            "gh": packs[1][0], "dTh": packs[1][1], "dRh": packs[1][2],
        })

    meta = {
        "NLOC": NLOC, "NLOCP": NLOCP, "NB": NB,
        "NGl": NGl, "NGh": NGh, "tiles_l": tiles_l, "tiles_h": tiles_h,
        "last_pass_for_blk": last_pass_for_blk,
        "newrow": newrow,
    }
    return meta, in_maps, (N, FIN)


def kernel(**inputs):
    global LAST_RESULTS
    meta, in_maps, (N, FIN) = prep(**inputs)
    NLOCP = meta["NLOCP"]
    key = (N, FIN, meta["NGl"], meta["NGh"],
           tuple(t[0] for t in meta["tiles_l"]),
           tuple(t[0] for t in meta["tiles_h"]))
    if key not in _prog_cache:
        _prog_cache[key] = _build(meta)
    nc = _prog_cache[key]

    want_trace = bool(os.environ.get("GAT_TRACE"))
    if want_trace:
        try:
            from antenv import axon_hooks  # noqa: F401
        except ImportError:
            want_trace = False
    res = run_bass_kernel_spmd(
        nc, in_maps, core_ids=list(range(NCORES)), trace=want_trace)
    LAST_RESULTS = res
    full = np.concatenate([res.results[c]["OUT"] for c in range(NCORES)], 0)
    return np.ascontiguousarray(full[meta["newrow"]])


# revision 46
# speedup vs baseline: 1.8867x; 1.0648x over previous
"""2-layer GAT on 8 Trainium2 NeuronCores (Bass/Tile, SPMD) — v5.

Strategy (edge-parallel, dst-sharded): destination nodes are bin-packed on
the host into 392 (core, block-of-128) bins balanced by in-degree, so every
block sees ~equal edge work (minimal tile padding, SPMD-uniform program).
Per core, edges are sorted by (src<32768 split, dst-block, src). Node tables
are bf16 [*, 128] rows (256B) with per-src attention logits folded into
spare columns, so one SWDGE gather per edge fetches everything src-side.
Per 128-edge tile the kernel
  - builds one-hot [edge x dst] and its transpose [dst x edge] on DVE
    (is_equal vs iota consts; dst ids arrive in both layouts, the [dst x
    edge] one via a partition-broadcast DMA),
  - gets per-edge dst logits ad_e with one small PE matmul (OHT^T @ ab_blk),
  - computes w = exp(leakyrelu(as_e + ad_e)) on DVE/ACT (ACT writes w
    straight into the msg tile),
  - segment-sums [w*h | w] into PSUM via one PE matmul with the one-hot as
    stationary (no dma_scatter_add RMW).
Self-loop edges never enter the edge stream: their contribution initializes
the per-block U/S SBUF accumulators element-wise during the dense phases.
Softmax is the U/S ratio so no per-edge normalization. Layer boundaries
fuse normalize+ELU+projection per block, interleaved into the tail edge
pass so the PE/DVE work overlaps the remaining gathers. 4 SWDGE queues
round-robin so descriptor generation overlaps DMA.
"""
import heapq
import os
import numpy as np
import ml_dtypes

from concourse import bacc, mybir, tile
from concourse.bass_utils import run_bass_kernel_spmd

NCORES = 8
CH = 64          # feature channels (L2 zero-padded 40->64); table rows 128
TW = 128         # table row width (bf16 -> 256B rows)
GC = int(os.environ.get("GAT_GC", "1024"))  # idxs per SWDGE gather call
TPG = GC // 128  # tiles per gather group
HBA = 25         # blocks per core in table half A (half B gets NB - HBA)
NQ = 4           # SWDGE queues, round-robin over gather calls
F32 = mybir.dt.float32
BF16 = mybir.dt.bfloat16
F8 = mybir.dt.float8e4
I16 = mybir.dt.int16
AL = mybir.AluOpType
AF = mybir.ActivationFunctionType

_prog_cache = {}
LAST_RESULTS = None  # BassKernelResults of the last device run (for test.py)


def _build(meta):
    NLOCP = meta["NLOCP"]
    NB = meta["NB"]
    NPAD = NCORES * NLOCP
    NGl, NGh = meta["NGl"], meta["NGh"]
    tiles_l, tiles_h = meta["tiles_l"], meta["tiles_h"]  # [(blk, first, last)]
    last_pass = meta["last_pass_for_blk"]
    SL, SH = NGl * GC, NGh * GC
    HA, HB = HBA * 128, (NB - HBA) * 128

    nc = bacc.Bacc(num_devices=NCORES, num_swdge_queues=NQ,
                   dynamic_dma_scratch_size=16 * GC)

    # ---- I/O ----
    xTs = nc.dram_tensor("xTs", [128, NLOCP], F32, kind="ExternalInput")
    W1 = nc.dram_tensor("W1", [128, CH], F32, kind="ExternalInput")
    W2p = nc.dram_tensor("W2p", [CH, CH], F32, kind="ExternalInput")
    IOTAB = nc.dram_tensor("IOTAB", [128, 128], BF16, kind="ExternalInput")
    IOTAPB = nc.dram_tensor("IOTAPB", [128, 1], BF16, kind="ExternalInput")
    IDN = nc.dram_tensor("IDN", [128, 128], F32, kind="ExternalInput")
    asrc1t = nc.dram_tensor("asrc1t", [128, CH], F32, kind="ExternalInput")
    adst1r = nc.dram_tensor("adst1r", [128, CH], F32, kind="ExternalInput")
    b1r = nc.dram_tensor("b1r", [128, CH], F32, kind="ExternalInput")
    b2r = nc.dram_tensor("b2r", [128, 40], F32, kind="ExternalInput")
    gl = nc.dram_tensor("gl", [128, SL // 16], I16, kind="ExternalInput")
    gh = nc.dram_tensor("gh", [128, SH // 16], I16, kind="ExternalInput")
    dTl = nc.dram_tensor("dTl", [128, SL // 128], BF16, kind="ExternalInput")
    dTh = nc.dram_tensor("dTh", [128, SH // 128], BF16, kind="ExternalInput")
    dRl = nc.dram_tensor("dRl", [1, SL], BF16, kind="ExternalInput")
    dRh = nc.dram_tensor("dRh", [1, SH], BF16, kind="ExternalInput")
    OUT = nc.dram_tensor("OUT", [NLOCP, 40], F32, kind="ExternalOutput")

    # ---- scratch ----
    h1locA = nc.dram_tensor("h1locA", [HA, TW], BF16, kind="Internal")
    h1locB = nc.dram_tensor("h1locB", [HB, TW], BF16, kind="Internal")
    h1fA = nc.dram_tensor("h1fA", [NCORES * HA, TW], BF16, kind="Internal",
                          addr_space="Shared")
    h1fB = nc.dram_tensor("h1fB", [NCORES * HB, TW], BF16, kind="Internal",
                          addr_space="Shared")
    h2locA = nc.dram_tensor("h2locA", [HA, TW], BF16, kind="Internal")
    h2locB = nc.dram_tensor("h2locB", [HB, TW], BF16, kind="Internal")
    h2fA = nc.dram_tensor("h2fA", [NCORES * HA, TW], BF16, kind="Internal",
                          addr_space="Shared")
    h2fB = nc.dram_tensor("h2fB", [NCORES * HB, TW], BF16, kind="Internal",
                          addr_space="Shared")

    def hloc_slice(hA, hB, i):
        return (hA[i * 128:(i + 1) * 128, :] if i < HBA
                else hB[(i - HBA) * 128:(i - HBA + 1) * 128, :])

    debug = bool(os.environ.get("GAT_DEBUG"))
    if debug:
        Dh1 = nc.dram_tensor("Dh1", [NLOCP, TW], BF16, kind="ExternalOutput")
        DU1 = nc.dram_tensor("DU1", [NLOCP, 68], F32, kind="ExternalOutput")
        Dh2 = nc.dram_tensor("Dh2", [NLOCP, TW], BF16, kind="ExternalOutput")
        DU2 = nc.dram_tensor("DU2", [NLOCP, 41], F32, kind="ExternalOutput")

    groups = [list(range(NCORES))]

    with tile.TileContext(nc) as tc:
        with (
            tc.tile_pool(name="const", bufs=1) as cpool,
            tc.tile_pool(name="dense", bufs=3) as dpool,
            tc.tile_pool(name="dpsum", bufs=1, space="PSUM") as dps,
            tc.tile_pool(name="apsum", bufs=2, space="PSUM") as aps,
            tc.tile_pool(name="upsum", bufs=3, space="PSUM") as ups,
            tc.tile_pool(name="idx", bufs=4) as ipool,
            tc.tile_pool(name="edge", bufs=4) as epool,
            tc.tile_pool(name="onehot", bufs=4) as opool,
            tc.tile_pool(name="msg", bufs=4) as mpool,
            tc.tile_pool(name="small", bufs=4) as spool,
        ):
            # constants
            w1sb = cpool.tile([128, CH], F32)
            nc.sync.dma_start(w1sb[:], W1[:])
            w2sb = cpool.tile([CH, CH], F32)
            nc.sync.dma_start(w2sb[:], W2p[:])
            iotab = cpool.tile([128, 128], BF16)
            nc.sync.dma_start(iotab[:], IOTAB[:])
            iotapb = cpool.tile([128, 1], BF16)
            nc.sync.dma_start(iotapb[:], IOTAPB[:])
            idn = cpool.tile([128, 128], F32)
            nc.sync.dma_start(idn[:], IDN[:])
            as1sb = cpool.tile([128, CH], F32)
            nc.sync.dma_start(as1sb[:], asrc1t[:])
            ad1sb = cpool.tile([128, CH], F32)
            nc.sync.dma_start(ad1sb[:], adst1r[:])
            b1sb = cpool.tile([128, CH], F32)
            nc.sync.dma_start(b1sb[:], b1r[:])
            b2sb = cpool.tile([128, 40], F32)
            nc.sync.dma_start(b2sb[:], b2r[:])

            # persistent per-block U/S accumulators (SBUF)
            U1 = cpool.tile([128, NB, 68], F32)
            U2 = cpool.tile([128, NB, 41], F32)

            # SBUF-resident per-dst attention tables (written by D1/F1,
            # read by the ad_e matmuls — no DRAM round-trip)
            AB1 = cpool.tile([128, NB, 4], BF16)
            AB2 = cpool.tile([128, NB, 1], BF16)

            # ---- D1 (batched x4): h1 = x @ W1; table row = [h1 | as | 0];
            #      AB1; U1 initialized with the self-loop contribution ----
            for i0 in range(0, NB, 4):
                nb = min(4, NB - i0)
                htf = dpool.tile([128, nb, CH], F32, tag=f"htf{nb}")
                for k in range(nb):
                    r0 = (i0 + k) * 128
                    xt = dpool.tile([128, 128], F32, tag="xt")
                    nc.sync.dma_start(xt[:], xTs[:, r0:r0 + 128])
                    ps = dps.tile([128, CH], F32, tag="mm")
                    nc.tensor.matmul(ps[:], xt[:], w1sb[:])
                    nc.vector.tensor_copy(htf[:, k, :], ps[:])
                htb = dpool.tile([128, nb, TW], BF16, tag=f"htb{nb}")
                nc.vector.tensor_copy(htb[:, :, 0:CH], htf[:])
                tmp = dpool.tile([128, nb, CH], F32, tag=f"tmp{nb}")
                nc.vector.tensor_mul(
                    tmp[:], htf[:],
                    as1sb[:].unsqueeze(1).broadcast_to([128, nb, CH]))
                asf = spool.tile([128, nb, 4], F32, tag=f"asf{nb}")
                nc.vector.tensor_reduce(
                    asf[:], tmp[:].rearrange("p t (h c) -> p t h c", c=16),
                    mybir.AxisListType.X, AL.add)
                nc.vector.tensor_copy(htb[:, :, CH:CH + 4], asf[:])
                nc.vector.memset(htb[:, :, CH + 4:TW], 0.0)
                tmp2 = dpool.tile([128, nb, CH], F32, tag=f"tmp2{nb}")
                nc.vector.tensor_mul(
                    tmp2[:], htf[:],
                    ad1sb[:].unsqueeze(1).broadcast_to([128, nb, CH]))
                dpf = spool.tile([128, nb, 4], F32, tag=f"dpf{nb}")
                nc.vector.tensor_reduce(
                    dpf[:], tmp2[:].rearrange("p t (h c) -> p t h c", c=16),
                    mybir.AxisListType.X, AL.add)
                nc.vector.tensor_copy(AB1[:, i0:i0 + nb, :], dpf[:])
                for k in range(nb):
                    nc.sync.dma_start(hloc_slice(h1locA, h1locB, i0 + k),
                                      htb[:, k, :])
                # self-loop: U1 = [w*h | w], w = exp(lrelu(as+ad))
                e0 = spool.tile([128, nb, 4], F32, tag=f"e0{nb}")
                nc.vector.tensor_add(e0[:], asf[:], dpf[:])
                lr0 = spool.tile([128, nb, 4], F32, tag=f"lr0{nb}")
                nc.vector.scalar_tensor_tensor(
                    lr0[:].rearrange("p a b -> p (a b)"),
                    e0[:].rearrange("p a b -> p (a b)"), 0.2,
                    e0[:].rearrange("p a b -> p (a b)"), AL.mult, AL.max)
                w0 = spool.tile([128, nb, 4], F32, tag=f"w0{nb}")
                nc.scalar.activation(w0[:], lr0[:], AF.Exp)
                nc.vector.tensor_mul(
                    U1[:, i0:i0 + nb, 0:CH].rearrange("p t (h c) -> p t h c", c=16),
                    htf[:].rearrange("p t (h c) -> p t h c", c=16),
                    w0[:].unsqueeze(3).broadcast_to([128, nb, 4, 16]))
                nc.vector.tensor_copy(U1[:, i0:i0 + nb, CH:CH + 4], w0[:])
                # chunked AllGather: half A ships while D1 computes half B
                if i0 <= HBA - 1 < i0 + nb:
                    nc.gpsimd.collective_compute(
                        "AllGather", AL.bypass, groups,
                        [h1locA[:, :]], [h1fA[:, :]])

            nc.gpsimd.collective_compute(
                "AllGather", AL.bypass, groups, [h1locB[:, :]], [h1fB[:, :]])

            def edge_pass(pass_id, tiles, NG, gidx, dT, dR, base, abt, U, H,
                          UW, as_col, qoff, fblock):
                """One lo/hi pass over all dst blocks of one layer.
                tiles: [(blk, seg_first, seg_last)] per tile slot.
                base: gather base AP; abt: [128, NB, H] bf16 SBUF per-dst
                logits. U: [128, NB, UW] f32 SBUF accumulator (pre-init
                with the self-loop term). as_col: first table column holding
                the H per-src logits. fblock(blk): emitted when blk's U is
                complete (layer-boundary fusion)."""
                U_ps = None
                for g in range(NG):
                    it = ipool.tile([128, GC // 16], I16, tag="it")
                    nc.sync.dma_start(it[:], gidx[:, g * (GC // 16):(g + 1) * (GC // 16)])
                    dt = ipool.tile([128, TPG], BF16, tag="dt")
                    nc.sync.dma_start(dt[:], dT[:, g * TPG:(g + 1) * TPG])
                    # dst ids replicated to all partitions ([d, e] layout);
                    # issued on the Activation HWDGE queue to offload Sync
                    db = ipool.tile([128, GC], BF16, tag="db")
                    nc.scalar.dma_start(
                        db[:],
                        dR[0:1, g * GC:(g + 1) * GC].broadcast_to([128, GC]))
                    G = epool.tile([128, TPG, TW], BF16, tag="G")
                    nc.gpsimd.dma_gather(
                        G[:], base, it[:], GC, GC, TW,
                        queue_num=(g + qoff) % NQ)

                    # one-hot [e, d] and transposed [d, e] (pads match nothing)
                    OH = opool.tile([128, TPG, 128], F8, tag="OH")
                    nc.vector.tensor_tensor(
                        OH[:],
                        iotab[:].unsqueeze(1).broadcast_to([128, TPG, 128]),
                        dt[:].unsqueeze(2).broadcast_to([128, TPG, 128]),
                        AL.is_equal)
                    OHT = opool.tile([128, TPG, 128], F8, tag="OHT")
                    nc.vector.tensor_tensor(
                        OHT[:],
                        db[:].rearrange("p (t e) -> p t e", e=128),
                        iotapb[:].unsqueeze(2).broadcast_to([128, TPG, 128]),
                        AL.is_equal)

                    # ad_e = OHT^T @ ab_blk (ab resident in SBUF)
                    ad_ps = aps.tile([128, TPG, H], F32, tag="ad")
                    for t in range(TPG):
                        blk = tiles[g * TPG + t][0]
                        nc.tensor.matmul(
                            ad_ps[:, t:t + 1, :].rearrange("p a b -> p (a b)"),
                            OHT[:, t:t + 1, :].rearrange("p a b -> p (a b)"),
                            abt[:, blk, :])

                    # w = exp(leakyrelu(as_e + ad_e)); ACT writes w into msg
                    e = spool.tile([128, TPG, H], F32, tag="e")
                    nc.vector.tensor_add(
                        e[:], G[:, :, as_col:as_col + H], ad_ps[:])
                    lr = spool.tile([128, TPG * H], F32, tag="lr")
                    nc.vector.scalar_tensor_tensor(
                        lr[:], e[:].rearrange("p a b -> p (a b)"), 0.2,
                        e[:].rearrange("p a b -> p (a b)"), AL.mult, AL.max)
                    wt = spool.tile([128, TPG * H], BF16, tag="wt")
                    nc.scalar.activation(wt[:], lr[:], AF.Exp)
                    wb = wt[:].rearrange("p (t h) -> p t h", h=H)
                    msg = mpool.tile([128, TPG, UW], BF16, tag="msg")
                    nc.scalar.activation(
                        msg[:, :, UW - H:UW], lr[:].rearrange(
                            "p (t h) -> p t h", h=H), AF.Exp)
                    if H > 1:
                        nc.vector.tensor_mul(
                            msg[:, :, 0:CH].rearrange("p t (h c) -> p t h c", c=CH // H),
                            G[:, :, 0:CH].rearrange("p t (h c) -> p t h c", c=CH // H),
                            wb.unsqueeze(3).broadcast_to([128, TPG, H, CH // H]))
                    else:
                        nc.vector.tensor_mul(
                            msg[:, :, 0:UW - 1], G[:, :, 0:UW - 1],
                            wb.broadcast_to([128, TPG, UW - 1]))

                    # segment-sum via PE: U_ps[d, :] += OH^T @ msg
                    for t in range(TPG):
                        blk, sfirst, slast = tiles[g * TPG + t]
                        if sfirst:
                            U_ps = ups.tile([128, UW], F32, tag="ups")
                        nc.tensor.matmul(
                            U_ps[:],
                            OH[:, t:t + 1, :].rearrange("p a b -> p (a b)"),
                            msg[:, t:t + 1, :].rearrange("p a b -> p (a b)"),
                            start=sfirst, stop=slast)
                        if slast:
                            nc.vector.tensor_add(
                                U[:, blk, :], U[:, blk, :], U_ps[:])
                            if fblock is not None and last_pass[blk] == pass_id:
                                fblock(blk)
                return NG

            # ---- F1 (batched over runs of completed blocks, fused into the
            #      E1 tail): z = U/S + b1; ELU; h2 = mid @ W2p; table + ab2;
            #      U2 self-loop init ----
            def f1_batch(batch):
                i0, nb = batch[0], len(batch)
                ut = U1[:, i0:i0 + nb, :]
                sp = spool.tile([128, nb, 4], F32, tag=f"sp{nb}")
                nc.vector.tensor_scalar(out=sp[:], in0=ut[:, :, 64:68],
                                        scalar1=1e-16, scalar2=None, op0=AL.add)
                rec = spool.tile([128, nb, 4], F32, tag=f"rec{nb}")
                nc.vector.reciprocal(rec[:], sp[:])
                z = dpool.tile([128, nb, CH], F32, tag=f"z{nb}")
                nc.vector.tensor_mul(
                    z[:].rearrange("p t (h c) -> p t h c", c=16),
                    ut[:, :, 0:CH].rearrange("p t (h c) -> p t h c", c=16),
                    rec[:].unsqueeze(3).broadcast_to([128, nb, 4, 16]))
                nc.vector.tensor_add(
                    z[:], z[:], b1sb[:].unsqueeze(1).broadcast_to([128, nb, CH]))
                # ELU(z) = relu(z) + exp(min(z,0)) - 1
                r = dpool.tile([128, nb, CH], F32, tag=f"r{nb}")
                nc.scalar.activation(r[:], z[:], AF.Relu)
                u = dpool.tile([128, nb, CH], F32, tag=f"u{nb}")
                nc.vector.tensor_scalar(out=u[:], in0=z[:], scalar1=0.0,
                                        scalar2=None, op0=AL.min)
                tE = dpool.tile([128, nb, CH], F32, tag=f"tE{nb}")
                nc.scalar.activation(tE[:], u[:], AF.Exp)
                mid = dpool.tile([128, nb, CH], F32, tag=f"mid{nb}")
                nc.vector.scalar_tensor_tensor(
                    mid[:].rearrange("p a b -> p (a b)"),
                    tE[:].rearrange("p a b -> p (a b)"), -1.0,
                    r[:].rearrange("p a b -> p (a b)"), AL.add, AL.add)
                h2bb = dpool.tile([128, nb, TW], BF16, tag=f"h2bb{nb}")
                nc.vector.memset(h2bb[:, :, CH:TW], 0.0)
                for k, i in enumerate(batch):
                    tp2 = dps.tile([CH, 128], F32, tag="tp2")
                    nc.tensor.transpose(
                        tp2[:],
                        mid[:, k:k + 1, :].rearrange("p a b -> p (a b)"),
                        idn[:])
                    tps2 = dpool.tile([CH, 128], F32, tag="tps2")
                    nc.vector.tensor_copy(tps2[:], tp2[:])
                    ps2 = dps.tile([128, CH], F32, tag="mm")
                    nc.tensor.matmul(ps2[:], tps2[:], w2sb[:])
                    nc.vector.tensor_copy(h2bb[:, k, 0:CH], ps2[:])
                for k, i in enumerate(batch):
                    nc.sync.dma_start(hloc_slice(h2locA, h2locB, i),
                                      h2bb[:, k, :])
                nc.vector.tensor_copy(AB2[:, i0:i0 + nb, :], h2bb[:, :, 41:42])
                # self-loop init for U2 blocks from the same tiles
                e2s = spool.tile([128, nb], F32, tag=f"e2s{nb}")
                nc.vector.tensor_add(
                    e2s[:], h2bb[:, :, 40:41].rearrange("p a b -> p (a b)"),
                    h2bb[:, :, 41:42].rearrange("p a b -> p (a b)"))
                lr2s = spool.tile([128, nb], F32, tag=f"lr2s{nb}")
                nc.vector.scalar_tensor_tensor(
                    lr2s[:], e2s[:], 0.2, e2s[:], AL.mult, AL.max)
                w2s = spool.tile([128, nb], F32, tag=f"w2s{nb}")
                nc.scalar.activation(w2s[:], lr2s[:], AF.Exp)
                nc.vector.tensor_mul(
                    U2[:, i0:i0 + nb, 0:40], h2bb[:, :, 0:40],
                    w2s[:].unsqueeze(2).broadcast_to([128, nb, 40]))
                nc.vector.tensor_copy(U2[:, i0:i0 + nb, 40:41],
                                      w2s[:].unsqueeze(2))

            # ---- F2 (batched, fused into E2 tail): out = U2/S2 + b2 ----
            def f2_batch(batch):
                i0, nb = batch[0], len(batch)
                ut = U2[:, i0:i0 + nb, :]
                sp = spool.tile([128, nb], F32, tag=f"sp2{nb}")
                nc.vector.tensor_scalar(
                    out=sp[:], in0=ut[:, :, 40:41].rearrange("p a b -> p (a b)"),
                    scalar1=1e-16, scalar2=None, op0=AL.add)
                rec = spool.tile([128, nb], F32, tag=f"rec2{nb}")
                nc.vector.reciprocal(rec[:], sp[:])
                ot = dpool.tile([128, nb, 40], F32, tag=f"ot{nb}")
                nc.vector.tensor_mul(
                    ot[:], ut[:, :, 0:40],
                    rec[:].unsqueeze(2).broadcast_to([128, nb, 40]))
                nc.vector.tensor_add(
                    ot[:], ot[:], b2sb[:].unsqueeze(1).broadcast_to([128, nb, 40]))
                for k, i in enumerate(batch):
                    nc.sync.dma_start(OUT[i * 128:(i + 1) * 128, :],
                                      ot[:, k, :])

            def batcher(emit):
                batch = []

                def add(blk):
                    if batch and (blk != batch[-1] + 1 or len(batch) == 4):
                        emit(batch[:])
                        batch.clear()
                    batch.append(blk)

                def flush():
                    if batch:
                        emit(batch[:])
                        batch.clear()
                return add, flush

            # ---- E1 (f1 fused into the final pass per block; AllGather of
            #      h2 half A fires as soon as blocks 0..HBA-1 are done) ----
            f1_done = set()
            ag2a = [False]

            def f1_emit(batch):
                f1_batch(batch)
                f1_done.update(batch)
                if not ag2a[0] and all(b in f1_done for b in range(HBA)):
                    ag2a[0] = True
                    nc.gpsimd.collective_compute(
                        "AllGather", AL.bypass, groups,
                        [h2locA[:, :]], [h2fA[:, :]])

            f1_add, f1_flush = batcher(f1_emit)
            edge_pass(0, tiles_l, NGl, gl, dTl, dRl, h1fA[:, :], AB1,
                      U1, 4, 68, CH, 0, f1_add)
            edge_pass(1, tiles_h, NGh, gh, dTh, dRh, h1fB[:, :],
                      AB1, U1, 4, 68, CH, NGl, f1_add)
            f1_flush()
            assert ag2a[0]

            nc.gpsimd.collective_compute(
                "AllGather", AL.bypass, groups, [h2locB[:, :]], [h2fB[:, :]])

            # ---- E2 (as/ad folded into table cols 40/41; f2 fused) ----
            f2_add, f2_flush = batcher(f2_batch)
            edge_pass(0, tiles_l, NGl, gl, dTl, dRl, h2fA[:, :], AB2,
                      U2, 1, 41, 40, 0, f2_add)
            edge_pass(1, tiles_h, NGh, gh, dTh, dRh, h2fB[:, :],
                      AB2, U2, 1, 41, 40, NGl, f2_add)
            f2_flush()

            if debug:
                for i in range(NB):
                    r0 = i * 128
                    for hA, hB, dst_d in ((h1locA, h1locB, Dh1),
                                          (h2locA, h2locB, Dh2)):
                        tt = dpool.tile([128, TW], BF16, tag="dbg")
                        nc.sync.dma_start(tt[:], hloc_slice(hA, hB, i))
                        nc.sync.dma_start(dst_d[r0:r0 + 128, :], tt[:])
                    du1 = dpool.tile([128, 68], F32, tag="du1")
                    nc.vector.tensor_copy(du1[:], U1[:, i, :])
                    nc.sync.dma_start(DU1[r0:r0 + 128, :], du1[:])
                    du2 = dpool.tile([128, 41], F32, tag="du2")
                    nc.vector.tensor_copy(du2[:], U2[:, i, :])
                    nc.sync.dma_start(DU2[r0:r0 + 128, :], du2[:])

    nc.finalize()
    return nc


def _wrap_idx(a):
    """int16 [cap] -> wrapped [16, cap/16] replicated to [128, cap/16]."""
    w = a.reshape(-1, 16).T.copy()
    return np.ascontiguousarray(np.tile(w, (8, 1)))


def prep(x, edge_index, W1, a_src1, a_dst1, b1, W2, a_src2, a_dst2, b2):
    """Host-side sharding/index prep. Returns (meta, in_maps, (N, FIN))."""
    x = np.asarray(x, np.float32)
    N, FIN = x.shape
    NLOC = (N + NCORES - 1) // NCORES                       # 6250
    NLOCP = ((NLOC + 127) // 128) * 128                     # 6272
    NB = NLOCP // 128                                       # 49
    NPAD = NCORES * NLOCP
    NBINS = NCORES * NB

    ei0 = np.asarray(edge_index[0]).astype(np.int64)
    ei1 = np.asarray(edge_index[1]).astype(np.int64)

    # balanced bin-pack: nodes -> 392 (core, block) bins by in-degree,
    # so per-block edge counts are ~equal across cores and blocks
    deg = np.bincount(ei1, minlength=N)
    order = np.argsort(-deg, kind="stable")
    heap = [(0, b) for b in range(NBINS)]
    heapq.heapify(heap)
    count = np.zeros(NBINS, np.int64)
    newrow = np.empty(N, np.int64)          # node -> global padded row
    for n in order:
        while True:
            load, b = heapq.heappop(heap)
            if count[b] < 128:
                break
        newrow[n] = b * 128 + count[b]
        count[b] += 1
        heapq.heappush(heap, (load + int(deg[n]), b))

    # source rows in the two half-tables (each < 32768 rows: int16-safe)
    HA, HB = HBA * 128, (NB - HBA) * 128
    csrc = newrow[ei0] // NLOCP
    q = newrow[ei0] % NLOCP
    in_b = q >= HA
    rmap = np.where(in_b, csrc * HB + (q - HA), csrc * HA + q)
    drow = newrow[ei1]
    core = drow // NLOCP
    dl = drow % NLOCP
    blk_all = dl // 128
    dlm_all = dl % 128

    # per (core, pass): edges sorted by (dst-block, src-row)
    per = []  # [core][pass] = (rs_sorted, blk_sorted, dlm_sorted)
    for c in range(NCORES):
        m = core == c
        rs_c, blk_c, dlm_c, inb_c = rmap[m], blk_all[m], dlm_all[m], in_b[m]
        rows = []
        for p, sel in enumerate((~inb_c, inb_c)):
            rs, blk, dlm = rs_c[sel], blk_c[sel], dlm_c[sel]
            o = np.lexsort((rs, blk))
            rows.append((rs[o], blk[o], dlm[o]))
        per.append(rows)

    # uniform tiles-per-(pass, block) across cores
    TPB = np.zeros((2, NB), np.int64)
    for c in range(NCORES):
        for p in range(2):
            cnt = np.bincount(per[c][p][1], minlength=NB)
            TPB[p] = np.maximum(TPB[p], (cnt + 127) // 128)
    last_pass_for_blk = np.where(TPB[1] > 0, 1, 0).tolist()

    def pass_meta(p):
        bids = np.repeat(np.arange(NB), TPB[p])
        NG = (len(bids) + TPG - 1) // TPG
        pad = NG * TPG - len(bids)
        if pad:
            bids = np.concatenate([bids, np.full(pad, bids[-1])])
        first = np.ones(len(bids), bool)
        first[1:] = bids[1:] != bids[:-1]
        last = np.ones(len(bids), bool)
        last[:-1] = bids[1:] != bids[:-1]
        return NG, list(zip(bids.tolist(), first.tolist(), last.tolist()))

    NGl, tiles_l = pass_meta(0)
    NGh, tiles_h = pass_meta(1)
    slot_base = [np.concatenate([[0], np.cumsum(TPB[p]) * 128]) for p in range(2)]

    # ---- constant inputs (replicated) ----
    W1 = np.asarray(W1, np.float32)
    W2p = np.zeros((CH, CH), np.float32)
    W2p[:, :40] = np.asarray(W2, np.float32)
    W2p[:, 40] = W2p[:, :40] @ np.asarray(a_src2, np.float32).reshape(40)
    W2p[:, 41] = W2p[:, :40] @ np.asarray(a_dst2, np.float32).reshape(40)
    IOTAB = np.ascontiguousarray(np.tile(
        np.arange(128, dtype=np.float32)[None, :], (128, 1))).astype(
            ml_dtypes.bfloat16)
    IOTAPB = np.arange(128, dtype=np.float32)[:, None].astype(
        ml_dtypes.bfloat16)
    IDN = np.eye(128, dtype=np.float32)
    as1 = np.asarray(a_src1, np.float32).reshape(CH)
    ad1 = np.asarray(a_dst1, np.float32).reshape(CH)
    asrc1t = np.ascontiguousarray(np.tile(as1[None, :], (128, 1)))
    adst1r = np.ascontiguousarray(np.tile(ad1[None, :], (128, 1)))
    b1r = np.ascontiguousarray(
        np.tile(np.asarray(b1, np.float32)[None, :], (128, 1)))
    b2r = np.ascontiguousarray(
        np.tile(np.asarray(b2, np.float32)[None, :], (128, 1)))

    xpad = np.zeros((NPAD, FIN), np.float32)
    xpad[newrow] = x

    in_maps = []
    for c in range(NCORES):
        packs = []
        for p, NG in ((0, NGl), (1, NGh)):
            slots = NG * GC
            idx_arr = np.zeros(slots, np.int64)
            dloc_arr = np.full(slots, 999.0, np.float32)
            rs, blk, dlm = per[c][p]
            if len(blk):
                starts = np.concatenate([[0], np.cumsum(np.bincount(blk, minlength=NB))])
                rank = np.arange(len(blk)) - starts[blk]
                pos = slot_base[p][blk] + rank
                idx_arr[pos] = rs
                dloc_arr[pos] = dlm.astype(np.float32)
            packs.append((
                _wrap_idx(idx_arr.astype(np.int16)),
                np.ascontiguousarray(
                    dloc_arr.reshape(-1, 128).T).astype(ml_dtypes.bfloat16),
                dloc_arr[None, :].astype(ml_dtypes.bfloat16)))
        xT = np.ascontiguousarray(xpad[c * NLOCP:(c + 1) * NLOCP].T)
        in_maps.append({
            "xTs": xT, "W1": W1, "W2p": W2p, "IOTAB": IOTAB, "IOTAPB": IOTAPB,
            "IDN": IDN,
            "asrc1t": asrc1t, "adst1r": adst1r, "b1r": b1r, "b2r": b2r,
            "gl": packs[0][0], "dTl": packs[0][1], "dRl": packs[0][2],
            "gh": packs[1][0], "dTh": packs[1][1], "dRh": packs[1][2],
        })

    meta = {
        "NLOC": NLOC, "NLOCP": NLOCP, "NB": NB,
        "NGl": NGl, "NGh": NGh, "tiles_l": tiles_l, "tiles_h": tiles_h,
        "last_pass_for_blk": last_pass_for_blk,
        "newrow": newrow,
    }
    return meta, in_maps, (N, FIN)


def kernel(**inputs):
    global LAST_RESULTS
    meta, in_maps, (N, FIN) = prep(**inputs)
    NLOCP = meta["NLOCP"]
    key = (N, FIN, meta["NGl"], meta["NGh"],
           tuple(t[0] for t in meta["tiles_l"]),
           tuple(t[0] for t in meta["tiles_h"]))
    if key not in _prog_cache:
        _prog_cache[key] = _build(meta)
    nc = _prog_cache[key]

    want_trace = bool(os.environ.get("GAT_TRACE"))
    if want_trace:
        try:
            from antenv import axon_hooks  # noqa: F401
        except ImportError:
            want_trace = False
    res = run_bass_kernel_spmd(
        nc, in_maps, core_ids=list(range(NCORES)), trace=want_trace)
    LAST_RESULTS = res
    full = np.concatenate([res.results[c]["OUT"] for c in range(NCORES)], 0)
    return np.ascontiguousarray(full[meta["newrow"]])


# revision 48
# speedup vs baseline: 1.9003x; 1.0072x over previous
"""2-layer GAT on 8 Trainium2 NeuronCores (Bass/Tile, SPMD) — v5.

Strategy (edge-parallel, dst-sharded): destination nodes are bin-packed on
the host into 392 (core, block-of-128) bins balanced by in-degree, so every
block sees ~equal edge work (minimal tile padding, SPMD-uniform program).
Per core, edges are sorted by (src<32768 split, dst-block, src). Node tables
are bf16 [*, 128] rows (256B) with per-src attention logits folded into
spare columns, so one SWDGE gather per edge fetches everything src-side.
Per 128-edge tile the kernel
  - builds one-hot [edge x dst] and its transpose [dst x edge] on DVE
    (is_equal vs iota consts; dst ids arrive in both layouts, the [dst x
    edge] one via a partition-broadcast DMA),
  - gets per-edge dst logits ad_e with one small PE matmul (OHT^T @ ab_blk),
  - computes w = exp(leakyrelu(as_e + ad_e)) on DVE/ACT (ACT writes w
    straight into the msg tile),
  - segment-sums [w*h | w] into PSUM via one PE matmul with the one-hot as
    stationary (no dma_scatter_add RMW).
Self-loop edges never enter the edge stream: their contribution initializes
the per-block U/S SBUF accumulators element-wise during the dense phases.
Softmax is the U/S ratio so no per-edge normalization. Layer boundaries
fuse normalize+ELU+projection per block, interleaved into the tail edge
pass so the PE/DVE work overlaps the remaining gathers. 4 SWDGE queues
round-robin so descriptor generation overlaps DMA.
"""
import heapq
import os
import numpy as np
import ml_dtypes

from concourse import bacc, mybir, tile
from concourse.bass_utils import run_bass_kernel_spmd

NCORES = 8
CH = 64          # feature channels (L2 zero-padded 40->64); table rows 128
TW = 128         # table row width (bf16 -> 256B rows)
GC = int(os.environ.get("GAT_GC", "1024"))  # idxs per SWDGE gather call
TPG = GC // 128  # tiles per gather group
HBA = 25         # blocks per core in table half A (half B gets NB - HBA)
NQ = 4           # SWDGE queues, round-robin over gather calls
F32 = mybir.dt.float32
BF16 = mybir.dt.bfloat16
F8 = mybir.dt.float8e4
I16 = mybir.dt.int16
AL = mybir.AluOpType
AF = mybir.ActivationFunctionType

_prog_cache = {}
LAST_RESULTS = None  # BassKernelResults of the last device run (for test.py)


def _build(meta):
    NLOCP = meta["NLOCP"]
    NB = meta["NB"]
    NPAD = NCORES * NLOCP
    NGl, NGh = meta["NGl"], meta["NGh"]
    tiles_l, tiles_h = meta["tiles_l"], meta["tiles_h"]  # [(blk, first, last)]
    last_pass = meta["last_pass_for_blk"]
    SL, SH = NGl * GC, NGh * GC
    HA, HB = HBA * 128, (NB - HBA) * 128

    nc = bacc.Bacc(num_devices=NCORES, num_swdge_queues=NQ,
                   dynamic_dma_scratch_size=16 * GC)

    # ---- I/O ----
    xTs = nc.dram_tensor("xTs", [128, NLOCP], F32, kind="ExternalInput")
    W1 = nc.dram_tensor("W1", [128, CH], F32, kind="ExternalInput")
    W2p = nc.dram_tensor("W2p", [CH, CH], F32, kind="ExternalInput")
    IOTAB = nc.dram_tensor("IOTAB", [128, 128], BF16, kind="ExternalInput")
    IOTAPB = nc.dram_tensor("IOTAPB", [128, 1], BF16, kind="ExternalInput")
    IDN = nc.dram_tensor("IDN", [128, 128], F32, kind="ExternalInput")
    asrc1t = nc.dram_tensor("asrc1t", [128, CH], F32, kind="ExternalInput")
    adst1r = nc.dram_tensor("adst1r", [128, CH], F32, kind="ExternalInput")
    b1r = nc.dram_tensor("b1r", [128, CH], F32, kind="ExternalInput")
    b2r = nc.dram_tensor("b2r", [128, 40], F32, kind="ExternalInput")
    gl = nc.dram_tensor("gl", [128, SL // 16], I16, kind="ExternalInput")
    gh = nc.dram_tensor("gh", [128, SH // 16], I16, kind="ExternalInput")
    dTl = nc.dram_tensor("dTl", [128, SL // 128], BF16, kind="ExternalInput")
    dTh = nc.dram_tensor("dTh", [128, SH // 128], BF16, kind="ExternalInput")
    dRl = nc.dram_tensor("dRl", [1, SL], BF16, kind="ExternalInput")
    dRh = nc.dram_tensor("dRh", [1, SH], BF16, kind="ExternalInput")
    OUT = nc.dram_tensor("OUT", [NLOCP, 40], F32, kind="ExternalOutput")

    # ---- scratch ----
    h1locA = nc.dram_tensor("h1locA", [HA, TW], BF16, kind="Internal")
    h1locB = nc.dram_tensor("h1locB", [HB, TW], BF16, kind="Internal")
    h1fA = nc.dram_tensor("h1fA", [NCORES * HA, TW], BF16, kind="Internal",
                          addr_space="Shared")
    h1fB = nc.dram_tensor("h1fB", [NCORES * HB, TW], BF16, kind="Internal",
                          addr_space="Shared")
    h2locA = nc.dram_tensor("h2locA", [HA, TW], BF16, kind="Internal")
    h2locB = nc.dram_tensor("h2locB", [HB, TW], BF16, kind="Internal")
    h2fA = nc.dram_tensor("h2fA", [NCORES * HA, TW], BF16, kind="Internal",
                          addr_space="Shared")
    h2fB = nc.dram_tensor("h2fB", [NCORES * HB, TW], BF16, kind="Internal",
                          addr_space="Shared")

    def hloc_slice(hA, hB, i):
        return (hA[i * 128:(i + 1) * 128, :] if i < HBA
                else hB[(i - HBA) * 128:(i - HBA + 1) * 128, :])

    debug = bool(os.environ.get("GAT_DEBUG"))
    if debug:
        Dh1 = nc.dram_tensor("Dh1", [NLOCP, TW], BF16, kind="ExternalOutput")
        DU1 = nc.dram_tensor("DU1", [NLOCP, 68], F32, kind="ExternalOutput")
        Dh2 = nc.dram_tensor("Dh2", [NLOCP, TW], BF16, kind="ExternalOutput")
        DU2 = nc.dram_tensor("DU2", [NLOCP, 41], F32, kind="ExternalOutput")

    groups = [list(range(NCORES))]

    with tile.TileContext(nc) as tc:
        with (
            tc.tile_pool(name="const", bufs=1) as cpool,
            tc.tile_pool(name="dense", bufs=3) as dpool,
            tc.tile_pool(name="dpsum", bufs=1, space="PSUM") as dps,
            tc.tile_pool(name="apsum", bufs=2, space="PSUM") as aps,
            tc.tile_pool(name="upsum", bufs=3, space="PSUM") as ups,
            tc.tile_pool(name="idx", bufs=4) as ipool,
            tc.tile_pool(name="edge", bufs=5) as epool,
            tc.tile_pool(name="onehot", bufs=4) as opool,
            tc.tile_pool(name="msg", bufs=4) as mpool,
            tc.tile_pool(name="small", bufs=4) as spool,
        ):
            # constants
            w1sb = cpool.tile([128, CH], F32)
            nc.sync.dma_start(w1sb[:], W1[:])
            w2sb = cpool.tile([CH, CH], F32)
            nc.sync.dma_start(w2sb[:], W2p[:])
            iotab = cpool.tile([128, 128], BF16)
            nc.sync.dma_start(iotab[:], IOTAB[:])
            iotapb = cpool.tile([128, 1], BF16)
            nc.sync.dma_start(iotapb[:], IOTAPB[:])
            idn = cpool.tile([128, 128], F32)
            nc.sync.dma_start(idn[:], IDN[:])
            as1sb = cpool.tile([128, CH], F32)
            nc.sync.dma_start(as1sb[:], asrc1t[:])
            ad1sb = cpool.tile([128, CH], F32)
            nc.sync.dma_start(ad1sb[:], adst1r[:])
            b1sb = cpool.tile([128, CH], F32)
            nc.sync.dma_start(b1sb[:], b1r[:])
            b2sb = cpool.tile([128, 40], F32)
            nc.sync.dma_start(b2sb[:], b2r[:])

            # persistent per-block U/S accumulators (SBUF)
            U1 = cpool.tile([128, NB, 68], F32)
            U2 = cpool.tile([128, NB, 41], F32)

            # SBUF-resident per-dst attention tables (written by D1/F1,
            # read by the ad_e matmuls — no DRAM round-trip)
            AB1 = cpool.tile([128, NB, 4], BF16)
            AB2 = cpool.tile([128, NB, 1], BF16)

            # ---- D1 (batched x4): h1 = x @ W1; table row = [h1 | as | 0];
            #      AB1; U1 initialized with the self-loop contribution ----
            for i0 in range(0, NB, 4):
                nb = min(4, NB - i0)
                htf = dpool.tile([128, nb, CH], F32, tag=f"htf{nb}")
                for k in range(nb):
                    r0 = (i0 + k) * 128
                    xt = dpool.tile([128, 128], F32, tag="xt")
                    nc.sync.dma_start(xt[:], xTs[:, r0:r0 + 128])
                    ps = dps.tile([128, CH], F32, tag="mm")
                    nc.tensor.matmul(ps[:], xt[:], w1sb[:])
                    nc.vector.tensor_copy(htf[:, k, :], ps[:])
                htb = dpool.tile([128, nb, TW], BF16, tag=f"htb{nb}")
                nc.vector.tensor_copy(htb[:, :, 0:CH], htf[:])
                tmp = dpool.tile([128, nb, CH], F32, tag=f"tmp{nb}")
                nc.vector.tensor_mul(
                    tmp[:], htf[:],
                    as1sb[:].unsqueeze(1).broadcast_to([128, nb, CH]))
                asf = spool.tile([128, nb, 4], F32, tag=f"asf{nb}")
                nc.vector.tensor_reduce(
                    asf[:], tmp[:].rearrange("p t (h c) -> p t h c", c=16),
                    mybir.AxisListType.X, AL.add)
                nc.vector.tensor_copy(htb[:, :, CH:CH + 4], asf[:])
                nc.vector.memset(htb[:, :, CH + 4:TW], 0.0)
                tmp2 = dpool.tile([128, nb, CH], F32, tag=f"tmp2{nb}")
                nc.vector.tensor_mul(
                    tmp2[:], htf[:],
                    ad1sb[:].unsqueeze(1).broadcast_to([128, nb, CH]))
                dpf = spool.tile([128, nb, 4], F32, tag=f"dpf{nb}")
                nc.vector.tensor_reduce(
                    dpf[:], tmp2[:].rearrange("p t (h c) -> p t h c", c=16),
                    mybir.AxisListType.X, AL.add)
                nc.vector.tensor_copy(AB1[:, i0:i0 + nb, :], dpf[:])
                for k in range(nb):
                    nc.sync.dma_start(hloc_slice(h1locA, h1locB, i0 + k),
                                      htb[:, k, :])
                # self-loop: U1 = [w*h | w], w = exp(lrelu(as+ad))
                e0 = spool.tile([128, nb, 4], F32, tag=f"e0{nb}")
                nc.vector.tensor_add(e0[:], asf[:], dpf[:])
                lr0 = spool.tile([128, nb, 4], F32, tag=f"lr0{nb}")
                nc.vector.scalar_tensor_tensor(
                    lr0[:].rearrange("p a b -> p (a b)"),
                    e0[:].rearrange("p a b -> p (a b)"), 0.2,
                    e0[:].rearrange("p a b -> p (a b)"), AL.mult, AL.max)
                w0 = spool.tile([128, nb, 4], F32, tag=f"w0{nb}")
                nc.scalar.activation(w0[:], lr0[:], AF.Exp)
                nc.vector.tensor_mul(
                    U1[:, i0:i0 + nb, 0:CH].rearrange("p t (h c) -> p t h c", c=16),
                    htf[:].rearrange("p t (h c) -> p t h c", c=16),
                    w0[:].unsqueeze(3).broadcast_to([128, nb, 4, 16]))
                nc.vector.tensor_copy(U1[:, i0:i0 + nb, CH:CH + 4], w0[:])
                # chunked AllGather: half A ships while D1 computes half B
                if i0 <= HBA - 1 < i0 + nb:
                    nc.gpsimd.collective_compute(
                        "AllGather", AL.bypass, groups,
                        [h1locA[:, :]], [h1fA[:, :]])

            nc.gpsimd.collective_compute(
                "AllGather", AL.bypass, groups, [h1locB[:, :]], [h1fB[:, :]])

            def edge_pass(pass_id, tiles, NG, gidx, dT, dR, base, abt, U, H,
                          UW, as_col, qoff, fblock):
                """One lo/hi pass over all dst blocks of one layer.
                tiles: [(blk, seg_first, seg_last)] per tile slot.
                base: gather base AP; abt: [128, NB, H] bf16 SBUF per-dst
                logits. U: [128, NB, UW] f32 SBUF accumulator (pre-init
                with the self-loop term). as_col: first table column holding
                the H per-src logits. fblock(blk): emitted when blk's U is
                complete (layer-boundary fusion)."""
                U_ps = None
                for g in range(NG):
                    it = ipool.tile([128, GC // 16], I16, tag="it")
                    nc.sync.dma_start(it[:], gidx[:, g * (GC // 16):(g + 1) * (GC // 16)])
                    dt = ipool.tile([128, TPG], BF16, tag="dt")
                    nc.sync.dma_start(dt[:], dT[:, g * TPG:(g + 1) * TPG])
                    # dst ids replicated to all partitions ([d, e] layout);
                    # issued on the Activation HWDGE queue to offload Sync
                    db = ipool.tile([128, GC], BF16, tag="db")
                    nc.scalar.dma_start(
                        db[:],
                        dR[0:1, g * GC:(g + 1) * GC].broadcast_to([128, GC]))
                    G = epool.tile([128, TPG, TW], BF16, tag="G")
                    nc.gpsimd.dma_gather(
                        G[:], base, it[:], GC, GC, TW,
                        queue_num=(g + qoff) % NQ)

                    # one-hot [e, d] and transposed [d, e] (pads match nothing)
                    OH = opool.tile([128, TPG, 128], F8, tag="OH")
                    nc.vector.tensor_tensor(
                        OH[:],
                        iotab[:].unsqueeze(1).broadcast_to([128, TPG, 128]),
                        dt[:].unsqueeze(2).broadcast_to([128, TPG, 128]),
                        AL.is_equal)
                    OHT = opool.tile([128, TPG, 128], F8, tag="OHT")
                    nc.vector.tensor_tensor(
                        OHT[:],
                        db[:].rearrange("p (t e) -> p t e", e=128),
                        iotapb[:].unsqueeze(2).broadcast_to([128, TPG, 128]),
                        AL.is_equal)

                    # ad_e = OHT^T @ ab_blk (ab resident in SBUF)
                    ad_ps = aps.tile([128, TPG, H], F32, tag="ad")
                    for t in range(TPG):
                        blk = tiles[g * TPG + t][0]
                        nc.tensor.matmul(
                            ad_ps[:, t:t + 1, :].rearrange("p a b -> p (a b)"),
                            OHT[:, t:t + 1, :].rearrange("p a b -> p (a b)"),
                            abt[:, blk, :])

                    # w = exp(leakyrelu(as_e + ad_e)); ACT writes w into msg
                    e = spool.tile([128, TPG, H], F32, tag="e")
                    nc.vector.tensor_add(
                        e[:], G[:, :, as_col:as_col + H], ad_ps[:])
                    lr = spool.tile([128, TPG * H], F32, tag="lr")
                    nc.vector.scalar_tensor_tensor(
                        lr[:], e[:].rearrange("p a b -> p (a b)"), 0.2,
                        e[:].rearrange("p a b -> p (a b)"), AL.mult, AL.max)
                    wt = spool.tile([128, TPG * H], BF16, tag="wt")
                    nc.scalar.activation(wt[:], lr[:], AF.Exp)
                    wb = wt[:].rearrange("p (t h) -> p t h", h=H)
                    msg = mpool.tile([128, TPG, UW], BF16, tag="msg")
                    nc.scalar.activation(
                        msg[:, :, UW - H:UW], lr[:].rearrange(
                            "p (t h) -> p t h", h=H), AF.Exp)
                    if H > 1:
                        nc.vector.tensor_mul(
                            msg[:, :, 0:CH].rearrange("p t (h c) -> p t h c", c=CH // H),
                            G[:, :, 0:CH].rearrange("p t (h c) -> p t h c", c=CH // H),
                            wb.unsqueeze(3).broadcast_to([128, TPG, H, CH // H]))
                    else:
                        nc.vector.tensor_mul(
                            msg[:, :, 0:UW - 1], G[:, :, 0:UW - 1],
                            wb.broadcast_to([128, TPG, UW - 1]))

                    # segment-sum via PE: U_ps[d, :] += OH^T @ msg
                    for t in range(TPG):
                        blk, sfirst, slast = tiles[g * TPG + t]
                        if sfirst:
                            U_ps = ups.tile([128, UW], F32, tag="ups")
                        nc.tensor.matmul(
                            U_ps[:],
                            OH[:, t:t + 1, :].rearrange("p a b -> p (a b)"),
                            msg[:, t:t + 1, :].rearrange("p a b -> p (a b)"),
                            start=sfirst, stop=slast)
                        if slast:
                            nc.vector.tensor_add(
                                U[:, blk, :], U[:, blk, :], U_ps[:])
                            if fblock is not None and last_pass[blk] == pass_id:
                                fblock(blk)
                return NG

            # ---- F1 (batched over runs of completed blocks, fused into the
            #      E1 tail): z = U/S + b1; ELU; h2 = mid @ W2p; table + ab2;
            #      U2 self-loop init ----
            def f1_batch(batch):
                i0, nb = batch[0], len(batch)
                ut = U1[:, i0:i0 + nb, :]
                sp = spool.tile([128, nb, 4], F32, tag=f"sp{nb}")
                nc.vector.tensor_scalar(out=sp[:], in0=ut[:, :, 64:68],
                                        scalar1=1e-16, scalar2=None, op0=AL.add)
                rec = spool.tile([128, nb, 4], F32, tag=f"rec{nb}")
                nc.vector.reciprocal(rec[:], sp[:])
                z = dpool.tile([128, nb, CH], F32, tag=f"z{nb}")
                nc.vector.tensor_mul(
                    z[:].rearrange("p t (h c) -> p t h c", c=16),
                    ut[:, :, 0:CH].rearrange("p t (h c) -> p t h c", c=16),
                    rec[:].unsqueeze(3).broadcast_to([128, nb, 4, 16]))
                nc.vector.tensor_add(
                    z[:], z[:], b1sb[:].unsqueeze(1).broadcast_to([128, nb, CH]))
                # ELU(z) = relu(z) + exp(min(z,0)) - 1
                r = dpool.tile([128, nb, CH], F32, tag=f"r{nb}")
                nc.scalar.activation(r[:], z[:], AF.Relu)
                u = dpool.tile([128, nb, CH], F32, tag=f"u{nb}")
                nc.vector.tensor_scalar(out=u[:], in0=z[:], scalar1=0.0,
                                        scalar2=None, op0=AL.min)
                tE = dpool.tile([128, nb, CH], F32, tag=f"tE{nb}")
                nc.scalar.activation(tE[:], u[:], AF.Exp)
                mid = dpool.tile([128, nb, CH], F32, tag=f"mid{nb}")
                nc.vector.scalar_tensor_tensor(
                    mid[:].rearrange("p a b -> p (a b)"),
                    tE[:].rearrange("p a b -> p (a b)"), -1.0,
                    r[:].rearrange("p a b -> p (a b)"), AL.add, AL.add)
                h2bb = dpool.tile([128, nb, TW], BF16, tag=f"h2bb{nb}")
                nc.vector.memset(h2bb[:, :, CH:TW], 0.0)
                for k, i in enumerate(batch):
                    tp2 = dps.tile([CH, 128], F32, tag="tp2")
                    nc.tensor.transpose(
                        tp2[:],
                        mid[:, k:k + 1, :].rearrange("p a b -> p (a b)"),
                        idn[:])
                    tps2 = dpool.tile([CH, 128], F32, tag="tps2")
                    nc.vector.tensor_copy(tps2[:], tp2[:])
                    ps2 = dps.tile([128, CH], F32, tag="mm")
                    nc.tensor.matmul(ps2[:], tps2[:], w2sb[:])
                    nc.vector.tensor_copy(h2bb[:, k, 0:CH], ps2[:])
                for k, i in enumerate(batch):
                    nc.sync.dma_start(hloc_slice(h2locA, h2locB, i),
                                      h2bb[:, k, :])
                nc.vector.tensor_copy(AB2[:, i0:i0 + nb, :], h2bb[:, :, 41:42])
                # self-loop init for U2 blocks from the same tiles
                e2s = spool.tile([128, nb], F32, tag=f"e2s{nb}")
                nc.vector.tensor_add(
                    e2s[:], h2bb[:, :, 40:41].rearrange("p a b -> p (a b)"),
                    h2bb[:, :, 41:42].rearrange("p a b -> p (a b)"))
                lr2s = spool.tile([128, nb], F32, tag=f"lr2s{nb}")
                nc.vector.scalar_tensor_tensor(
                    lr2s[:], e2s[:], 0.2, e2s[:], AL.mult, AL.max)
                w2s = spool.tile([128, nb], F32, tag=f"w2s{nb}")
                nc.scalar.activation(w2s[:], lr2s[:], AF.Exp)
                nc.vector.tensor_mul(
                    U2[:, i0:i0 + nb, 0:40], h2bb[:, :, 0:40],
                    w2s[:].unsqueeze(2).broadcast_to([128, nb, 40]))
                nc.vector.tensor_copy(U2[:, i0:i0 + nb, 40:41],
                                      w2s[:].unsqueeze(2))

            # ---- F2 (batched, fused into E2 tail): out = U2/S2 + b2 ----
            def f2_batch(batch):
                i0, nb = batch[0], len(batch)
                ut = U2[:, i0:i0 + nb, :]
                sp = spool.tile([128, nb], F32, tag=f"sp2{nb}")
                nc.vector.tensor_scalar(
                    out=sp[:], in0=ut[:, :, 40:41].rearrange("p a b -> p (a b)"),
                    scalar1=1e-16, scalar2=None, op0=AL.add)
                rec = spool.tile([128, nb], F32, tag=f"rec2{nb}")
                nc.vector.reciprocal(rec[:], sp[:])
                ot = dpool.tile([128, nb, 40], F32, tag=f"ot{nb}")
                nc.vector.tensor_mul(
                    ot[:], ut[:, :, 0:40],
                    rec[:].unsqueeze(2).broadcast_to([128, nb, 40]))
                nc.vector.tensor_add(
                    ot[:], ot[:], b2sb[:].unsqueeze(1).broadcast_to([128, nb, 40]))
                for k, i in enumerate(batch):
                    nc.sync.dma_start(OUT[i * 128:(i + 1) * 128, :],
                                      ot[:, k, :])

            def batcher(emit):
                batch = []

                def add(blk):
                    if batch and (blk != batch[-1] + 1 or len(batch) == 8):
                        emit(batch[:])
                        batch.clear()
                    batch.append(blk)

                def flush():
                    if batch:
                        emit(batch[:])
                        batch.clear()
                return add, flush

            # ---- E1 (f1 fused into the final pass per block; AllGather of
            #      h2 half A fires as soon as blocks 0..HBA-1 are done) ----
            f1_done = set()
            ag2a = [False]

            def f1_emit(batch):
                f1_batch(batch)
                f1_done.update(batch)
                if not ag2a[0] and all(b in f1_done for b in range(HBA)):
                    ag2a[0] = True
                    nc.gpsimd.collective_compute(
                        "AllGather", AL.bypass, groups,
                        [h2locA[:, :]], [h2fA[:, :]])

            f1_add, f1_flush = batcher(f1_emit)
            edge_pass(0, tiles_l, NGl, gl, dTl, dRl, h1fA[:, :], AB1,
                      U1, 4, 68, CH, 0, f1_add)
            edge_pass(1, tiles_h, NGh, gh, dTh, dRh, h1fB[:, :],
                      AB1, U1, 4, 68, CH, NGl, f1_add)
            f1_flush()
            assert ag2a[0]

            nc.gpsimd.collective_compute(
                "AllGather", AL.bypass, groups, [h2locB[:, :]], [h2fB[:, :]])

            # ---- E2 (as/ad folded into table cols 40/41; f2 fused) ----
            f2_add, f2_flush = batcher(f2_batch)
            edge_pass(0, tiles_l, NGl, gl, dTl, dRl, h2fA[:, :], AB2,
                      U2, 1, 41, 40, 0, f2_add)
            edge_pass(1, tiles_h, NGh, gh, dTh, dRh, h2fB[:, :],
                      AB2, U2, 1, 41, 40, NGl, f2_add)
            f2_flush()

            if debug:
                for i in range(NB):
                    r0 = i * 128
                    for hA, hB, dst_d in ((h1locA, h1locB, Dh1),
                                          (h2locA, h2locB, Dh2)):
                        tt = dpool.tile([128, TW], BF16, tag="dbg")
                        nc.sync.dma_start(tt[:], hloc_slice(hA, hB, i))
                        nc.sync.dma_start(dst_d[r0:r0 + 128, :], tt[:])
                    du1 = dpool.tile([128, 68], F32, tag="du1")
                    nc.vector.tensor_copy(du1[:], U1[:, i, :])
                    nc.sync.dma_start(DU1[r0:r0 + 128, :], du1[:])
                    du2 = dpool.tile([128, 41], F32, tag="du2")
                    nc.vector.tensor_copy(du2[:], U2[:, i, :])
                    nc.sync.dma_start(DU2[r0:r0 + 128, :], du2[:])

    nc.finalize()
    return nc


def _wrap_idx(a):
    """int16 [cap] -> wrapped [16, cap/16] replicated to [128, cap/16]."""
    w = a.reshape(-1, 16).T.copy()
    return np.ascontiguousarray(np.tile(w, (8, 1)))


def prep(x, edge_index, W1, a_src1, a_dst1, b1, W2, a_src2, a_dst2, b2):
    """Host-side sharding/index prep. Returns (meta, in_maps, (N, FIN))."""
    x = np.asarray(x, np.float32)
    N, FIN = x.shape
    NLOC = (N + NCORES - 1) // NCORES                       # 6250
    NLOCP = ((NLOC + 127) // 128) * 128                     # 6272
    NB = NLOCP // 128                                       # 49
    NPAD = NCORES * NLOCP
    NBINS = NCORES * NB

    ei0 = np.asarray(edge_index[0]).astype(np.int64)
    ei1 = np.asarray(edge_index[1]).astype(np.int64)

    # balanced bin-pack: nodes -> 392 (core, block) bins by in-degree,
    # so per-block edge counts are ~equal across cores and blocks
    deg = np.bincount(ei1, minlength=N)
    order = np.argsort(-deg, kind="stable")
    heap = [(0, b) for b in range(NBINS)]
    heapq.heapify(heap)
    count = np.zeros(NBINS, np.int64)
    newrow = np.empty(N, np.int64)          # node -> global padded row
    for n in order:
        while True:
            load, b = heapq.heappop(heap)
            if count[b] < 128:
                break
        newrow[n] = b * 128 + count[b]
        count[b] += 1
        heapq.heappush(heap, (load + int(deg[n]), b))

    # source rows in the two half-tables (each < 32768 rows: int16-safe)
    HA, HB = HBA * 128, (NB - HBA) * 128
    csrc = newrow[ei0] // NLOCP
    q = newrow[ei0] % NLOCP
    in_b = q >= HA
    rmap = np.where(in_b, csrc * HB + (q - HA), csrc * HA + q)
    drow = newrow[ei1]
    core = drow // NLOCP
    dl = drow % NLOCP
    blk_all = dl // 128
    dlm_all = dl % 128

    # per (core, pass): edges sorted by (dst-block, src-row)
    per = []  # [core][pass] = (rs_sorted, blk_sorted, dlm_sorted)
    for c in range(NCORES):
        m = core == c
        rs_c, blk_c, dlm_c, inb_c = rmap[m], blk_all[m], dlm_all[m], in_b[m]
        rows = []
        for p, sel in enumerate((~inb_c, inb_c)):
            rs, blk, dlm = rs_c[sel], blk_c[sel], dlm_c[sel]
            o = np.lexsort((rs, blk))
            rows.append((rs[o], blk[o], dlm[o]))
        per.append(rows)

    # uniform tiles-per-(pass, block) across cores
    TPB = np.zeros((2, NB), np.int64)
    for c in range(NCORES):
        for p in range(2):
            cnt = np.bincount(per[c][p][1], minlength=NB)
            TPB[p] = np.maximum(TPB[p], (cnt + 127) // 128)
    last_pass_for_blk = np.where(TPB[1] > 0, 1, 0).tolist()

    def pass_meta(p):
        bids = np.repeat(np.arange(NB), TPB[p])
        NG = (len(bids) + TPG - 1) // TPG
        pad = NG * TPG - len(bids)
        if pad:
            bids = np.concatenate([bids, np.full(pad, bids[-1])])
        first = np.ones(len(bids), bool)
        first[1:] = bids[1:] != bids[:-1]
        last = np.ones(len(bids), bool)
        last[:-1] = bids[1:] != bids[:-1]
        return NG, list(zip(bids.tolist(), first.tolist(), last.tolist()))

    NGl, tiles_l = pass_meta(0)
    NGh, tiles_h = pass_meta(1)
    slot_base = [np.concatenate([[0], np.cumsum(TPB[p]) * 128]) for p in range(2)]

    # ---- constant inputs (replicated) ----
    W1 = np.asarray(W1, np.float32)
    W2p = np.zeros((CH, CH), np.float32)
    W2p[:, :40] = np.asarray(W2, np.float32)
    W2p[:, 40] = W2p[:, :40] @ np.asarray(a_src2, np.float32).reshape(40)
    W2p[:, 41] = W2p[:, :40] @ np.asarray(a_dst2, np.float32).reshape(40)
    IOTAB = np.ascontiguousarray(np.tile(
        np.arange(128, dtype=np.float32)[None, :], (128, 1))).astype(
            ml_dtypes.bfloat16)
    IOTAPB = np.arange(128, dtype=np.float32)[:, None].astype(
        ml_dtypes.bfloat16)
    IDN = np.eye(128, dtype=np.float32)
    as1 = np.asarray(a_src1, np.float32).reshape(CH)
    ad1 = np.asarray(a_dst1, np.float32).reshape(CH)
    asrc1t = np.ascontiguousarray(np.tile(as1[None, :], (128, 1)))
    adst1r = np.ascontiguousarray(np.tile(ad1[None, :], (128, 1)))
    b1r = np.ascontiguousarray(
        np.tile(np.asarray(b1, np.float32)[None, :], (128, 1)))
    b2r = np.ascontiguousarray(
        np.tile(np.asarray(b2, np.float32)[None, :], (128, 1)))

    xpad = np.zeros((NPAD, FIN), np.float32)
    xpad[newrow] = x

    in_maps = []
    for c in range(NCORES):
        packs = []
        for p, NG in ((0, NGl), (1, NGh)):
            slots = NG * GC
            idx_arr = np.zeros(slots, np.int64)
            dloc_arr = np.full(slots, 999.0, np.float32)
            rs, blk, dlm = per[c][p]
            if len(blk):
                starts = np.concatenate([[0], np.cumsum(np.bincount(blk, minlength=NB))])
                rank = np.arange(len(blk)) - starts[blk]
                pos = slot_base[p][blk] + rank
                idx_arr[pos] = rs
                dloc_arr[pos] = dlm.astype(np.float32)
            packs.append((
                _wrap_idx(idx_arr.astype(np.int16)),
                np.ascontiguousarray(
                    dloc_arr.reshape(-1, 128).T).astype(ml_dtypes.bfloat16),
                dloc_arr[None, :].astype(ml_dtypes.bfloat16)))
        xT = np.ascontiguousarray(xpad[c * NLOCP:(c + 1) * NLOCP].T)
        in_maps.append({
            "xTs": xT, "W1": W1, "W2p": W2p, "IOTAB": IOTAB, "IOTAPB": IOTAPB,
            "IDN": IDN,
            "asrc1t": asrc1t, "adst1r": adst1r, "b1r": b1r, "b2r": b2r,
            "gl": packs[0][0], "dTl": packs[0][1], "dRl": packs[0][2],
            "gh": packs[1][0], "dTh": packs[1][1], "dRh": packs[1][2],
        })

    meta = {
        "NLOC": NLOC, "NLOCP": NLOCP, "NB": NB,
        "NGl": NGl, "NGh": NGh, "tiles_l": tiles_l, "tiles_h": tiles_h,
        "last_pass_for_blk": last_pass_for_blk,
        "newrow": newrow,
    }
    return meta, in_maps, (N, FIN)


def kernel(**inputs):
    global LAST_RESULTS
    meta, in_maps, (N, FIN) = prep(**inputs)
    NLOCP = meta["NLOCP"]
    key = (N, FIN, meta["NGl"], meta["NGh"],
           tuple(t[0] for t in meta["tiles_l"]),
           tuple(t[0] for t in meta["tiles_h"]))
    if key not in _prog_cache:
        _prog_cache[key] = _build(meta)
    nc = _prog_cache[key]

    want_trace = bool(os.environ.get("GAT_TRACE"))
    if want_trace:
        try:
            from antenv import axon_hooks  # noqa: F401
        except ImportError:
            want_trace = False
    res = run_bass_kernel_spmd(
        nc, in_maps, core_ids=list(range(NCORES)), trace=want_trace)
    LAST_RESULTS = res
    full = np.concatenate([res.results[c]["OUT"] for c in range(NCORES)], 0)
    return np.ascontiguousarray(full[meta["newrow"]])
